# revision 10
# baseline (speedup 1.0000x reference)
"""MultiHeadAttention (B=4, S=2048, d_model=1024, H=16, dh=64) on 8 trn2 cores.

Sharding: core (b, g) = batch b in 0..3, head-group g in 0..1 (8 heads each).

v2 (causal path): fp8e4 + DoubleRow matmuls for QKV projections, AV, and the
output projection (0.5 cyc/row); scores stay fp32r with head-pair row packing.
Softmax exp split between ACT (true exp, fp8 out) and DVE (Schraudolph
affine-int8 trick -> bitcast fp8e4). Causal straddle tiles are handled with a
static tril fp8 multiply on a 128-wide band plus a memset of the fully-masked
prefix (replaces the full-tile gpsimd affine_select). Normalization: rowsum
rides the AV matmul as a 65th V column; 1/l via reciprocal_approx_fast +
gpsimd partition_broadcast; the normalize TT reads ctx straight from PSUM and
writes the fp8 DoubleRow-interleaved ctx tile for the output projection.
Output y is DMA'd directly from PSUM.

Dense-mask path: original fp32r kernel (unchanged).
"""
import sys
sys.path.insert(0, "/opt/trn_rl_repo")

import os
import numpy as np
import ml_dtypes

import concourse.bass as bass
import concourse.mybir as mybir
import concourse.tile as tile
from concourse import bacc
from concourse.bass_utils import run_bass_kernel_spmd

F32 = mybir.dt.float32
F32R = mybir.dt.float32r
F8 = mybir.dt.float8e4
I8 = mybir.dt.int8
AF = mybir.ActivationFunctionType
AL = mybir.AluOpType
PM = mybir.MatmulPerfMode

NP_F8 = ml_dtypes.float8_e4m3

INTERLEAVE = os.environ.get("KNOILV") != "1"
EXP_DVE8 = int(os.environ.get("KEXPDVE8", "3"))   # of 8 exp ops, how many on DVE
EXPB = float(os.environ.get("KEXPB", "16248.6"))  # Schraudolph int16/bf16 bias
BAND_GP = os.environ.get("KBANDGP", "1") == "1"   # tril band mask on gpsimd
Y_COPY = os.environ.get("KYCOPY") == "1"          # stage y in SBUF vs psum DMA

B, S, D, H, DH = 4, 2048, 1024, 16, 64
NC = 8
G = 2              # head groups (cores per batch)
HPC = H // G       # 8 heads per core
EH = HPC * DH      # 512
NQT = S // 512     # 4 q-tiles
NKT = S // 128     # 16 k-tiles
NKT2 = S // 256    # 8 k256-tiles
SCALE = 1.0 / np.sqrt(DH)
LOG2E = 1.4426950408889634

_cache = {}
MM_NAMES = {"sc0": set(), "sc1": set(), "av": set(), "pj": set(), "p3": set()}


# ======================================================================
# v2 causal/affine path
# ======================================================================

def _build_v2(k_needed, mixed):
    """k_needed[qt]: leading k128-tiles per q-tile (even). mixed: straddle set."""
    nc = bacc.Bacc("TRN2", target_bir_lowering=False, debug=False, num_devices=NC)

    BF = mybir.dt.bfloat16
    xqb_d = nc.dram_tensor("xqb", [128, 8, S], BF, kind="ExternalInput").ap()
    xkb_d = nc.dram_tensor("xkb", [128, 8, S], BF, kind="ExternalInput").ap()
    xvb_d = nc.dram_tensor("xvb", [128, 8, S], BF, kind="ExternalInput").ap()
    wqb_d = nc.dram_tensor("wqb", [128, 8, EH], BF, kind="ExternalInput").ap()
    wkb_d = nc.dram_tensor("wkb", [128, 8, EH], BF, kind="ExternalInput").ap()
    wvb_d = nc.dram_tensor("wvb", [128, 8, EH], BF, kind="ExternalInput").ap()
    wob_d = nc.dram_tensor("wob", [128, 4, D], BF, kind="ExternalInput").ap()
    bq_d = nc.dram_tensor("bq", [128, 4], F32, kind="ExternalInput").ap()
    bk_d = nc.dram_tensor("bk", [128, 4], F32, kind="ExternalInput").ap()
    bv_d = nc.dram_tensor("bv", [128, HPC, DH], F32, kind="ExternalInput").ap()
    tril_d = nc.dram_tensor("trilb", [128, 128], BF, kind="ExternalInput").ap()
    y_d = nc.dram_tensor("y", [S, D], mybir.dt.bfloat16, kind="ExternalOutput").ap()

    with tile.TileContext(nc) as tc:
        with nc.allow_low_precision(reason="bf16/f32r attention within 2e-2 gate"):
            _body_v2(nc, tc, k_needed, mixed,
                     xqb_d, xkb_d, xvb_d, wqb_d, wkb_d, wvb_d, wob_d,
                     bq_d, bk_d, bv_d, tril_d, y_d)
    nc.compile()
    return nc


def _body_v2(nc, tc, k_needed, mixed,
             xqb_d, xkb_d, xvb_d, wqb_d, wkb_d, wvb_d, wob_d,
             bq_d, bk_d, bv_d, tril_d, y_d):
    BF = mybir.dt.bfloat16
    I16 = mybir.dt.int16
    # Schraudolph exp in bf16 bit space: i16 = round(A16*s + B16); bitcast bf16
    A16 = float(SCALE * 128.0 * LOG2E)
    B16 = float(EXPB)

    pers_cm = tc.tile_pool(name="pers", bufs=1)
    pers = pers_cm.__enter__()
    KT = pers.tile([128, 4, S], F32R)             # [eh%128, et, t]
    V65 = pers.tile([128, NKT, HPC, 65], BF)      # [t%128, kk, h, e|1]
    wqb_t = pers.tile([128, 8, EH], BF)
    wkb_t = pers.tile([128, 8, EH], BF)
    wvb_t = pers.tile([128, 8, EH], BF)
    wob_t = pers.tile([128, 4, D], BF)
    bq_t = pers.tile([128, 4], F32)
    bk_t = pers.tile([128, 4], F32)
    bv_t = pers.tile([128, HPC, DH], F32)
    tril_t = pers.tile([128, 128], BF)
    nc.sync.dma_start(bq_t[:], bq_d)
    nc.sync.dma_start(bk_t[:], bk_d)
    nc.sync.dma_start(bv_t[:], bv_d)
    nc.sync.dma_start(tril_t[:], tril_d)
    nc.sync.dma_start(wqb_t[:], wqb_d)
    nc.sync.dma_start(wkb_t[:], wkb_d)
    nc.sync.dma_start(wvb_t[:], wvb_d)
    nc.vector.memset(V65[:, :, :, 64:65], 1.0)

    px_cm = tc.tile_pool(name="px", bufs=3)
    px = px_cm.__enter__()
    pq_cm = tc.tile_pool(name="pq", bufs=2)
    pq = pq_cm.__enter__()
    pcw_cm = tc.tile_pool(name="pcw", bufs=3)
    pcw = pcw_cm.__enter__()
    ppt_cm = tc.tile_pool(name="ppt", bufs=2)
    ppt = ppt_cm.__enter__()
    pnrm_cm = tc.tile_pool(name="pnrm", bufs=1)
    pnrm = pnrm_cm.__enter__()
    py_cm = tc.tile_pool(name="py", bufs=2)
    py = py_cm.__enter__()
    pp_cm = tc.tile_pool(name="pp", bufs=2, space="PSUM")
    pp = pp_cm.__enter__()
    psc_cm = tc.tile_pool(name="psc", bufs=2, space="PSUM")
    psc = psc_cm.__enter__()
    pav_cm = tc.tile_pool(name="pav", bufs=1, space="PSUM")
    pav = pav_cm.__enter__()

    qwin = {}    # tq -> [128, 4, 512] Q^T window tile (f32r)
    ctxw = {}    # qt -> [128, 2, 2, 512] fp8 interleaved ctx tile
    state = {}
    expctr = [0]

    # ---- projection chunks ----
    def qk_chunks(tq):
        def load():
            xq_t = px.tile([128, 8, 512], BF, tag="x", name=f"xq{tq}")
            nc.sync.dma_start(xq_t[:], xqb_d[:, :, tq * 512:(tq + 1) * 512])
            xk_t = px.tile([128, 8, 512], BF, tag="x", name=f"xk{tq}")
            nc.sync.dma_start(xk_t[:], xkb_d[:, :, tq * 512:(tq + 1) * 512])
            state["xq"], state["xk"] = xq_t, xk_t
            qwin[tq] = pq.tile([128, 4, 512], F32R, tag="qw", name=f"qw{tq}")
        load.mms = 0

        def mmgroup(et, kind):
            def f():
                w_t = wqb_t if kind == "q" else wkb_t
                x_t = state["xq" if kind == "q" else "xk"]
                ps_t = pp.tile([128, 512], F32, tag="pj", name=f"ps_{kind}{tq}_{et}")
                for dk in range(8):
                    mi = nc.tensor.matmul(ps_t[:],
                                          w_t[:, dk, et * 128:(et + 1) * 128],
                                          x_t[:, dk, :],
                                          start=(dk == 0), stop=(dk == 7))
                    MM_NAMES["pj"].add(mi.ins.name)
                if kind == "q":
                    nc.vector.tensor_tensor(
                        qwin[tq][:, et, :], ps_t[:],
                        bq_t[:, et:et + 1].to_broadcast([128, 512]), AL.add)
                else:
                    nc.vector.tensor_tensor(
                        KT[:, et, tq * 512:(tq + 1) * 512], ps_t[:],
                        bk_t[:, et:et + 1].to_broadcast([128, 512]), AL.add)
            f.mms = 4
            return f

        out = [load]
        for kind in ("q", "k"):
            for et in range(4):
                out.append(mmgroup(et, kind))
        return out

    def v_chunks(tv):
        def load():
            xv_t = px.tile([128, 8, 512], BF, tag="x", name=f"xv{tv}")
            nc.sync.dma_start(xv_t[:], xvb_d[:, :, tv * 512:(tv + 1) * 512])
            state["xv"] = xv_t
        load.mms = 0

        def mmgroup(tl):
            def f():
                x_t = state["xv"]
                tt = tv * 4 + tl
                ps_t = pp.tile([128, 512], F32, tag="pj", name=f"ps_v{tt}")
                for dk in range(8):
                    mi = nc.tensor.matmul(ps_t[:],
                                          x_t[:, dk, tl * 128:(tl + 1) * 128],
                                          wvb_t[:, dk, :],
                                          start=(dk == 0), stop=(dk == 7))
                    MM_NAMES["pj"].add(mi.ins.name)
                nc.vector.tensor_tensor(
                    V65[:, tt, :, 0:64],
                    ps_t.rearrange("p (h e) -> p h e", h=HPC),
                    bv_t[:], AL.add)
            f.mms = 4
            return f

        out = [load]
        for tl in range(4):
            out.append(mmgroup(tl))
        return out

    def p3_chunks(qt):
        p3state = {}

        def half_a(tl, mc):
            def f():
                cw = ctxw[qt]
                tt = qt * 4 + tl
                ps_t = pp.tile([128, 512], F32, tag="pj", name=f"ps_o{tt}_{mc}")
                p3state[(tl, mc)] = ps_t
                for hp in range(2):
                    mi = nc.tensor.matmul(ps_t[:],
                                          cw[:, hp, tl * 128:(tl + 1) * 128],
                                          wob_t[:, hp, mc * 512:(mc + 1) * 512],
                                          start=(hp == 0), stop=False)
                    MM_NAMES["p3"].add(mi.ins.name)
            f.mms = 2
            return f

        def half_b(tl, mc):
            def f():
                cw = ctxw[qt]
                tt = qt * 4 + tl
                ps_t = p3state.pop((tl, mc))
                for hp in range(2, 4):
                    mi = nc.tensor.matmul(ps_t[:],
                                          cw[:, hp, tl * 128:(tl + 1) * 128],
                                          wob_t[:, hp, mc * 512:(mc + 1) * 512],
                                          start=False, stop=(hp == 3))
                    MM_NAMES["p3"].add(mi.ins.name)
                y_t = py.tile([128, 512], BF, tag="y", name=f"y{tt}_{mc}")
                if Y_COPY:
                    nc.vector.tensor_copy(y_t[:], ps_t[:])
                else:
                    nc.scalar.copy(y_t[:], ps_t[:])
                nc.sync.dma_start(
                    y_d[tt * 128:(tt + 1) * 128, mc * 512:(mc + 1) * 512],
                    y_t[:])
            f.mms = 2
            return f

        out = []
        for tl in range(4):
            for mc in range(2):
                out.append(half_a(tl, mc))
                out.append(half_b(tl, mc))
        return out

    # ---- attention window ----
    def window(qt, work):
        if not INTERLEAVE:
            for f in work:
                f()
            work = []
        klim = k_needed[qt]          # in k128 units
        q0 = qt * 512
        n_units = 4 * klim
        cw = pcw.tile([128, 4, 512], BF, tag="cw", name=f"cw{qt}")
        ctxw[qt] = cw
        done = 0
        wi = 0
        total_mms = sum(getattr(f, "mms", 1) for f in work) or 1
        emitted = 0
        qw = qwin[qt]
        for hp in range(4):
            av = pav.tile([65, 2, 512], F32, tag="av")
            for kk in range(klim):
                straddle = (qt, kk) in mixed
                qoff = max(0, kk * 128 - q0) if straddle else 0
                moff = 128 if qoff >= 128 else 0
                s_t = psc.tile([128, 2, 512], F32, tag="sc")
                for j in range(2):
                    mi = nc.tensor.matmul(
                        s_t[:, j, moff:512],
                        KT[j * 64:(j + 1) * 64, hp, kk * 128:(kk + 1) * 128],
                        qw[j * 64:(j + 1) * 64, hp, moff:512],
                        start=True, stop=True, tile_position=(j * 64, 0))
                    MM_NAMES[f"sc{j}"].add(mi.ins.name)
                p_t = ppt.tile([128, 2, 512], BF, tag="pt")
                use_dve = (expctr[0] % 8) < EXP_DVE8
                expctr[0] += 1
                if use_dve:
                    p_i = p_t[:].bitcast(I16)
                    nc.vector.tensor_scalar(
                        p_i[:, :, qoff:512], s_t[:, :, qoff:512],
                        A16, B16, op0=AL.mult, op1=AL.add)
                else:
                    nc.scalar.activation(p_t[:, :, qoff:512],
                                         s_t[:, :, qoff:512],
                                         AF.Exp, scale=float(SCALE))
                if straddle:
                    if qoff > 0:
                        nc.gpsimd.memset(p_t[:, :, 0:qoff], 0)
                    eng = nc.gpsimd if BAND_GP else nc.vector
                    eng.tensor_tensor(
                        p_t[:, :, qoff:qoff + 128],
                        p_t[:, :, qoff:qoff + 128],
                        tril_t[:, None, :].to_broadcast([128, 2, 128]),
                        AL.mult)
                for j in range(2):
                    mi = nc.tensor.matmul(av[:, j, :],
                                          V65[:, kk, 2 * hp + j, :],
                                          p_t[:, j, :],
                                          start=(kk == 0), stop=(kk == klim - 1))
                    MM_NAMES["av"].add(mi.ins.name)
                done += 1
                if INTERLEAVE:
                    target = done * total_mms / n_units
                    while wi < len(work) and emitted < target:
                        emitted += getattr(work[wi], "mms", 1)
                        work[wi]()
                        wi += 1
            # ---- normalize: ctx * (1/l) -> bf16 ctx tile ----
            L2 = pnrm.tile([1, 2, 512], F32, tag="lin")
            nc.scalar.copy(L2[:], av[64:65, :, :])
            lrec = pnrm.tile([1, 2, 512], F32, tag="lrec")
            nc.vector.reciprocal_approx_fast(lrec[:], L2[:])
            rec_bc = pnrm.tile([128, 2, 512], F32, tag="rbc")
            nc.gpsimd.partition_broadcast(rec_bc[:], lrec[0:1, :, :])
            for j in range(2):
                nc.vector.tensor_tensor(
                    cw[j * 64:(j + 1) * 64, hp, :],
                    av[0:64, j, :],
                    rec_bc[j * 64:(j + 1) * 64, j, :], AL.mult)
        while wi < len(work):
            work[wi]()
            wi += 1

    # ---- prologue ----
    nc.sync.dma_start(wob_t[:], wob_d)
    for f in qk_chunks(0):
        f()
    for f in v_chunks(0):
        f()

    for qt in range(NQT):
        work = []
        if qt + 1 < NQT:
            work += qk_chunks(qt + 1)
            work += v_chunks(qt + 1)
        if qt == 2:
            work += p3_chunks(0)
        elif qt == 3:
            work += p3_chunks(1)
            work += p3_chunks(2)
        window(qt, work)
    for f in p3_chunks(NQT - 1):
        f()

    for cm in (pav_cm, psc_cm, pp_cm, py_cm, pnrm_cm, ppt_cm, pcw_cm, pq_cm,
               px_cm, pers_cm):
        cm.__exit__(None, None, None)


def _prep_inputs_v2(x_q, x_k, x_v, wq, wk, wv, bq, bk, bv, wo):
    f32 = np.float32
    bf16 = ml_dtypes.bfloat16
    trilb = np.triu(np.ones((128, 128), f32)).astype(bf16)
    in_maps = []

    def xb_of(x):
        # [128, 8, S]: xb[p, dk, t] = x[t, dk*128 + p]
        xr = np.asarray(x, f32).T.reshape(8, 128, S)
        return np.ascontiguousarray(xr.transpose(1, 0, 2)).astype(bf16)

    def wb_of(w, hs):
        wt = np.asarray(w[hs], f32).transpose(2, 0, 1).reshape(D, EH)
        return np.ascontiguousarray(
            wt.reshape(8, 128, EH).transpose(1, 0, 2)).astype(bf16)

    xq_c = {}
    for core in range(NC):
        b, g = divmod(core, G)
        hs = slice(g * HPC, (g + 1) * HPC)
        if b not in xq_c:
            xq_c[b] = (xb_of(x_q[b]), xb_of(x_k[b]), xb_of(x_v[b]))
        xqb, xkb, xvb = xq_c[b]
        im = {
            "xqb": xqb, "xkb": xkb, "xvb": xvb,
            "wqb": wb_of(wq, hs),
            "wkb": wb_of(wk, hs),
            "wvb": wb_of(wv, hs),
            "trilb": trilb,
        }
        woT = np.asarray(wo[:, g * EH:(g + 1) * EH], f32).T   # [EH, D]
        im["wob"] = np.ascontiguousarray(
            woT.reshape(4, 128, D).transpose(1, 0, 2)).astype(bf16)
        for name, bb in (("bq", bq), ("bk", bk)):
            flat = np.asarray(bb[hs], f32).reshape(EH)
            im[name] = np.ascontiguousarray(flat.reshape(4, 128).T)
        im["bv"] = np.broadcast_to(
            np.asarray(bv[hs], f32).reshape(1, HPC, DH), (128, HPC, DH)).copy()
        in_maps.append(im)
    return in_maps


# ======================================================================
# dense-mask fallback: original fp32r kernel
# ======================================================================

def _build_dense(k_needed, k_full, mixed):
    nc = bacc.Bacc("TRN2", target_bir_lowering=False, debug=False, num_devices=NC)

    xqT_d = nc.dram_tensor("xqT", [D, S], F32R, kind="ExternalInput").ap()
    xkT_d = nc.dram_tensor("xkT", [D, S], F32R, kind="ExternalInput").ap()
    xvT_d = nc.dram_tensor("xvT", [D, S], F32R, kind="ExternalInput").ap()
    wqT_d = nc.dram_tensor("wqT", [128, 8, EH], F32R, kind="ExternalInput").ap()
    wkT_d = nc.dram_tensor("wkT", [128, 8, EH], F32R, kind="ExternalInput").ap()
    wvT_d = nc.dram_tensor("wvT", [128, 8, EH], F32R, kind="ExternalInput").ap()
    bq_d = nc.dram_tensor("bq", [128, 4], F32, kind="ExternalInput").ap()
    bk_d = nc.dram_tensor("bk", [128, 4], F32, kind="ExternalInput").ap()
    bv_d = nc.dram_tensor("bv", [128, EH], F32, kind="ExternalInput").ap()
    woT_d = nc.dram_tensor("woT", [128, 4, D], F32R, kind="ExternalInput").ap()
    ones_d = nc.dram_tensor("ones1", [128, 1], F32R, kind="ExternalInput").ap()
    mT_d = nc.dram_tensor("maskT", [S, S], F32R, kind="ExternalInput").ap()
    mT_v = mT_d.rearrange("(kt p) q -> p kt q", p=128)
    y_d = nc.dram_tensor("y", [S, D], F32, kind="ExternalOutput").ap()

    xq_v = xqT_d.rearrange("(dk p) t -> p dk t", p=128)
    xk_v = xkT_d.rearrange("(dk p) t -> p dk t", p=128)
    xv_v = xvT_d.rearrange("(dk p) t -> p dk t", p=128)

    with tile.TileContext(nc) as tc:
        with nc.allow_low_precision(reason="fp32r storage has fp32 width"):
            _body_dense(nc, tc, k_needed, mixed,
                        xq_v, xk_v, xv_v, wqT_d, wkT_d, wvT_d,
                        bq_d, bk_d, bv_d, woT_d, ones_d, mT_v, y_d)
    nc.compile()
    return nc


def _body_dense(nc, tc, k_needed, mixed,
                xq_v, xk_v, xv_v, wqT_d, wkT_d, wvT_d,
                bq_d, bk_d, bv_d, woT_d, ones_d, mT_v, y_d):
    pers_cm = tc.tile_pool(name="pers", bufs=1)
    pers = pers_cm.__enter__()
    KT = pers.tile([128, 4, S], F32R)
    V65 = pers.tile([128, NKT, HPC, 65], F32R)
    wo_t = pers.tile([128, 4, D], F32R)
    wv_t = pers.tile([128, 8, EH], F32R)
    bq_t = pers.tile([128, 4], F32)
    bk_t = pers.tile([128, 4], F32)
    bv_t = pers.tile([128, EH], F32)
    ones_t = pers.tile([128, 1], F32R)
    nc.sync.dma_start(bq_t[:], bq_d)
    nc.sync.dma_start(bk_t[:], bk_d)
    nc.sync.dma_start(bv_t[:], bv_d)
    nc.sync.dma_start(ones_t[:], ones_d)
    nc.sync.dma_start(wv_t[:], wvT_d)
    nc.vector.tensor_copy(V65[:, :, :, 64:65],
                          ones_t[:, 0:1].to_broadcast([128, NKT, HPC, 1]))

    pw_cm = tc.tile_pool(name="pw", bufs=2)
    pw = pw_cm.__enter__()
    px_cm = tc.tile_pool(name="px", bufs=3)
    px = px_cm.__enter__()
    pq_cm = tc.tile_pool(name="pq", bufs=2)
    pq = pq_cm.__enter__()
    pcw_cm = tc.tile_pool(name="pcw", bufs=3)
    pcw = pcw_cm.__enter__()
    ppt_cm = tc.tile_pool(name="ppt", bufs=2)
    ppt = ppt_cm.__enter__()
    pnrm_cm = tc.tile_pool(name="pnrm", bufs=1)
    pnrm = pnrm_cm.__enter__()
    py_cm = tc.tile_pool(name="py", bufs=2)
    py = py_cm.__enter__()
    pp_cm = tc.tile_pool(name="pp", bufs=2, space="PSUM")
    pp = pp_cm.__enter__()
    psc_cm = tc.tile_pool(name="psc", bufs=2, space="PSUM")
    psc = psc_cm.__enter__()
    pav_cm = tc.tile_pool(name="pav", bufs=1, space="PSUM")
    pav = pav_cm.__enter__()

    qwin = {}
    ctxw = {}
    state = {}

    def qk_chunks(tq):
        def load(w_d, x_v, kind, half):
            def f():
                hs = slice(half * 4, half * 4 + 4)
                w_t = pw.tile([128, 4, EH], F32R, tag="w", name=f"w_{kind}{tq}{half}")
                nc.sync.dma_start(w_t[:], w_d[:, hs, :])
                x_t = px.tile([128, 4, 512], F32R, tag="x", name=f"x_{kind}{tq}{half}")
                nc.sync.dma_start(x_t[:], x_v[:, hs, tq * 512:(tq + 1) * 512])
                state[f"w{half}"], state[f"x{half}"] = w_t, x_t
                if kind == "q" and half == 0:
                    qwin[tq] = pq.tile([128, 4, 512], F32R, tag="qw", name=f"qw{tq}")
            return f

        def mmgroup(et, kind):
            def f():
                ps_t = pp.tile([128, 512], F32, tag="pj", name=f"ps_{kind}{tq}_{et}")
                for dk in range(8):
                    w_t = state[f"w{dk // 4}"]
                    x_t = state[f"x{dk // 4}"]
                    nc.tensor.matmul(ps_t[:],
                                     w_t[:, dk % 4, et * 128:(et + 1) * 128],
                                     x_t[:, dk % 4, :],
                                     start=(dk == 0), stop=(dk == 7))
                if kind == "q":
                    nc.vector.tensor_tensor(
                        qwin[tq][:, et, :], ps_t[:],
                        bq_t[:, et:et + 1].to_broadcast([128, 512]), AL.add)
                else:
                    nc.vector.tensor_tensor(
                        KT[:, et, tq * 512:(tq + 1) * 512], ps_t[:],
                        bk_t[:, et:et + 1].to_broadcast([128, 512]), AL.add)
            return f

        out = []
        for kind, w_d, x_v in (("q", wqT_d, xq_v), ("k", wkT_d, xk_v)):
            for half in (0, 1):
                g = load(w_d, x_v, kind, half)
                g.mms = 0
                out.append(g)
            for et in range(4):
                g = mmgroup(et, kind)
                g.mms = 8
                out.append(g)
        return out

    def v_chunks(tv):
        def load(half):
            def f():
                hs = slice(half * 4, half * 4 + 4)
                x_t = px.tile([128, 4, 512], F32R, tag="x", name=f"x_v{tv}{half}")
                nc.sync.dma_start(x_t[:], xv_v[:, hs, tv * 512:(tv + 1) * 512])
                state[f"x{half}"] = x_t
            return f

        def mmgroup(tl):
            def f():
                tt = tv * 4 + tl
                ps_t = pp.tile([128, 512], F32, tag="pj", name=f"ps_v{tt}")
                for dk in range(8):
                    x_t = state[f"x{dk // 4}"]
                    nc.tensor.matmul(ps_t[:],
                                     x_t[:, dk % 4, tl * 128:(tl + 1) * 128],
                                     wv_t[:, dk, :],
                                     start=(dk == 0), stop=(dk == 7))
                nc.vector.tensor_tensor(
                    V65[:, tt, :, 0:64],
                    ps_t.rearrange("p (h e) -> p h e", h=HPC),
                    bv_t.rearrange("p (h e) -> p h e", h=HPC), AL.add)
            return f

        out = []
        for half in (0, 1):
            g = load(half)
            g.mms = 0
            out.append(g)
        for tl in range(4):
            g = mmgroup(tl)
            g.mms = 8
            out.append(g)
        return out

    def p3_chunks(qt):
        p3state = {}

        def half_a(tl, mc):
            def f():
                cw = ctxw[qt]
                tt = qt * 4 + tl
                ps_t = pp.tile([128, 512], F32, tag="pj", name=f"ps_o{tt}_{mc}")
                p3state[(tl, mc)] = ps_t
                for hp in range(2):
                    nc.tensor.matmul(ps_t[:],
                                     cw[:, hp, tl * 128:(tl + 1) * 128],
                                     wo_t[:, hp, mc * 512:(mc + 1) * 512],
                                     start=(hp == 0), stop=False)
            f.mms = 2
            return f

        def half_b(tl, mc):
            def f():
                cw = ctxw[qt]
                tt = qt * 4 + tl
                ps_t = p3state.pop((tl, mc))
                for hp in range(2, 4):
                    nc.tensor.matmul(ps_t[:],
                                     cw[:, hp, tl * 128:(tl + 1) * 128],
                                     wo_t[:, hp, mc * 512:(mc + 1) * 512],
                                     start=False, stop=(hp == 3))
                y_t = py.tile([128, 512], F32, tag="y", name=f"y{tt}_{mc}")
                nc.vector.tensor_copy(y_t[:], ps_t[:])
                nc.sync.dma_start(
                    y_d[tt * 128:(tt + 1) * 128, mc * 512:(mc + 1) * 512],
                    y_t[:])
            f.mms = 2
            return f

        out = []
        for tl in range(4):
            for mc in range(2):
                out.append(half_a(tl, mc))
                out.append(half_b(tl, mc))
        return out

    def window(qt, work):
        if not INTERLEAVE:
            for f in work:
                f()
            work = []
        klim = k_needed[qt]
        q0 = qt * 512
        n_tiles = 4 * klim
        cw = pcw.tile([128, 4, 512], F32R, tag="cw", name=f"cw{qt}")
        ctxw[qt] = cw
        done = 0
        wi = 0
        total_mms = sum(getattr(f, "mms", 4) for f in work) or 1
        emitted = 0
        qw = qwin[qt]
        for hp in range(4):
            av0 = pav.tile([65, 512], F32, tag="av0")
            av1 = pav.tile([65, 512], F32, tag="av1")
            first = True
            for kk in range(klim):
                straddle = (qt, kk) in mixed
                s_t = psc.tile([128, 2, 512], F32, tag="sc")
                for j in range(2):
                    nc.tensor.matmul(
                        s_t[:, j, :],
                        KT[j * 64:(j + 1) * 64, hp, kk * 128:(kk + 1) * 128],
                        qw[j * 64:(j + 1) * 64, hp, :],
                        start=True, stop=True, tile_position=(j * 64, 0))
                p_t = ppt.tile([128, 2, 512], F32R, tag="pt")
                nc.scalar.activation(p_t[:], s_t[:], AF.Exp, scale=float(SCALE))
                if straddle:
                    sel_t = ppt.tile([128, 512], F32R, tag="sel")
                    nc.sync.dma_start(sel_t[:], mT_v[:, kk, q0:q0 + 512])
                    nc.vector.tensor_tensor(
                        p_t[:], p_t[:],
                        sel_t[:, None, :].to_broadcast([128, 2, 512]),
                        AL.mult)
                for j, av in ((0, av0), (1, av1)):
                    nc.tensor.matmul(av[:], V65[:, kk, 2 * hp + j, :],
                                     p_t[:, j, :],
                                     start=first, stop=(kk == klim - 1))
                first = False
                done += 1
                if INTERLEAVE:
                    target = done * total_mms / n_tiles
                    while wi < len(work) and emitted < target:
                        emitted += getattr(work[wi], "mms", 4)
                        work[wi]()
                        wi += 1
            avc = pnrm.tile([128, 512], F32, tag="avc", bufs=1)
            lin = pnrm.tile([1, 2, 512], F32, tag="lin")
            nc.vector.tensor_copy(avc[0:64, :], av0[0:64, :])
            nc.vector.tensor_copy(avc[64:128, :], av1[0:64, :])
            nc.vector.tensor_copy(lin[:, 0, :], av0[64:65, :])
            nc.vector.tensor_copy(lin[:, 1, :], av1[64:65, :])
            lrec = pnrm.tile([1, 2, 512], F32, tag="lrec")
            scr = pnrm.tile([1, 512], F32, tag="scr")
            nc.vector.reciprocal_approx_accurate(lrec[:, 0, :], lin[:, 0, :], scr[:])
            nc.vector.reciprocal_approx_accurate(lrec[:, 1, :], lin[:, 1, :], scr[:])
            rec_bc = pnrm.tile([128, 2, 512], F32, tag="rbc")
            nc.gpsimd.partition_broadcast(rec_bc[:], lrec[0:1, :, :])
            nc.vector.tensor_tensor(cw[0:64, hp, :],
                                    avc[0:64, :], rec_bc[0:64, 0, :], AL.mult)
            nc.vector.tensor_tensor(cw[64:128, hp, :],
                                    avc[64:128, :], rec_bc[64:128, 1, :], AL.mult)
        while wi < len(work):
            work[wi]()
            wi += 1

    for f in qk_chunks(0):
        f()
    for f in v_chunks(0):
        f()

    def wo_load():
        nc.sync.dma_start(wo_t[:], woT_d)
    wo_load.mms = 0

    for qt in range(NQT):
        work = []
        if qt == 0:
            work.append(wo_load)
        if qt + 1 < NQT:
            work += qk_chunks(qt + 1)
            work += v_chunks(qt + 1)
        if qt == 2:
            work += p3_chunks(0)
        elif qt == 3:
            work += p3_chunks(1)
            work += p3_chunks(2)
        window(qt, work)
    for f in p3_chunks(NQT - 1):
        f()

    for cm in (pav_cm, psc_cm, pp_cm, py_cm, pnrm_cm, ppt_cm, pcw_cm, pq_cm,
               px_cm, pw_cm, pers_cm):
        cm.__exit__(None, None, None)


def _prep_inputs_dense(x_q, x_k, x_v, mask, wq, wk, wv, bq, bk, bv, wo):
    f32 = np.float32
    in_maps = []
    ones1 = np.ones((128, 1), f32)
    for core in range(NC):
        b, g = divmod(core, G)
        hs = slice(g * HPC, (g + 1) * HPC)
        im = {
            "xqT": np.ascontiguousarray(np.asarray(x_q[b], f32).T),
            "xkT": np.ascontiguousarray(np.asarray(x_k[b], f32).T),
            "xvT": np.ascontiguousarray(np.asarray(x_v[b], f32).T),
            "ones1": ones1,
        }
        for name, w in (("wqT", wq), ("wkT", wk), ("wvT", wv)):
            wt = np.asarray(w[hs], f32).transpose(2, 0, 1).reshape(D, EH)
            im[name] = np.ascontiguousarray(wt.reshape(8, 128, EH))\
                .transpose(1, 0, 2).copy()
        for name, bb in (("bq", bq), ("bk", bk)):
            flat = np.asarray(bb[hs], f32).reshape(EH)
            im[name] = np.ascontiguousarray(flat.reshape(4, 128).T)
        im["bv"] = np.broadcast_to(np.asarray(bv[hs], f32).reshape(1, EH),
                                   (128, EH)).copy()
        woT = np.asarray(wo[:, g * EH:(g + 1) * EH], f32).T
        im["woT"] = np.ascontiguousarray(woT.reshape(4, 128, D))\
            .transpose(1, 0, 2).copy()
        im["maskT"] = np.ascontiguousarray(np.asarray(mask[b], f32).T)
        in_maps.append(im)
    return in_maps


# ======================================================================
# mask analysis + dispatch
# ======================================================================

def _analyze_mask(mask):
    m = np.asarray(mask)
    iota = np.arange(S)
    n = m.sum(axis=2)
    causal = bool((n == iota[None, :] + 1).all()) and \
        bool((m == (iota[None, None, :] < n[..., None])).all())
    allones = bool((m == 1).all())

    k_needed, k_full, mixed = [], [], set()
    if allones:
        mode = "affine"
        k_needed = [NKT] * NQT
        k_full = [NKT] * NQT
    elif causal:
        mode = "affine"
        for qt in range(NQT):
            k_needed.append(4 * qt + 4)
            k_full.append(4 * qt)
            for kk in range(4 * qt, 4 * qt + 4):
                mixed.add((qt, kk))
    else:
        mode = "dense"
        for qt in range(NQT):
            sl = m[:, qt * 512:(qt + 1) * 512, :]
            need = 0
            full = NKT
            for kk in range(NKT):
                blk = sl[:, :, kk * 128:(kk + 1) * 128]
                if blk.any():
                    need = kk + 1
                if not blk.all():
                    full = min(full, kk)
            need = max(need, 1)
            if mode == "dense":
                # keep k-tile count even for pairing safety (harmless extra)
                pass
            full = min(full, need)
            k_needed.append(need)
            k_full.append(full)
            for kk in range(full, need):
                blk = sl[:, :, kk * 128:(kk + 1) * 128]
                if not blk.all():
                    mixed.add((qt, kk))
    return mode, tuple(k_needed), tuple(k_full), frozenset(mixed)


def _run(x_q, x_k, x_v, mask, wq, wk, wv, bq, bk, bv, wo, bo,
         trace=False, trace_cores=None):
    mode, k_needed, k_full, mixed = _analyze_mask(mask)
    key = (mode, k_needed, k_full, mixed)
    if key not in _cache:
        if mode == "affine":
            _cache[key] = _build_v2(k_needed, mixed)
        else:
            _cache[key] = _build_dense(k_needed, k_full, mixed)
    nc = _cache[key]
    if mode == "affine":
        in_maps = _prep_inputs_v2(x_q, x_k, x_v, wq, wk, wv, bq, bk, bv, wo)
    else:
        in_maps = _prep_inputs_dense(x_q, x_k, x_v, mask, wq, wk, wv,
                                     bq, bk, bv, wo)
    res = run_bass_kernel_spmd(nc, in_maps, core_ids=list(range(NC)),
                               trace=trace, trace_cores=trace_cores)
    bo = np.asarray(bo, np.float32)
    out = np.empty((B, S, D), np.float32)
    for b in range(B):
        out[b] = (np.asarray(res.results[2 * b]["y"], np.float32)
                  + np.asarray(res.results[2 * b + 1]["y"], np.float32) + bo)
    return out, res


def kernel(x_q, x_k, x_v, mask, wq, wk, wv, bq, bk, bv, wo, bo):
    out, _ = _run(x_q, x_k, x_v, mask, wq, wk, wv, bq, bk, bv, wo, bo)
    return out


# revision 17
# speedup vs baseline: 1.0878x; 1.0878x over previous
"""MultiHeadAttention (B=4, S=2048, d_model=1024, H=16, dh=64) on 8 trn2 cores.

Sharding: core (b, g) = batch b in 0..3, head-group g in 0..1 (8 heads each).

v2 (causal path): fp8e4 + DoubleRow matmuls for QKV projections, AV, and the
output projection (0.5 cyc/row); scores stay fp32r with head-pair row packing.
Softmax exp split between ACT (true exp, fp8 out) and DVE (Schraudolph
affine-int8 trick -> bitcast fp8e4). Causal straddle tiles are handled with a
static tril fp8 multiply on a 128-wide band plus a memset of the fully-masked
prefix (replaces the full-tile gpsimd affine_select). Normalization: rowsum
rides the AV matmul as a 65th V column; 1/l via reciprocal_approx_fast +
gpsimd partition_broadcast; the normalize TT reads ctx straight from PSUM and
writes the fp8 DoubleRow-interleaved ctx tile for the output projection.
Output y is DMA'd directly from PSUM.

Dense-mask path: original fp32r kernel (unchanged).
"""
import sys
sys.path.insert(0, "/opt/trn_rl_repo")

import os
import numpy as np
import ml_dtypes

import concourse.bass as bass
import concourse.mybir as mybir
import concourse.tile as tile
from concourse import bacc
from concourse.bass_utils import run_bass_kernel_spmd

F32 = mybir.dt.float32
F32R = mybir.dt.float32r
F8 = mybir.dt.float8e4
I8 = mybir.dt.int8
AF = mybir.ActivationFunctionType
AL = mybir.AluOpType
PM = mybir.MatmulPerfMode

NP_F8 = ml_dtypes.float8_e4m3

INTERLEAVE = os.environ.get("KNOILV") != "1"
EXP_DVE8 = int(os.environ.get("KEXPDVE8", "3"))   # of 8 exp ops, how many on DVE
EXPB = float(os.environ.get("KEXPB", "16248.6"))  # Schraudolph int16/bf16 bias
BAND_GP = os.environ.get("KBANDGP", "1") == "1"   # tril band mask on gpsimd
Y_COPY = os.environ.get("KYCOPY") == "1"          # stage y in SBUF vs psum DMA

B, S, D, H, DH = 4, 2048, 1024, 16, 64
NC = 8
G = 2              # head groups (cores per batch)
HPC = H // G       # 8 heads per core
EH = HPC * DH      # 512
NQT = S // 512     # 4 q-tiles
NKT = S // 128     # 16 k-tiles
NKT2 = S // 256    # 8 k256-tiles
SCALE = 1.0 / np.sqrt(DH)
LOG2E = 1.4426950408889634

_cache = {}
MM_NAMES = {"sc0": set(), "sc1": set(), "av": set(), "pj": set(), "p3": set()}


# ======================================================================
# v2 causal/affine path
# ======================================================================

def _build_v2(k_needed, mixed):
    """k_needed[qt]: leading k128-tiles per q-tile (even). mixed: straddle set."""
    nc = bacc.Bacc("TRN2", target_bir_lowering=False, debug=False, num_devices=NC)

    BF = mybir.dt.bfloat16
    xqb_d = nc.dram_tensor("xqb", [128, 8, S], BF, kind="ExternalInput").ap()
    xkb_d = nc.dram_tensor("xkb", [128, 8, S], BF, kind="ExternalInput").ap()
    xvb_d = nc.dram_tensor("xvb", [128, 8, S], BF, kind="ExternalInput").ap()
    wqb_d = nc.dram_tensor("wqb", [128, 8, EH], BF, kind="ExternalInput").ap()
    wkb_d = nc.dram_tensor("wkb", [128, 8, EH], BF, kind="ExternalInput").ap()
    wvb_d = nc.dram_tensor("wvb", [128, 8, EH], BF, kind="ExternalInput").ap()
    wob_d = nc.dram_tensor("wob", [128, 4, D], BF, kind="ExternalInput").ap()
    bq_d = nc.dram_tensor("bq", [128, 4], F32, kind="ExternalInput").ap()
    bk_d = nc.dram_tensor("bk", [128, 4], F32, kind="ExternalInput").ap()
    bv_d = nc.dram_tensor("bv", [128, HPC, DH], F32, kind="ExternalInput").ap()
    tril_d = nc.dram_tensor("trilb", [128, 128], BF, kind="ExternalInput").ap()
    y_d = nc.dram_tensor("y", [S, D], mybir.dt.bfloat16, kind="ExternalOutput").ap()

    with tile.TileContext(nc) as tc:
        with nc.allow_low_precision(reason="bf16/f32r attention within 2e-2 gate"):
            _body_v2(nc, tc, k_needed, mixed,
                     xqb_d, xkb_d, xvb_d, wqb_d, wkb_d, wvb_d, wob_d,
                     bq_d, bk_d, bv_d, tril_d, y_d)
    nc.compile()
    return nc


def _body_v2(nc, tc, k_needed, mixed,
             xqb_d, xkb_d, xvb_d, wqb_d, wkb_d, wvb_d, wob_d,
             bq_d, bk_d, bv_d, tril_d, y_d):
    BF = mybir.dt.bfloat16
    I16 = mybir.dt.int16
    # Schraudolph exp in bf16 bit space: i16 = round(A16*s + B16); bitcast bf16
    A16 = float(SCALE * 128.0 * LOG2E)
    B16 = float(EXPB)

    pers_cm = tc.tile_pool(name="pers", bufs=1)
    pers = pers_cm.__enter__()
    KT = pers.tile([128, 4, S], F32R)             # [eh%128, et, t]
    V65 = pers.tile([128, NKT, HPC, 65], BF)      # [t%128, kk, h, e|1]
    wqb_t = pers.tile([128, 8, EH], BF)
    wkb_t = pers.tile([128, 8, EH], BF)
    wvb_t = pers.tile([128, 8, EH], BF)
    wob_t = pers.tile([128, 4, D], BF)
    bq_t = pers.tile([128, 4], F32)
    bk_t = pers.tile([128, 4], F32)
    bv_t = pers.tile([128, HPC, DH], F32)
    tril_t = pers.tile([128, 128], BF)
    nc.sync.dma_start(bq_t[:], bq_d)
    nc.sync.dma_start(bk_t[:], bk_d)
    nc.sync.dma_start(bv_t[:], bv_d)
    nc.sync.dma_start(tril_t[:], tril_d)
    nc.sync.dma_start(wqb_t[:], wqb_d)
    nc.sync.dma_start(wkb_t[:], wkb_d)
    nc.sync.dma_start(wvb_t[:], wvb_d)
    nc.vector.memset(V65[:, :, :, 64:65], 1.0)

    px_cm = tc.tile_pool(name="px", bufs=3)
    px = px_cm.__enter__()
    pq_cm = tc.tile_pool(name="pq", bufs=2)
    pq = pq_cm.__enter__()
    pcw_cm = tc.tile_pool(name="pcw", bufs=3)
    pcw = pcw_cm.__enter__()
    ppt_cm = tc.tile_pool(name="ppt", bufs=2)
    ppt = ppt_cm.__enter__()
    pnrm_cm = tc.tile_pool(name="pnrm", bufs=2)
    pnrm = pnrm_cm.__enter__()
    py_cm = tc.tile_pool(name="py", bufs=2)
    py = py_cm.__enter__()
    pp_cm = tc.tile_pool(name="pp", bufs=2, space="PSUM")
    pp = pp_cm.__enter__()
    psc_cm = tc.tile_pool(name="psc", bufs=2, space="PSUM")
    psc = psc_cm.__enter__()
    pav_cm = tc.tile_pool(name="pav", bufs=1, space="PSUM")
    pav = pav_cm.__enter__()

    qwin = {}    # tq -> [128, 4, 512] Q^T window tile (f32r)
    ctxw = {}    # qt -> [128, 2, 2, 512] fp8 interleaved ctx tile
    state = {}
    expctr = [0]

    # ---- projection chunks ----
    def qk_chunks(tq):
        def load():
            xq_t = px.tile([128, 8, 512], BF, tag="x", name=f"xq{tq}")
            nc.sync.dma_start(xq_t[:], xqb_d[:, :, tq * 512:(tq + 1) * 512])
            xk_t = px.tile([128, 8, 512], BF, tag="x", name=f"xk{tq}")
            nc.sync.dma_start(xk_t[:], xkb_d[:, :, tq * 512:(tq + 1) * 512])
            state["xq"], state["xk"] = xq_t, xk_t
            qwin[tq] = pq.tile([128, 4, 512], F32R, tag="qw", name=f"qw{tq}")
        load.mms = 0

        def mmgroup(et, kind):
            def f():
                w_t = wqb_t if kind == "q" else wkb_t
                x_t = state["xq" if kind == "q" else "xk"]
                ps_t = pp.tile([128, 512], F32, tag="pj", name=f"ps_{kind}{tq}_{et}")
                for dk in range(8):
                    mi = nc.tensor.matmul(ps_t[:],
                                          w_t[:, dk, et * 128:(et + 1) * 128],
                                          x_t[:, dk, :],
                                          start=(dk == 0), stop=(dk == 7))
                    MM_NAMES["pj"].add(mi.ins.name)
                if kind == "q":
                    nc.vector.tensor_tensor(
                        qwin[tq][:, et, :], ps_t[:],
                        bq_t[:, et:et + 1].to_broadcast([128, 512]), AL.add)
                else:
                    nc.vector.tensor_tensor(
                        KT[:, et, tq * 512:(tq + 1) * 512], ps_t[:],
                        bk_t[:, et:et + 1].to_broadcast([128, 512]), AL.add)
            f.mms = 4
            return f

        out = [load]
        for kind in ("q", "k"):
            for et in range(4):
                out.append(mmgroup(et, kind))
        return out

    def v_chunks(tv):
        def load():
            xv_t = px.tile([128, 8, 512], BF, tag="x", name=f"xv{tv}")
            nc.sync.dma_start(xv_t[:], xvb_d[:, :, tv * 512:(tv + 1) * 512])
            state["xv"] = xv_t
        load.mms = 0

        def mmgroup(tl):
            def f():
                x_t = state["xv"]
                tt = tv * 4 + tl
                ps_t = pp.tile([128, 512], F32, tag="pj", name=f"ps_v{tt}")
                for dk in range(8):
                    mi = nc.tensor.matmul(ps_t[:],
                                          x_t[:, dk, tl * 128:(tl + 1) * 128],
                                          wvb_t[:, dk, :],
                                          start=(dk == 0), stop=(dk == 7))
                    MM_NAMES["pj"].add(mi.ins.name)
                nc.vector.tensor_tensor(
                    V65[:, tt, :, 0:64],
                    ps_t.rearrange("p (h e) -> p h e", h=HPC),
                    bv_t[:], AL.add)
            f.mms = 4
            return f

        out = [load]
        for tl in range(4):
            out.append(mmgroup(tl))
        return out

    def p3_chunks(qt):
        p3state = {}

        def half_a(tl, mc):
            def f():
                cw = ctxw[qt]
                tt = qt * 4 + tl
                ps_t = pp.tile([128, 512], F32, tag="pj", name=f"ps_o{tt}_{mc}")
                p3state[(tl, mc)] = ps_t
                for hp in range(2):
                    mi = nc.tensor.matmul(ps_t[:],
                                          cw[:, hp, tl * 128:(tl + 1) * 128],
                                          wob_t[:, hp, mc * 512:(mc + 1) * 512],
                                          start=(hp == 0), stop=False)
                    MM_NAMES["p3"].add(mi.ins.name)
            f.mms = 2
            return f

        def half_b(tl, mc):
            def f():
                cw = ctxw[qt]
                tt = qt * 4 + tl
                ps_t = p3state.pop((tl, mc))
                for hp in range(2, 4):
                    mi = nc.tensor.matmul(ps_t[:],
                                          cw[:, hp, tl * 128:(tl + 1) * 128],
                                          wob_t[:, hp, mc * 512:(mc + 1) * 512],
                                          start=False, stop=(hp == 3))
                    MM_NAMES["p3"].add(mi.ins.name)
                y_t = py.tile([128, 512], BF, tag="y", name=f"y{tt}_{mc}")
                if Y_COPY:
                    nc.vector.tensor_copy(y_t[:], ps_t[:])
                else:
                    nc.scalar.copy(y_t[:], ps_t[:])
                nc.sync.dma_start(
                    y_d[tt * 128:(tt + 1) * 128, mc * 512:(mc + 1) * 512],
                    y_t[:])
            f.mms = 2
            return f

        out = []
        for tl in range(4):
            for mc in range(2):
                out.append(half_a(tl, mc))
                out.append(half_b(tl, mc))
        return out

    # ---- attention window ----
    def window(qt, work):
        if not INTERLEAVE:
            for f in work:
                f()
            work = []
        klim = k_needed[qt]          # in k128 units
        q0 = qt * 512
        n_units = 4 * klim
        cw = pcw.tile([128, 4, 512], BF, tag="cw", name=f"cw{qt}")
        ctxw[qt] = cw
        done = 0
        wi = 0
        total_mms = sum(getattr(f, "mms", 1) for f in work) or 1
        emitted = 0
        qw = qwin[qt]
        for hp in range(4):
            av = pav.tile([65, 2, 512], F32, tag="av")
            for kk in range(klim):
                straddle = (qt, kk) in mixed
                qoff = max(0, kk * 128 - q0) if straddle else 0
                moff = 128 if qoff >= 128 else 0
                s_t = psc.tile([128, 2, 512], F32, tag="sc")
                for j in range(2):
                    mi = nc.tensor.matmul(
                        s_t[:, j, moff:512],
                        KT[j * 64:(j + 1) * 64, hp, kk * 128:(kk + 1) * 128],
                        qw[j * 64:(j + 1) * 64, hp, moff:512],
                        start=True, stop=True, tile_position=(j * 64, 0))
                    MM_NAMES[f"sc{j}"].add(mi.ins.name)
                p_t = ppt.tile([128, 2, 512], BF, tag="pt")
                use_dve = (expctr[0] % 8) < EXP_DVE8
                expctr[0] += 1
                if use_dve:
                    p_i = p_t[:].bitcast(I16)
                    nc.vector.tensor_scalar(
                        p_i[:, :, qoff:512], s_t[:, :, qoff:512],
                        A16, B16, op0=AL.mult, op1=AL.add)
                else:
                    nc.scalar.activation(p_t[:, :, qoff:512],
                                         s_t[:, :, qoff:512],
                                         AF.Exp, scale=float(SCALE))
                if straddle:
                    if qoff > 0:
                        nc.gpsimd.memset(p_t[:, :, 0:qoff], 0)
                    eng = nc.gpsimd if BAND_GP else nc.vector
                    eng.tensor_tensor(
                        p_t[:, :, qoff:qoff + 128],
                        p_t[:, :, qoff:qoff + 128],
                        tril_t[:, None, :].to_broadcast([128, 2, 128]),
                        AL.mult)
                for j in range(2):
                    mi = nc.tensor.matmul(av[:, j, :],
                                          V65[:, kk, 2 * hp + j, :],
                                          p_t[:, j, :],
                                          start=(kk == 0), stop=(kk == klim - 1))
                    MM_NAMES["av"].add(mi.ins.name)
                done += 1
                if INTERLEAVE:
                    target = done * total_mms / n_units
                    while wi < len(work) and emitted < target:
                        emitted += getattr(work[wi], "mms", 1)
                        work[wi]()
                        wi += 1
            # ---- normalize: drain av psum fast, then ctx*(1/l) with slack ----
            avc = pnrm.tile([128, 512], F32, tag="avc")
            nc.vector.tensor_copy(avc[0:64, :], av[0:64, 0, :])
            nc.vector.tensor_copy(avc[64:128, :], av[0:64, 1, :])
            L2 = pnrm.tile([1, 2, 512], F32, tag="lin")
            nc.scalar.copy(L2[:], av[64:65, :, :])
            lrec = pnrm.tile([1, 2, 512], F32, tag="lrec")
            nc.vector.reciprocal_approx_fast(lrec[:], L2[:])
            rec_bc = pnrm.tile([128, 2, 512], F32, tag="rbc")
            nc.gpsimd.partition_broadcast(rec_bc[:], lrec[0:1, :, :])
            for j in range(2):
                nc.vector.tensor_tensor(
                    cw[j * 64:(j + 1) * 64, hp, :],
                    avc[j * 64:(j + 1) * 64, :],
                    rec_bc[j * 64:(j + 1) * 64, j, :], AL.mult)
        while wi < len(work):
            work[wi]()
            wi += 1

    # ---- prologue ----
    nc.sync.dma_start(wob_t[:], wob_d)
    for f in qk_chunks(0):
        f()
    for f in v_chunks(0):
        f()

    for qt in range(NQT):
        work = []
        if qt + 1 < NQT:
            work += qk_chunks(qt + 1)
            work += v_chunks(qt + 1)
        if qt == 2:
            work += p3_chunks(0)
        elif qt == 3:
            work += p3_chunks(1)
            work += p3_chunks(2)
        window(qt, work)
    for f in p3_chunks(NQT - 1):
        f()

    for cm in (pav_cm, psc_cm, pp_cm, py_cm, pnrm_cm, ppt_cm, pcw_cm, pq_cm,
               px_cm, pers_cm):
        cm.__exit__(None, None, None)


def _prep_inputs_v2(x_q, x_k, x_v, wq, wk, wv, bq, bk, bv, wo):
    f32 = np.float32
    bf16 = ml_dtypes.bfloat16
    trilb = np.triu(np.ones((128, 128), f32)).astype(bf16)
    in_maps = []

    def xb_of(x):
        # [128, 8, S]: xb[p, dk, t] = x[t, dk*128 + p]
        xr = np.asarray(x, f32).T.reshape(8, 128, S)
        return np.ascontiguousarray(xr.transpose(1, 0, 2)).astype(bf16)

    def wb_of(w, hs):
        wt = np.asarray(w[hs], f32).transpose(2, 0, 1).reshape(D, EH)
        return np.ascontiguousarray(
            wt.reshape(8, 128, EH).transpose(1, 0, 2)).astype(bf16)

    xq_c = {}
    for core in range(NC):
        b, g = divmod(core, G)
        hs = slice(g * HPC, (g + 1) * HPC)
        if b not in xq_c:
            xq_c[b] = (xb_of(x_q[b]), xb_of(x_k[b]), xb_of(x_v[b]))
        xqb, xkb, xvb = xq_c[b]
        im = {
            "xqb": xqb, "xkb": xkb, "xvb": xvb,
            "wqb": wb_of(wq, hs),
            "wkb": wb_of(wk, hs),
            "wvb": wb_of(wv, hs),
            "trilb": trilb,
        }
        woT = np.asarray(wo[:, g * EH:(g + 1) * EH], f32).T   # [EH, D]
        im["wob"] = np.ascontiguousarray(
            woT.reshape(4, 128, D).transpose(1, 0, 2)).astype(bf16)
        for name, bb in (("bq", bq), ("bk", bk)):
            flat = np.asarray(bb[hs], f32).reshape(EH)
            im[name] = np.ascontiguousarray(flat.reshape(4, 128).T)
        im["bv"] = np.broadcast_to(
            np.asarray(bv[hs], f32).reshape(1, HPC, DH), (128, HPC, DH)).copy()
        in_maps.append(im)
    return in_maps


# ======================================================================
# dense-mask fallback: original fp32r kernel
# ======================================================================

def _build_dense(k_needed, k_full, mixed):
    nc = bacc.Bacc("TRN2", target_bir_lowering=False, debug=False, num_devices=NC)

    xqT_d = nc.dram_tensor("xqT", [D, S], F32R, kind="ExternalInput").ap()
    xkT_d = nc.dram_tensor("xkT", [D, S], F32R, kind="ExternalInput").ap()
    xvT_d = nc.dram_tensor("xvT", [D, S], F32R, kind="ExternalInput").ap()
    wqT_d = nc.dram_tensor("wqT", [128, 8, EH], F32R, kind="ExternalInput").ap()
    wkT_d = nc.dram_tensor("wkT", [128, 8, EH], F32R, kind="ExternalInput").ap()
    wvT_d = nc.dram_tensor("wvT", [128, 8, EH], F32R, kind="ExternalInput").ap()
    bq_d = nc.dram_tensor("bq", [128, 4], F32, kind="ExternalInput").ap()
    bk_d = nc.dram_tensor("bk", [128, 4], F32, kind="ExternalInput").ap()
    bv_d = nc.dram_tensor("bv", [128, EH], F32, kind="ExternalInput").ap()
    woT_d = nc.dram_tensor("woT", [128, 4, D], F32R, kind="ExternalInput").ap()
    ones_d = nc.dram_tensor("ones1", [128, 1], F32R, kind="ExternalInput").ap()
    mT_d = nc.dram_tensor("maskT", [S, S], F32R, kind="ExternalInput").ap()
    mT_v = mT_d.rearrange("(kt p) q -> p kt q", p=128)
    y_d = nc.dram_tensor("y", [S, D], F32, kind="ExternalOutput").ap()

    xq_v = xqT_d.rearrange("(dk p) t -> p dk t", p=128)
    xk_v = xkT_d.rearrange("(dk p) t -> p dk t", p=128)
    xv_v = xvT_d.rearrange("(dk p) t -> p dk t", p=128)

    with tile.TileContext(nc) as tc:
        with nc.allow_low_precision(reason="fp32r storage has fp32 width"):
            _body_dense(nc, tc, k_needed, mixed,
                        xq_v, xk_v, xv_v, wqT_d, wkT_d, wvT_d,
                        bq_d, bk_d, bv_d, woT_d, ones_d, mT_v, y_d)
    nc.compile()
    return nc


def _body_dense(nc, tc, k_needed, mixed,
                xq_v, xk_v, xv_v, wqT_d, wkT_d, wvT_d,
                bq_d, bk_d, bv_d, woT_d, ones_d, mT_v, y_d):
    pers_cm = tc.tile_pool(name="pers", bufs=1)
    pers = pers_cm.__enter__()
    KT = pers.tile([128, 4, S], F32R)
    V65 = pers.tile([128, NKT, HPC, 65], F32R)
    wo_t = pers.tile([128, 4, D], F32R)
    wv_t = pers.tile([128, 8, EH], F32R)
    bq_t = pers.tile([128, 4], F32)
    bk_t = pers.tile([128, 4], F32)
    bv_t = pers.tile([128, EH], F32)
    ones_t = pers.tile([128, 1], F32R)
    nc.sync.dma_start(bq_t[:], bq_d)
    nc.sync.dma_start(bk_t[:], bk_d)
    nc.sync.dma_start(bv_t[:], bv_d)
    nc.sync.dma_start(ones_t[:], ones_d)
    nc.sync.dma_start(wv_t[:], wvT_d)
    nc.vector.tensor_copy(V65[:, :, :, 64:65],
                          ones_t[:, 0:1].to_broadcast([128, NKT, HPC, 1]))

    pw_cm = tc.tile_pool(name="pw", bufs=2)
    pw = pw_cm.__enter__()
    px_cm = tc.tile_pool(name="px", bufs=3)
    px = px_cm.__enter__()
    pq_cm = tc.tile_pool(name="pq", bufs=2)
    pq = pq_cm.__enter__()
    pcw_cm = tc.tile_pool(name="pcw", bufs=3)
    pcw = pcw_cm.__enter__()
    ppt_cm = tc.tile_pool(name="ppt", bufs=2)
    ppt = ppt_cm.__enter__()
    pnrm_cm = tc.tile_pool(name="pnrm", bufs=1)
    pnrm = pnrm_cm.__enter__()
    py_cm = tc.tile_pool(name="py", bufs=2)
    py = py_cm.__enter__()
    pp_cm = tc.tile_pool(name="pp", bufs=2, space="PSUM")
    pp = pp_cm.__enter__()
    psc_cm = tc.tile_pool(name="psc", bufs=2, space="PSUM")
    psc = psc_cm.__enter__()
    pav_cm = tc.tile_pool(name="pav", bufs=1, space="PSUM")
    pav = pav_cm.__enter__()

    qwin = {}
    ctxw = {}
    state = {}

    def qk_chunks(tq):
        def load(w_d, x_v, kind, half):
            def f():
                hs = slice(half * 4, half * 4 + 4)
                w_t = pw.tile([128, 4, EH], F32R, tag="w", name=f"w_{kind}{tq}{half}")
                nc.sync.dma_start(w_t[:], w_d[:, hs, :])
                x_t = px.tile([128, 4, 512], F32R, tag="x", name=f"x_{kind}{tq}{half}")
                nc.sync.dma_start(x_t[:], x_v[:, hs, tq * 512:(tq + 1) * 512])
                state[f"w{half}"], state[f"x{half}"] = w_t, x_t
                if kind == "q" and half == 0:
                    qwin[tq] = pq.tile([128, 4, 512], F32R, tag="qw", name=f"qw{tq}")
            return f

        def mmgroup(et, kind):
            def f():
                ps_t = pp.tile([128, 512], F32, tag="pj", name=f"ps_{kind}{tq}_{et}")
                for dk in range(8):
                    w_t = state[f"w{dk // 4}"]
                    x_t = state[f"x{dk // 4}"]
                    nc.tensor.matmul(ps_t[:],
                                     w_t[:, dk % 4, et * 128:(et + 1) * 128],
                                     x_t[:, dk % 4, :],
                                     start=(dk == 0), stop=(dk == 7))
                if kind == "q":
                    nc.vector.tensor_tensor(
                        qwin[tq][:, et, :], ps_t[:],
                        bq_t[:, et:et + 1].to_broadcast([128, 512]), AL.add)
                else:
                    nc.vector.tensor_tensor(
                        KT[:, et, tq * 512:(tq + 1) * 512], ps_t[:],
                        bk_t[:, et:et + 1].to_broadcast([128, 512]), AL.add)
            return f

        out = []
        for kind, w_d, x_v in (("q", wqT_d, xq_v), ("k", wkT_d, xk_v)):
            for half in (0, 1):
                g = load(w_d, x_v, kind, half)
                g.mms = 0
                out.append(g)
            for et in range(4):
                g = mmgroup(et, kind)
                g.mms = 8
                out.append(g)
        return out

    def v_chunks(tv):
        def load(half):
            def f():
                hs = slice(half * 4, half * 4 + 4)
                x_t = px.tile([128, 4, 512], F32R, tag="x", name=f"x_v{tv}{half}")
                nc.sync.dma_start(x_t[:], xv_v[:, hs, tv * 512:(tv + 1) * 512])
                state[f"x{half}"] = x_t
            return f

        def mmgroup(tl):
            def f():
                tt = tv * 4 + tl
                ps_t = pp.tile([128, 512], F32, tag="pj", name=f"ps_v{tt}")
                for dk in range(8):
                    x_t = state[f"x{dk // 4}"]
                    nc.tensor.matmul(ps_t[:],
                                     x_t[:, dk % 4, tl * 128:(tl + 1) * 128],
                                     wv_t[:, dk, :],
                                     start=(dk == 0), stop=(dk == 7))
                nc.vector.tensor_tensor(
                    V65[:, tt, :, 0:64],
                    ps_t.rearrange("p (h e) -> p h e", h=HPC),
                    bv_t.rearrange("p (h e) -> p h e", h=HPC), AL.add)
            return f

        out = []
        for half in (0, 1):
            g = load(half)
            g.mms = 0
            out.append(g)
        for tl in range(4):
            g = mmgroup(tl)
            g.mms = 8
            out.append(g)
        return out

    def p3_chunks(qt):
        p3state = {}

        def half_a(tl, mc):
            def f():
                cw = ctxw[qt]
                tt = qt * 4 + tl
                ps_t = pp.tile([128, 512], F32, tag="pj", name=f"ps_o{tt}_{mc}")
                p3state[(tl, mc)] = ps_t
                for hp in range(2):
                    nc.tensor.matmul(ps_t[:],
                                     cw[:, hp, tl * 128:(tl + 1) * 128],
                                     wo_t[:, hp, mc * 512:(mc + 1) * 512],
                                     start=(hp == 0), stop=False)
            f.mms = 2
            return f

        def half_b(tl, mc):
            def f():
                cw = ctxw[qt]
                tt = qt * 4 + tl
                ps_t = p3state.pop((tl, mc))
                for hp in range(2, 4):
                    nc.tensor.matmul(ps_t[:],
                                     cw[:, hp, tl * 128:(tl + 1) * 128],
                                     wo_t[:, hp, mc * 512:(mc + 1) * 512],
                                     start=False, stop=(hp == 3))
                y_t = py.tile([128, 512], F32, tag="y", name=f"y{tt}_{mc}")
                nc.vector.tensor_copy(y_t[:], ps_t[:])
                nc.sync.dma_start(
                    y_d[tt * 128:(tt + 1) * 128, mc * 512:(mc + 1) * 512],
                    y_t[:])
            f.mms = 2
            return f

        out = []
        for tl in range(4):
            for mc in range(2):
                out.append(half_a(tl, mc))
                out.append(half_b(tl, mc))
        return out

    def window(qt, work):
        if not INTERLEAVE:
            for f in work:
                f()
            work = []
        klim = k_needed[qt]
        q0 = qt * 512
        n_tiles = 4 * klim
        cw = pcw.tile([128, 4, 512], F32R, tag="cw", name=f"cw{qt}")
        ctxw[qt] = cw
        done = 0
        wi = 0
        total_mms = sum(getattr(f, "mms", 4) for f in work) or 1
        emitted = 0
        qw = qwin[qt]
        for hp in range(4):
            av0 = pav.tile([65, 512], F32, tag="av0")
            av1 = pav.tile([65, 512], F32, tag="av1")
            first = True
            for kk in range(klim):
                straddle = (qt, kk) in mixed
                s_t = psc.tile([128, 2, 512], F32, tag="sc")
                for j in range(2):
                    nc.tensor.matmul(
                        s_t[:, j, :],
                        KT[j * 64:(j + 1) * 64, hp, kk * 128:(kk + 1) * 128],
                        qw[j * 64:(j + 1) * 64, hp, :],
                        start=True, stop=True, tile_position=(j * 64, 0))
                p_t = ppt.tile([128, 2, 512], F32R, tag="pt")
                nc.scalar.activation(p_t[:], s_t[:], AF.Exp, scale=float(SCALE))
                if straddle:
                    sel_t = ppt.tile([128, 512], F32R, tag="sel")
                    nc.sync.dma_start(sel_t[:], mT_v[:, kk, q0:q0 + 512])
                    nc.vector.tensor_tensor(
                        p_t[:], p_t[:],
                        sel_t[:, None, :].to_broadcast([128, 2, 512]),
                        AL.mult)
                for j, av in ((0, av0), (1, av1)):
                    nc.tensor.matmul(av[:], V65[:, kk, 2 * hp + j, :],
                                     p_t[:, j, :],
                                     start=first, stop=(kk == klim - 1))
                first = False
                done += 1
                if INTERLEAVE:
                    target = done * total_mms / n_tiles
                    while wi < len(work) and emitted < target:
                        emitted += getattr(work[wi], "mms", 4)
                        work[wi]()
                        wi += 1
            avc = pnrm.tile([128, 512], F32, tag="avc", bufs=1)
            lin = pnrm.tile([1, 2, 512], F32, tag="lin")
            nc.vector.tensor_copy(avc[0:64, :], av0[0:64, :])
            nc.vector.tensor_copy(avc[64:128, :], av1[0:64, :])
            nc.vector.tensor_copy(lin[:, 0, :], av0[64:65, :])
            nc.vector.tensor_copy(lin[:, 1, :], av1[64:65, :])
            lrec = pnrm.tile([1, 2, 512], F32, tag="lrec")
            scr = pnrm.tile([1, 512], F32, tag="scr")
            nc.vector.reciprocal_approx_accurate(lrec[:, 0, :], lin[:, 0, :], scr[:])
            nc.vector.reciprocal_approx_accurate(lrec[:, 1, :], lin[:, 1, :], scr[:])
            rec_bc = pnrm.tile([128, 2, 512], F32, tag="rbc")
            nc.gpsimd.partition_broadcast(rec_bc[:], lrec[0:1, :, :])
            nc.vector.tensor_tensor(cw[0:64, hp, :],
                                    avc[0:64, :], rec_bc[0:64, 0, :], AL.mult)
            nc.vector.tensor_tensor(cw[64:128, hp, :],
                                    avc[64:128, :], rec_bc[64:128, 1, :], AL.mult)
        while wi < len(work):
            work[wi]()
            wi += 1

    for f in qk_chunks(0):
        f()
    for f in v_chunks(0):
        f()

    def wo_load():
        nc.sync.dma_start(wo_t[:], woT_d)
    wo_load.mms = 0

    for qt in range(NQT):
        work = []
        if qt == 0:
            work.append(wo_load)
        if qt + 1 < NQT:
            work += qk_chunks(qt + 1)
            work += v_chunks(qt + 1)
        if qt == 2:
            work += p3_chunks(0)
        elif qt == 3:
            work += p3_chunks(1)
            work += p3_chunks(2)
        window(qt, work)
    for f in p3_chunks(NQT - 1):
        f()

    for cm in (pav_cm, psc_cm, pp_cm, py_cm, pnrm_cm, ppt_cm, pcw_cm, pq_cm,
               px_cm, pw_cm, pers_cm):
        cm.__exit__(None, None, None)


def _prep_inputs_dense(x_q, x_k, x_v, mask, wq, wk, wv, bq, bk, bv, wo):
    f32 = np.float32
    in_maps = []
    ones1 = np.ones((128, 1), f32)
    for core in range(NC):
        b, g = divmod(core, G)
        hs = slice(g * HPC, (g + 1) * HPC)
        im = {
            "xqT": np.ascontiguousarray(np.asarray(x_q[b], f32).T),
            "xkT": np.ascontiguousarray(np.asarray(x_k[b], f32).T),
            "xvT": np.ascontiguousarray(np.asarray(x_v[b], f32).T),
            "ones1": ones1,
        }
        for name, w in (("wqT", wq), ("wkT", wk), ("wvT", wv)):
            wt = np.asarray(w[hs], f32).transpose(2, 0, 1).reshape(D, EH)
            im[name] = np.ascontiguousarray(wt.reshape(8, 128, EH))\
                .transpose(1, 0, 2).copy()
        for name, bb in (("bq", bq), ("bk", bk)):
            flat = np.asarray(bb[hs], f32).reshape(EH)
            im[name] = np.ascontiguousarray(flat.reshape(4, 128).T)
        im["bv"] = np.broadcast_to(np.asarray(bv[hs], f32).reshape(1, EH),
                                   (128, EH)).copy()
        woT = np.asarray(wo[:, g * EH:(g + 1) * EH], f32).T
        im["woT"] = np.ascontiguousarray(woT.reshape(4, 128, D))\
            .transpose(1, 0, 2).copy()
        im["maskT"] = np.ascontiguousarray(np.asarray(mask[b], f32).T)
        in_maps.append(im)
    return in_maps


# ======================================================================
# mask analysis + dispatch
# ======================================================================

def _analyze_mask(mask):
    m = np.asarray(mask)
    iota = np.arange(S)
    n = m.sum(axis=2)
    causal = bool((n == iota[None, :] + 1).all()) and \
        bool((m == (iota[None, None, :] < n[..., None])).all())
    allones = bool((m == 1).all())

    k_needed, k_full, mixed = [], [], set()
    if allones:
        mode = "affine"
        k_needed = [NKT] * NQT
        k_full = [NKT] * NQT
    elif causal:
        mode = "affine"
        for qt in range(NQT):
            k_needed.append(4 * qt + 4)
            k_full.append(4 * qt)
            for kk in range(4 * qt, 4 * qt + 4):
                mixed.add((qt, kk))
    else:
        mode = "dense"
        for qt in range(NQT):
            sl = m[:, qt * 512:(qt + 1) * 512, :]
            need = 0
            full = NKT
            for kk in range(NKT):
                blk = sl[:, :, kk * 128:(kk + 1) * 128]
                if blk.any():
                    need = kk + 1
                if not blk.all():
                    full = min(full, kk)
            need = max(need, 1)
            if mode == "dense":
                # keep k-tile count even for pairing safety (harmless extra)
                pass
            full = min(full, need)
            k_needed.append(need)
            k_full.append(full)
            for kk in range(full, need):
                blk = sl[:, :, kk * 128:(kk + 1) * 128]
                if not blk.all():
                    mixed.add((qt, kk))
    return mode, tuple(k_needed), tuple(k_full), frozenset(mixed)


def _run(x_q, x_k, x_v, mask, wq, wk, wv, bq, bk, bv, wo, bo,
         trace=False, trace_cores=None):
    mode, k_needed, k_full, mixed = _analyze_mask(mask)
    key = (mode, k_needed, k_full, mixed)
    if key not in _cache:
        if mode == "affine":
            _cache[key] = _build_v2(k_needed, mixed)
        else:
            _cache[key] = _build_dense(k_needed, k_full, mixed)
    nc = _cache[key]
    if mode == "affine":
        in_maps = _prep_inputs_v2(x_q, x_k, x_v, wq, wk, wv, bq, bk, bv, wo)
    else:
        in_maps = _prep_inputs_dense(x_q, x_k, x_v, mask, wq, wk, wv,
                                     bq, bk, bv, wo)
    res = run_bass_kernel_spmd(nc, in_maps, core_ids=list(range(NC)),
                               trace=trace, trace_cores=trace_cores)
    bo = np.asarray(bo, np.float32)
    out = np.empty((B, S, D), np.float32)
    for b in range(B):
        out[b] = (np.asarray(res.results[2 * b]["y"], np.float32)
                  + np.asarray(res.results[2 * b + 1]["y"], np.float32) + bo)
    return out, res


def kernel(x_q, x_k, x_v, mask, wq, wk, wv, bq, bk, bv, wo, bo):
    out, _ = _run(x_q, x_k, x_v, mask, wq, wk, wv, bq, bk, bv, wo, bo)
    return out


# revision 18
# speedup vs baseline: 1.5199x; 1.3972x over previous
"""MultiHeadAttention (B=4, S=2048, d_model=1024, H=16, dh=64) on 8 trn2 cores.

Sharding: core (b, g) = batch b in 0..3, head-group g in 0..1 (8 heads each).

v2 (causal path): fp8e4 + DoubleRow matmuls for QKV projections, AV, and the
output projection (0.5 cyc/row); scores stay fp32r with head-pair row packing.
Softmax exp split between ACT (true exp, fp8 out) and DVE (Schraudolph
affine-int8 trick -> bitcast fp8e4). Causal straddle tiles are handled with a
static tril fp8 multiply on a 128-wide band plus a memset of the fully-masked
prefix (replaces the full-tile gpsimd affine_select). Normalization: rowsum
rides the AV matmul as a 65th V column; 1/l via reciprocal_approx_fast +
gpsimd partition_broadcast; the normalize TT reads ctx straight from PSUM and
writes the fp8 DoubleRow-interleaved ctx tile for the output projection.
Output y is DMA'd directly from PSUM.

Dense-mask path: original fp32r kernel (unchanged).
"""
import sys
sys.path.insert(0, "/opt/trn_rl_repo")

import os
import numpy as np
import ml_dtypes

import concourse.bass as bass
import concourse.mybir as mybir
import concourse.tile as tile
from concourse import bacc
from concourse.bass_utils import run_bass_kernel_spmd

F32 = mybir.dt.float32
F32R = mybir.dt.float32r
F8 = mybir.dt.float8e4
I8 = mybir.dt.int8
AF = mybir.ActivationFunctionType
AL = mybir.AluOpType
PM = mybir.MatmulPerfMode

NP_F8 = ml_dtypes.float8_e4m3

INTERLEAVE = os.environ.get("KNOILV") != "1"
EXP_DVE8 = int(os.environ.get("KEXPDVE8", "3"))   # of 8 exp ops, how many on DVE
EXPB = float(os.environ.get("KEXPB", "16248.6"))  # Schraudolph int16/bf16 bias
BAND_GP = os.environ.get("KBANDGP", "1") == "1"   # tril band mask on gpsimd
Y_COPY = os.environ.get("KYCOPY") == "1"          # stage y in SBUF vs psum DMA

B, S, D, H, DH = 4, 2048, 1024, 16, 64
NC = 8
G = 2              # head groups (cores per batch)
HPC = H // G       # 8 heads per core
EH = HPC * DH      # 512
NQT = S // 512     # 4 q-tiles
NKT = S // 128     # 16 k-tiles
NKT2 = S // 256    # 8 k256-tiles
SCALE = 1.0 / np.sqrt(DH)
LOG2E = 1.4426950408889634

_cache = {}
MM_NAMES = {"sc0": set(), "sc1": set(), "av": set(), "pj": set(), "p3": set()}


# ======================================================================
# v2 causal/affine path
# ======================================================================

def _build_v2(k_needed, mixed):
    """k_needed[qt]: leading k128-tiles per q-tile (even). mixed: straddle set."""
    nc = bacc.Bacc("TRN2", target_bir_lowering=False, debug=False, num_devices=NC)

    BF = mybir.dt.bfloat16
    xqb_d = nc.dram_tensor("xqb", [128, 8, S], BF, kind="ExternalInput").ap()
    xkb_d = nc.dram_tensor("xkb", [128, 8, S], BF, kind="ExternalInput").ap()
    xvb_d = nc.dram_tensor("xvb", [128, 8, S], BF, kind="ExternalInput").ap()
    wqb_d = nc.dram_tensor("wqb", [128, 8, EH], BF, kind="ExternalInput").ap()
    wkb_d = nc.dram_tensor("wkb", [128, 8, EH], BF, kind="ExternalInput").ap()
    wvb_d = nc.dram_tensor("wvb", [128, 8, EH], BF, kind="ExternalInput").ap()
    wob_d = nc.dram_tensor("wob", [128, 4, D], BF, kind="ExternalInput").ap()
    bq_d = nc.dram_tensor("bq", [128, 4], F32, kind="ExternalInput").ap()
    bk_d = nc.dram_tensor("bk", [128, 4], F32, kind="ExternalInput").ap()
    bv_d = nc.dram_tensor("bv", [128, HPC, DH], F32, kind="ExternalInput").ap()
    tril_d = nc.dram_tensor("trilb", [128, 128], BF, kind="ExternalInput").ap()
    y_d = nc.dram_tensor("y", [S, D], mybir.dt.bfloat16, kind="ExternalOutput").ap()

    with tile.TileContext(nc) as tc:
        with nc.allow_low_precision(reason="bf16/f32r attention within 2e-2 gate"):
            _body_v2(nc, tc, k_needed, mixed,
                     xqb_d, xkb_d, xvb_d, wqb_d, wkb_d, wvb_d, wob_d,
                     bq_d, bk_d, bv_d, tril_d, y_d)
    nc.compile()
    return nc


def _body_v2(nc, tc, k_needed, mixed,
             xqb_d, xkb_d, xvb_d, wqb_d, wkb_d, wvb_d, wob_d,
             bq_d, bk_d, bv_d, tril_d, y_d):
    BF = mybir.dt.bfloat16
    I16 = mybir.dt.int16
    # Schraudolph exp in bf16 bit space: i16 = round(A16*s + B16); bitcast bf16
    A16 = float(SCALE * 128.0 * LOG2E)
    B16 = float(EXPB)

    pers_cm = tc.tile_pool(name="pers", bufs=1)
    pers = pers_cm.__enter__()
    KT = pers.tile([128, 4, S], F32R)             # [eh%128, et, t]
    V65 = pers.tile([128, NKT, HPC, 65], BF)      # [t%128, kk, h, e|1]
    wqb_t = pers.tile([128, 8, EH], BF)
    wkb_t = pers.tile([128, 8, EH], BF)
    wvb_t = pers.tile([128, 8, EH], BF)
    wob_t = pers.tile([128, 4, D], BF)
    bq_t = pers.tile([128, 4], F32)
    bk_t = pers.tile([128, 4], F32)
    bv_t = pers.tile([128, HPC, DH], F32)
    tril_t = pers.tile([128, 128], BF)
    nc.sync.dma_start(bq_t[:], bq_d)
    nc.sync.dma_start(bk_t[:], bk_d)
    nc.sync.dma_start(bv_t[:], bv_d)
    nc.sync.dma_start(tril_t[:], tril_d)
    nc.sync.dma_start(wqb_t[:], wqb_d)
    nc.sync.dma_start(wkb_t[:], wkb_d)
    nc.sync.dma_start(wvb_t[:], wvb_d)
    nc.vector.memset(V65[:, :, :, 64:65], 1.0)

    px_cm = tc.tile_pool(name="px", bufs=3)
    px = px_cm.__enter__()
    pq_cm = tc.tile_pool(name="pq", bufs=2)
    pq = pq_cm.__enter__()
    pcw_cm = tc.tile_pool(name="pcw", bufs=3)
    pcw = pcw_cm.__enter__()
    ppt_cm = tc.tile_pool(name="ppt", bufs=2)
    ppt = ppt_cm.__enter__()
    pnrm_cm = tc.tile_pool(name="pnrm", bufs=2)
    pnrm = pnrm_cm.__enter__()
    py_cm = tc.tile_pool(name="py", bufs=2)
    py = py_cm.__enter__()
    pp_cm = tc.tile_pool(name="pp", bufs=2, space="PSUM")
    pp = pp_cm.__enter__()
    psc_cm = tc.tile_pool(name="psc", bufs=2, space="PSUM")
    psc = psc_cm.__enter__()
    pav_cm = tc.tile_pool(name="pav", bufs=1, space="PSUM")
    pav = pav_cm.__enter__()

    qwin = {}    # tq -> [128, 4, 512] Q^T window tile (f32r)
    ctxw = {}    # qt -> [128, 2, 2, 512] fp8 interleaved ctx tile
    state = {}
    expctr = [0]

    # ---- projection chunks ----
    def qk_chunks(tq):
        def load():
            xq_t = px.tile([128, 8, 512], BF, tag="x", name=f"xq{tq}")
            nc.sync.dma_start(xq_t[:], xqb_d[:, :, tq * 512:(tq + 1) * 512])
            xk_t = px.tile([128, 8, 512], BF, tag="x", name=f"xk{tq}")
            nc.sync.dma_start(xk_t[:], xkb_d[:, :, tq * 512:(tq + 1) * 512])
            state["xq"], state["xk"] = xq_t, xk_t
            qwin[tq] = pq.tile([128, 4, 512], F32R, tag="qw", name=f"qw{tq}")
        load.mms = 0

        def mmgroup(et, kind):
            def f():
                w_t = wqb_t if kind == "q" else wkb_t
                x_t = state["xq" if kind == "q" else "xk"]
                ps_t = pp.tile([128, 512], F32, tag="pj", name=f"ps_{kind}{tq}_{et}")
                for dk in range(8):
                    mi = nc.tensor.matmul(ps_t[:],
                                          w_t[:, dk, et * 128:(et + 1) * 128],
                                          x_t[:, dk, :],
                                          start=(dk == 0), stop=(dk == 7))
                    MM_NAMES["pj"].add(mi.ins.name)
                if kind == "q":
                    nc.vector.tensor_tensor(
                        qwin[tq][:, et, :], ps_t[:],
                        bq_t[:, et:et + 1].to_broadcast([128, 512]), AL.add)
                else:
                    nc.vector.tensor_tensor(
                        KT[:, et, tq * 512:(tq + 1) * 512], ps_t[:],
                        bk_t[:, et:et + 1].to_broadcast([128, 512]), AL.add)
            f.mms = 4
            return f

        out = [load]
        for kind in ("q", "k"):
            for et in range(4):
                out.append(mmgroup(et, kind))
        return out

    def v_chunks(tv):
        def load():
            xv_t = px.tile([128, 8, 512], BF, tag="x", name=f"xv{tv}")
            nc.sync.dma_start(xv_t[:], xvb_d[:, :, tv * 512:(tv + 1) * 512])
            state["xv"] = xv_t
        load.mms = 0

        def mmgroup(tl):
            def f():
                x_t = state["xv"]
                tt = tv * 4 + tl
                ps_t = pp.tile([128, 512], F32, tag="pj", name=f"ps_v{tt}")
                for dk in range(8):
                    mi = nc.tensor.matmul(ps_t[:],
                                          x_t[:, dk, tl * 128:(tl + 1) * 128],
                                          wvb_t[:, dk, :],
                                          start=(dk == 0), stop=(dk == 7))
                    MM_NAMES["pj"].add(mi.ins.name)
                nc.vector.tensor_tensor(
                    V65[:, tt, :, 0:64],
                    ps_t.rearrange("p (h e) -> p h e", h=HPC),
                    bv_t[:], AL.add)
            f.mms = 4
            return f

        out = [load]
        for tl in range(4):
            out.append(mmgroup(tl))
        return out

    def p3_chunks(qt):
        p3state = {}

        def half_a(tl, mc):
            def f():
                cw = ctxw[qt]
                tt = qt * 4 + tl
                ps_t = pp.tile([128, 512], F32, tag="pj", name=f"ps_o{tt}_{mc}")
                p3state[(tl, mc)] = ps_t
                for hp in range(2):
                    mi = nc.tensor.matmul(ps_t[:],
                                          cw[:, hp, tl * 128:(tl + 1) * 128],
                                          wob_t[:, hp, mc * 512:(mc + 1) * 512],
                                          start=(hp == 0), stop=False)
                    MM_NAMES["p3"].add(mi.ins.name)
            f.mms = 2
            return f

        def half_b(tl, mc):
            def f():
                cw = ctxw[qt]
                tt = qt * 4 + tl
                ps_t = p3state.pop((tl, mc))
                for hp in range(2, 4):
                    mi = nc.tensor.matmul(ps_t[:],
                                          cw[:, hp, tl * 128:(tl + 1) * 128],
                                          wob_t[:, hp, mc * 512:(mc + 1) * 512],
                                          start=False, stop=(hp == 3))
                    MM_NAMES["p3"].add(mi.ins.name)
                y_t = py.tile([128, 512], BF, tag="y", name=f"y{tt}_{mc}")
                if Y_COPY:
                    nc.vector.tensor_copy(y_t[:], ps_t[:])
                else:
                    nc.scalar.copy(y_t[:], ps_t[:])
                nc.sync.dma_start(
                    y_d[tt * 128:(tt + 1) * 128, mc * 512:(mc + 1) * 512],
                    y_t[:])
            f.mms = 2
            return f

        out = []
        for tl in range(4):
            for mc in range(2):
                out.append(half_a(tl, mc))
                out.append(half_b(tl, mc))
        return out

    # ---- attention window ----
    def window(qt, work):
        if not INTERLEAVE:
            for f in work:
                f()
            work = []
        klim = k_needed[qt]          # in k128 units
        q0 = qt * 512
        n_units = 4 * klim
        cw = pcw.tile([128, 4, 512], BF, tag="cw", name=f"cw{qt}")
        ctxw[qt] = cw
        done = 0
        wi = 0
        total_mms = sum(getattr(f, "mms", 1) for f in work) or 1
        emitted = 0
        qw = qwin[qt]
        for hp in range(4):
            av = pav.tile([65, 2, 512], F32, tag="av")
            for kk in range(klim):
                straddle = (qt, kk) in mixed
                qoff = max(0, kk * 128 - q0) if straddle else 0
                moff = 128 if qoff >= 128 else 0
                s_t = psc.tile([128, 2, 512], F32, tag="sc")
                for j in range(2):
                    mi = nc.tensor.matmul(
                        s_t[:, j, moff:512],
                        KT[j * 64:(j + 1) * 64, hp, kk * 128:(kk + 1) * 128],
                        qw[j * 64:(j + 1) * 64, hp, moff:512],
                        start=True, stop=True, tile_position=(j * 64, 0))
                    MM_NAMES[f"sc{j}"].add(mi.ins.name)
                p_t = ppt.tile([128, 2, 512], BF, tag="pt")
                use_dve = (expctr[0] % 8) < EXP_DVE8
                expctr[0] += 1
                if use_dve:
                    p_i = p_t[:].bitcast(I16)
                    nc.vector.tensor_scalar(
                        p_i[:, :, qoff:512], s_t[:, :, qoff:512],
                        A16, B16, op0=AL.mult, op1=AL.add)
                else:
                    nc.scalar.activation(p_t[:, :, qoff:512],
                                         s_t[:, :, qoff:512],
                                         AF.Exp, scale=float(SCALE))
                if straddle:
                    # zero q < qoff+p: fully-masked prefix plus diagonal band,
                    # restricted to [0:qoff+128) (beyond is all-keep)
                    w = qoff + 128
                    nc.gpsimd.affine_select(
                        p_t[:, :, 0:w], p_t[:, :, 0:w],
                        pattern=[[0, 2], [1, w]],
                        compare_op=AL.is_ge, fill=0.0,
                        base=q0 - kk * 128, channel_multiplier=-1)
                for j in range(2):
                    mi = nc.tensor.matmul(av[:, j, :],
                                          V65[:, kk, 2 * hp + j, :],
                                          p_t[:, j, :],
                                          start=(kk == 0), stop=(kk == klim - 1))
                    MM_NAMES["av"].add(mi.ins.name)
                done += 1
                if INTERLEAVE:
                    target = done * total_mms / n_units
                    while wi < len(work) and emitted < target:
                        emitted += getattr(work[wi], "mms", 1)
                        work[wi]()
                        wi += 1
            # ---- normalize: drain av psum fast, then ctx*(1/l) with slack ----
            avc = pnrm.tile([128, 512], F32, tag="avc")
            nc.vector.tensor_copy(avc[0:64, :], av[0:64, 0, :])
            nc.vector.tensor_copy(avc[64:128, :], av[0:64, 1, :])
            L2 = pnrm.tile([1, 2, 512], F32, tag="lin")
            nc.scalar.copy(L2[:], av[64:65, :, :])
            lrec = pnrm.tile([1, 2, 512], F32, tag="lrec")
            nc.vector.reciprocal_approx_fast(lrec[:], L2[:])
            rec_bc = pnrm.tile([128, 2, 512], F32, tag="rbc")
            nc.gpsimd.partition_broadcast(rec_bc[:], lrec[0:1, :, :])
            for j in range(2):
                nc.vector.tensor_tensor(
                    cw[j * 64:(j + 1) * 64, hp, :],
                    avc[j * 64:(j + 1) * 64, :],
                    rec_bc[j * 64:(j + 1) * 64, j, :], AL.mult)
        while wi < len(work):
            work[wi]()
            wi += 1

    # ---- prologue ----
    nc.sync.dma_start(wob_t[:], wob_d)
    for f in qk_chunks(0):
        f()
    for f in v_chunks(0):
        f()

    for qt in range(NQT):
        work = []
        if qt + 1 < NQT:
            work += qk_chunks(qt + 1)
            work += v_chunks(qt + 1)
        if qt == 2:
            work += p3_chunks(0)
        elif qt == 3:
            work += p3_chunks(1)
            work += p3_chunks(2)
        window(qt, work)
    for f in p3_chunks(NQT - 1):
        f()

    for cm in (pav_cm, psc_cm, pp_cm, py_cm, pnrm_cm, ppt_cm, pcw_cm, pq_cm,
               px_cm, pers_cm):
        cm.__exit__(None, None, None)


def _prep_inputs_v2(x_q, x_k, x_v, wq, wk, wv, bq, bk, bv, wo):
    f32 = np.float32
    bf16 = ml_dtypes.bfloat16
    trilb = np.triu(np.ones((128, 128), f32)).astype(bf16)
    in_maps = []

    def xb_of(x):
        # [128, 8, S]: xb[p, dk, t] = x[t, dk*128 + p]
        xr = np.asarray(x, f32).T.reshape(8, 128, S)
        return np.ascontiguousarray(xr.transpose(1, 0, 2)).astype(bf16)

    def wb_of(w, hs):
        wt = np.asarray(w[hs], f32).transpose(2, 0, 1).reshape(D, EH)
        return np.ascontiguousarray(
            wt.reshape(8, 128, EH).transpose(1, 0, 2)).astype(bf16)

    xq_c = {}
    for core in range(NC):
        b, g = divmod(core, G)
        hs = slice(g * HPC, (g + 1) * HPC)
        if b not in xq_c:
            xq_c[b] = (xb_of(x_q[b]), xb_of(x_k[b]), xb_of(x_v[b]))
        xqb, xkb, xvb = xq_c[b]
        im = {
            "xqb": xqb, "xkb": xkb, "xvb": xvb,
            "wqb": wb_of(wq, hs),
            "wkb": wb_of(wk, hs),
            "wvb": wb_of(wv, hs),
            "trilb": trilb,
        }
        woT = np.asarray(wo[:, g * EH:(g + 1) * EH], f32).T   # [EH, D]
        im["wob"] = np.ascontiguousarray(
            woT.reshape(4, 128, D).transpose(1, 0, 2)).astype(bf16)
        for name, bb in (("bq", bq), ("bk", bk)):
            flat = np.asarray(bb[hs], f32).reshape(EH)
            im[name] = np.ascontiguousarray(flat.reshape(4, 128).T)
        im["bv"] = np.broadcast_to(
            np.asarray(bv[hs], f32).reshape(1, HPC, DH), (128, HPC, DH)).copy()
        in_maps.append(im)
    return in_maps


# ======================================================================
# dense-mask fallback: original fp32r kernel
# ======================================================================

def _build_dense(k_needed, k_full, mixed):
    nc = bacc.Bacc("TRN2", target_bir_lowering=False, debug=False, num_devices=NC)

    xqT_d = nc.dram_tensor("xqT", [D, S], F32R, kind="ExternalInput").ap()
    xkT_d = nc.dram_tensor("xkT", [D, S], F32R, kind="ExternalInput").ap()
    xvT_d = nc.dram_tensor("xvT", [D, S], F32R, kind="ExternalInput").ap()
    wqT_d = nc.dram_tensor("wqT", [128, 8, EH], F32R, kind="ExternalInput").ap()
    wkT_d = nc.dram_tensor("wkT", [128, 8, EH], F32R, kind="ExternalInput").ap()
    wvT_d = nc.dram_tensor("wvT", [128, 8, EH], F32R, kind="ExternalInput").ap()
    bq_d = nc.dram_tensor("bq", [128, 4], F32, kind="ExternalInput").ap()
    bk_d = nc.dram_tensor("bk", [128, 4], F32, kind="ExternalInput").ap()
    bv_d = nc.dram_tensor("bv", [128, EH], F32, kind="ExternalInput").ap()
    woT_d = nc.dram_tensor("woT", [128, 4, D], F32R, kind="ExternalInput").ap()
    ones_d = nc.dram_tensor("ones1", [128, 1], F32R, kind="ExternalInput").ap()
    mT_d = nc.dram_tensor("maskT", [S, S], F32R, kind="ExternalInput").ap()
    mT_v = mT_d.rearrange("(kt p) q -> p kt q", p=128)
    y_d = nc.dram_tensor("y", [S, D], F32, kind="ExternalOutput").ap()

    xq_v = xqT_d.rearrange("(dk p) t -> p dk t", p=128)
    xk_v = xkT_d.rearrange("(dk p) t -> p dk t", p=128)
    xv_v = xvT_d.rearrange("(dk p) t -> p dk t", p=128)

    with tile.TileContext(nc) as tc:
        with nc.allow_low_precision(reason="fp32r storage has fp32 width"):
            _body_dense(nc, tc, k_needed, mixed,
                        xq_v, xk_v, xv_v, wqT_d, wkT_d, wvT_d,
                        bq_d, bk_d, bv_d, woT_d, ones_d, mT_v, y_d)
    nc.compile()
    return nc


def _body_dense(nc, tc, k_needed, mixed,
                xq_v, xk_v, xv_v, wqT_d, wkT_d, wvT_d,
                bq_d, bk_d, bv_d, woT_d, ones_d, mT_v, y_d):
    pers_cm = tc.tile_pool(name="pers", bufs=1)
    pers = pers_cm.__enter__()
    KT = pers.tile([128, 4, S], F32R)
    V65 = pers.tile([128, NKT, HPC, 65], F32R)
    wo_t = pers.tile([128, 4, D], F32R)
    wv_t = pers.tile([128, 8, EH], F32R)
    bq_t = pers.tile([128, 4], F32)
    bk_t = pers.tile([128, 4], F32)
    bv_t = pers.tile([128, EH], F32)
    ones_t = pers.tile([128, 1], F32R)
    nc.sync.dma_start(bq_t[:], bq_d)
    nc.sync.dma_start(bk_t[:], bk_d)
    nc.sync.dma_start(bv_t[:], bv_d)
    nc.sync.dma_start(ones_t[:], ones_d)
    nc.sync.dma_start(wv_t[:], wvT_d)
    nc.vector.tensor_copy(V65[:, :, :, 64:65],
                          ones_t[:, 0:1].to_broadcast([128, NKT, HPC, 1]))

    pw_cm = tc.tile_pool(name="pw", bufs=2)
    pw = pw_cm.__enter__()
    px_cm = tc.tile_pool(name="px", bufs=3)
    px = px_cm.__enter__()
    pq_cm = tc.tile_pool(name="pq", bufs=2)
    pq = pq_cm.__enter__()
    pcw_cm = tc.tile_pool(name="pcw", bufs=3)
    pcw = pcw_cm.__enter__()
    ppt_cm = tc.tile_pool(name="ppt", bufs=2)
    ppt = ppt_cm.__enter__()
    pnrm_cm = tc.tile_pool(name="pnrm", bufs=1)
    pnrm = pnrm_cm.__enter__()
    py_cm = tc.tile_pool(name="py", bufs=2)
    py = py_cm.__enter__()
    pp_cm = tc.tile_pool(name="pp", bufs=2, space="PSUM")
    pp = pp_cm.__enter__()
    psc_cm = tc.tile_pool(name="psc", bufs=2, space="PSUM")
    psc = psc_cm.__enter__()
    pav_cm = tc.tile_pool(name="pav", bufs=1, space="PSUM")
    pav = pav_cm.__enter__()

    qwin = {}
    ctxw = {}
    state = {}

    def qk_chunks(tq):
        def load(w_d, x_v, kind, half):
            def f():
                hs = slice(half * 4, half * 4 + 4)
                w_t = pw.tile([128, 4, EH], F32R, tag="w", name=f"w_{kind}{tq}{half}")
                nc.sync.dma_start(w_t[:], w_d[:, hs, :])
                x_t = px.tile([128, 4, 512], F32R, tag="x", name=f"x_{kind}{tq}{half}")
                nc.sync.dma_start(x_t[:], x_v[:, hs, tq * 512:(tq + 1) * 512])
                state[f"w{half}"], state[f"x{half}"] = w_t, x_t
                if kind == "q" and half == 0:
                    qwin[tq] = pq.tile([128, 4, 512], F32R, tag="qw", name=f"qw{tq}")
            return f

        def mmgroup(et, kind):
            def f():
                ps_t = pp.tile([128, 512], F32, tag="pj", name=f"ps_{kind}{tq}_{et}")
                for dk in range(8):
                    w_t = state[f"w{dk // 4}"]
                    x_t = state[f"x{dk // 4}"]
                    nc.tensor.matmul(ps_t[:],
                                     w_t[:, dk % 4, et * 128:(et + 1) * 128],
                                     x_t[:, dk % 4, :],
                                     start=(dk == 0), stop=(dk == 7))
                if kind == "q":
                    nc.vector.tensor_tensor(
                        qwin[tq][:, et, :], ps_t[:],
                        bq_t[:, et:et + 1].to_broadcast([128, 512]), AL.add)
                else:
                    nc.vector.tensor_tensor(
                        KT[:, et, tq * 512:(tq + 1) * 512], ps_t[:],
                        bk_t[:, et:et + 1].to_broadcast([128, 512]), AL.add)
            return f

        out = []
        for kind, w_d, x_v in (("q", wqT_d, xq_v), ("k", wkT_d, xk_v)):
            for half in (0, 1):
                g = load(w_d, x_v, kind, half)
                g.mms = 0
                out.append(g)
            for et in range(4):
                g = mmgroup(et, kind)
                g.mms = 8
                out.append(g)
        return out

    def v_chunks(tv):
        def load(half):
            def f():
                hs = slice(half * 4, half * 4 + 4)
                x_t = px.tile([128, 4, 512], F32R, tag="x", name=f"x_v{tv}{half}")
                nc.sync.dma_start(x_t[:], xv_v[:, hs, tv * 512:(tv + 1) * 512])
                state[f"x{half}"] = x_t
            return f

        def mmgroup(tl):
            def f():
                tt = tv * 4 + tl
                ps_t = pp.tile([128, 512], F32, tag="pj", name=f"ps_v{tt}")
                for dk in range(8):
                    x_t = state[f"x{dk // 4}"]
                    nc.tensor.matmul(ps_t[:],
                                     x_t[:, dk % 4, tl * 128:(tl + 1) * 128],
                                     wv_t[:, dk, :],
                                     start=(dk == 0), stop=(dk == 7))
                nc.vector.tensor_tensor(
                    V65[:, tt, :, 0:64],
                    ps_t.rearrange("p (h e) -> p h e", h=HPC),
                    bv_t.rearrange("p (h e) -> p h e", h=HPC), AL.add)
            return f

        out = []
        for half in (0, 1):
            g = load(half)
            g.mms = 0
            out.append(g)
        for tl in range(4):
            g = mmgroup(tl)
            g.mms = 8
            out.append(g)
        return out

    def p3_chunks(qt):
        p3state = {}

        def half_a(tl, mc):
            def f():
                cw = ctxw[qt]
                tt = qt * 4 + tl
                ps_t = pp.tile([128, 512], F32, tag="pj", name=f"ps_o{tt}_{mc}")
                p3state[(tl, mc)] = ps_t
                for hp in range(2):
                    nc.tensor.matmul(ps_t[:],
                                     cw[:, hp, tl * 128:(tl + 1) * 128],
                                     wo_t[:, hp, mc * 512:(mc + 1) * 512],
                                     start=(hp == 0), stop=False)
            f.mms = 2
            return f

        def half_b(tl, mc):
            def f():
                cw = ctxw[qt]
                tt = qt * 4 + tl
                ps_t = p3state.pop((tl, mc))
                for hp in range(2, 4):
                    nc.tensor.matmul(ps_t[:],
                                     cw[:, hp, tl * 128:(tl + 1) * 128],
                                     wo_t[:, hp, mc * 512:(mc + 1) * 512],
                                     start=False, stop=(hp == 3))
                y_t = py.tile([128, 512], F32, tag="y", name=f"y{tt}_{mc}")
                nc.vector.tensor_copy(y_t[:], ps_t[:])
                nc.sync.dma_start(
                    y_d[tt * 128:(tt + 1) * 128, mc * 512:(mc + 1) * 512],
                    y_t[:])
            f.mms = 2
            return f

        out = []
        for tl in range(4):
            for mc in range(2):
                out.append(half_a(tl, mc))
                out.append(half_b(tl, mc))
        return out

    def window(qt, work):
        if not INTERLEAVE:
            for f in work:
                f()
            work = []
        klim = k_needed[qt]
        q0 = qt * 512
        n_tiles = 4 * klim
        cw = pcw.tile([128, 4, 512], F32R, tag="cw", name=f"cw{qt}")
        ctxw[qt] = cw
        done = 0
        wi = 0
        total_mms = sum(getattr(f, "mms", 4) for f in work) or 1
        emitted = 0
        qw = qwin[qt]
        for hp in range(4):
            av0 = pav.tile([65, 512], F32, tag="av0")
            av1 = pav.tile([65, 512], F32, tag="av1")
            first = True
            for kk in range(klim):
                straddle = (qt, kk) in mixed
                s_t = psc.tile([128, 2, 512], F32, tag="sc")
                for j in range(2):
                    nc.tensor.matmul(
                        s_t[:, j, :],
                        KT[j * 64:(j + 1) * 64, hp, kk * 128:(kk + 1) * 128],
                        qw[j * 64:(j + 1) * 64, hp, :],
                        start=True, stop=True, tile_position=(j * 64, 0))
                p_t = ppt.tile([128, 2, 512], F32R, tag="pt")
                nc.scalar.activation(p_t[:], s_t[:], AF.Exp, scale=float(SCALE))
                if straddle:
                    sel_t = ppt.tile([128, 512], F32R, tag="sel")
                    nc.sync.dma_start(sel_t[:], mT_v[:, kk, q0:q0 + 512])
                    nc.vector.tensor_tensor(
                        p_t[:], p_t[:],
                        sel_t[:, None, :].to_broadcast([128, 2, 512]),
                        AL.mult)
                for j, av in ((0, av0), (1, av1)):
                    nc.tensor.matmul(av[:], V65[:, kk, 2 * hp + j, :],
                                     p_t[:, j, :],
                                     start=first, stop=(kk == klim - 1))
                first = False
                done += 1
                if INTERLEAVE:
                    target = done * total_mms / n_tiles
                    while wi < len(work) and emitted < target:
                        emitted += getattr(work[wi], "mms", 4)
                        work[wi]()
                        wi += 1
            avc = pnrm.tile([128, 512], F32, tag="avc", bufs=1)
            lin = pnrm.tile([1, 2, 512], F32, tag="lin")
            nc.vector.tensor_copy(avc[0:64, :], av0[0:64, :])
            nc.vector.tensor_copy(avc[64:128, :], av1[0:64, :])
            nc.vector.tensor_copy(lin[:, 0, :], av0[64:65, :])
            nc.vector.tensor_copy(lin[:, 1, :], av1[64:65, :])
            lrec = pnrm.tile([1, 2, 512], F32, tag="lrec")
            scr = pnrm.tile([1, 512], F32, tag="scr")
            nc.vector.reciprocal_approx_accurate(lrec[:, 0, :], lin[:, 0, :], scr[:])
            nc.vector.reciprocal_approx_accurate(lrec[:, 1, :], lin[:, 1, :], scr[:])
            rec_bc = pnrm.tile([128, 2, 512], F32, tag="rbc")
            nc.gpsimd.partition_broadcast(rec_bc[:], lrec[0:1, :, :])
            nc.vector.tensor_tensor(cw[0:64, hp, :],
                                    avc[0:64, :], rec_bc[0:64, 0, :], AL.mult)
            nc.vector.tensor_tensor(cw[64:128, hp, :],
                                    avc[64:128, :], rec_bc[64:128, 1, :], AL.mult)
        while wi < len(work):
            work[wi]()
            wi += 1

    for f in qk_chunks(0):
        f()
    for f in v_chunks(0):
        f()

    def wo_load():
        nc.sync.dma_start(wo_t[:], woT_d)
    wo_load.mms = 0

    for qt in range(NQT):
        work = []
        if qt == 0:
            work.append(wo_load)
        if qt + 1 < NQT:
            work += qk_chunks(qt + 1)
            work += v_chunks(qt + 1)
        if qt == 2:
            work += p3_chunks(0)
        elif qt == 3:
            work += p3_chunks(1)
            work += p3_chunks(2)
        window(qt, work)
    for f in p3_chunks(NQT - 1):
        f()

    for cm in (pav_cm, psc_cm, pp_cm, py_cm, pnrm_cm, ppt_cm, pcw_cm, pq_cm,
               px_cm, pw_cm, pers_cm):
        cm.__exit__(None, None, None)


def _prep_inputs_dense(x_q, x_k, x_v, mask, wq, wk, wv, bq, bk, bv, wo):
    f32 = np.float32
    in_maps = []
    ones1 = np.ones((128, 1), f32)
    for core in range(NC):
        b, g = divmod(core, G)
        hs = slice(g * HPC, (g + 1) * HPC)
        im = {
            "xqT": np.ascontiguousarray(np.asarray(x_q[b], f32).T),
            "xkT": np.ascontiguousarray(np.asarray(x_k[b], f32).T),
            "xvT": np.ascontiguousarray(np.asarray(x_v[b], f32).T),
            "ones1": ones1,
        }
        for name, w in (("wqT", wq), ("wkT", wk), ("wvT", wv)):
            wt = np.asarray(w[hs], f32).transpose(2, 0, 1).reshape(D, EH)
            im[name] = np.ascontiguousarray(wt.reshape(8, 128, EH))\
                .transpose(1, 0, 2).copy()
        for name, bb in (("bq", bq), ("bk", bk)):
            flat = np.asarray(bb[hs], f32).reshape(EH)
            im[name] = np.ascontiguousarray(flat.reshape(4, 128).T)
        im["bv"] = np.broadcast_to(np.asarray(bv[hs], f32).reshape(1, EH),
                                   (128, EH)).copy()
        woT = np.asarray(wo[:, g * EH:(g + 1) * EH], f32).T
        im["woT"] = np.ascontiguousarray(woT.reshape(4, 128, D))\
            .transpose(1, 0, 2).copy()
        im["maskT"] = np.ascontiguousarray(np.asarray(mask[b], f32).T)
        in_maps.append(im)
    return in_maps


# ======================================================================
# mask analysis + dispatch
# ======================================================================

def _analyze_mask(mask):
    m = np.asarray(mask)
    iota = np.arange(S)
    n = m.sum(axis=2)
    causal = bool((n == iota[None, :] + 1).all()) and \
        bool((m == (iota[None, None, :] < n[..., None])).all())
    allones = bool((m == 1).all())

    k_needed, k_full, mixed = [], [], set()
    if allones:
        mode = "affine"
        k_needed = [NKT] * NQT
        k_full = [NKT] * NQT
    elif causal:
        mode = "affine"
        for qt in range(NQT):
            k_needed.append(4 * qt + 4)
            k_full.append(4 * qt)
            for kk in range(4 * qt, 4 * qt + 4):
                mixed.add((qt, kk))
    else:
        mode = "dense"
        for qt in range(NQT):
            sl = m[:, qt * 512:(qt + 1) * 512, :]
            need = 0
            full = NKT
            for kk in range(NKT):
                blk = sl[:, :, kk * 128:(kk + 1) * 128]
                if blk.any():
                    need = kk + 1
                if not blk.all():
                    full = min(full, kk)
            need = max(need, 1)
            if mode == "dense":
                # keep k-tile count even for pairing safety (harmless extra)
                pass
            full = min(full, need)
            k_needed.append(need)
            k_full.append(full)
            for kk in range(full, need):
                blk = sl[:, :, kk * 128:(kk + 1) * 128]
                if not blk.all():
                    mixed.add((qt, kk))
    return mode, tuple(k_needed), tuple(k_full), frozenset(mixed)


def _run(x_q, x_k, x_v, mask, wq, wk, wv, bq, bk, bv, wo, bo,
         trace=False, trace_cores=None):
    mode, k_needed, k_full, mixed = _analyze_mask(mask)
    key = (mode, k_needed, k_full, mixed)
    if key not in _cache:
        if mode == "affine":
            _cache[key] = _build_v2(k_needed, mixed)
        else:
            _cache[key] = _build_dense(k_needed, k_full, mixed)
    nc = _cache[key]
    if mode == "affine":
        in_maps = _prep_inputs_v2(x_q, x_k, x_v, wq, wk, wv, bq, bk, bv, wo)
    else:
        in_maps = _prep_inputs_dense(x_q, x_k, x_v, mask, wq, wk, wv,
                                     bq, bk, bv, wo)
    res = run_bass_kernel_spmd(nc, in_maps, core_ids=list(range(NC)),
                               trace=trace, trace_cores=trace_cores)
    bo = np.asarray(bo, np.float32)
    out = np.empty((B, S, D), np.float32)
    for b in range(B):
        out[b] = (np.asarray(res.results[2 * b]["y"], np.float32)
                  + np.asarray(res.results[2 * b + 1]["y"], np.float32) + bo)
    return out, res


def kernel(x_q, x_k, x_v, mask, wq, wk, wv, bq, bk, bv, wo, bo):
    out, _ = _run(x_q, x_k, x_v, mask, wq, wk, wv, bq, bk, bv, wo, bo)
    return out


# revision 19
# speedup vs baseline: 1.6001x; 1.0528x over previous
"""MultiHeadAttention (B=4, S=2048, d_model=1024, H=16, dh=64) on 8 trn2 cores.

Sharding: core (b, g) = batch b in 0..3, head-group g in 0..1 (8 heads each).

v2 (causal path): fp8e4 + DoubleRow matmuls for QKV projections, AV, and the
output projection (0.5 cyc/row); scores stay fp32r with head-pair row packing.
Softmax exp split between ACT (true exp, fp8 out) and DVE (Schraudolph
affine-int8 trick -> bitcast fp8e4). Causal straddle tiles are handled with a
static tril fp8 multiply on a 128-wide band plus a memset of the fully-masked
prefix (replaces the full-tile gpsimd affine_select). Normalization: rowsum
rides the AV matmul as a 65th V column; 1/l via reciprocal_approx_fast +
gpsimd partition_broadcast; the normalize TT reads ctx straight from PSUM and
writes the fp8 DoubleRow-interleaved ctx tile for the output projection.
Output y is DMA'd directly from PSUM.

Dense-mask path: original fp32r kernel (unchanged).
"""
import sys
sys.path.insert(0, "/opt/trn_rl_repo")

import os
import numpy as np
import ml_dtypes

import concourse.bass as bass
import concourse.mybir as mybir
import concourse.tile as tile
from concourse import bacc
from concourse.bass_utils import run_bass_kernel_spmd

F32 = mybir.dt.float32
F32R = mybir.dt.float32r
F8 = mybir.dt.float8e4
I8 = mybir.dt.int8
AF = mybir.ActivationFunctionType
AL = mybir.AluOpType
PM = mybir.MatmulPerfMode

NP_F8 = ml_dtypes.float8_e4m3

INTERLEAVE = os.environ.get("KNOILV") != "1"
EXP_DVE8 = int(os.environ.get("KEXPDVE8", "4"))   # of 8 exp ops, how many on DVE
EXPB = float(os.environ.get("KEXPB", "16248.6"))  # Schraudolph int16/bf16 bias
BAND_GP = os.environ.get("KBANDGP", "1") == "1"   # tril band mask on gpsimd
Y_COPY = os.environ.get("KYCOPY") == "1"          # stage y in SBUF vs psum DMA

B, S, D, H, DH = 4, 2048, 1024, 16, 64
NC = 8
G = 2              # head groups (cores per batch)
HPC = H // G       # 8 heads per core
EH = HPC * DH      # 512
NQT = S // 512     # 4 q-tiles
NKT = S // 128     # 16 k-tiles
NKT2 = S // 256    # 8 k256-tiles
SCALE = 1.0 / np.sqrt(DH)
LOG2E = 1.4426950408889634

_cache = {}
MM_NAMES = {"sc0": set(), "sc1": set(), "av": set(), "pj": set(), "p3": set()}


# ======================================================================
# v2 causal/affine path
# ======================================================================

def _build_v2(k_needed, mixed):
    """k_needed[qt]: leading k128-tiles per q-tile (even). mixed: straddle set."""
    nc = bacc.Bacc("TRN2", target_bir_lowering=False, debug=False, num_devices=NC)

    BF = mybir.dt.bfloat16
    xqb_d = nc.dram_tensor("xqb", [128, 8, S], BF, kind="ExternalInput").ap()
    xkb_d = nc.dram_tensor("xkb", [128, 8, S], BF, kind="ExternalInput").ap()
    xvb_d = nc.dram_tensor("xvb", [128, 8, S], BF, kind="ExternalInput").ap()
    wqb_d = nc.dram_tensor("wqb", [128, 8, EH], BF, kind="ExternalInput").ap()
    wkb_d = nc.dram_tensor("wkb", [128, 8, EH], BF, kind="ExternalInput").ap()
    wvb_d = nc.dram_tensor("wvb", [128, 8, EH], BF, kind="ExternalInput").ap()
    wob_d = nc.dram_tensor("wob", [128, 4, D], BF, kind="ExternalInput").ap()
    bq_d = nc.dram_tensor("bq", [128, 4], F32, kind="ExternalInput").ap()
    bk_d = nc.dram_tensor("bk", [128, 4], F32, kind="ExternalInput").ap()
    bv_d = nc.dram_tensor("bv", [128, HPC, DH], F32, kind="ExternalInput").ap()
    tril_d = nc.dram_tensor("trilb", [128, 128], BF, kind="ExternalInput").ap()
    y_d = nc.dram_tensor("y", [S, D], mybir.dt.bfloat16, kind="ExternalOutput").ap()

    with tile.TileContext(nc) as tc:
        with nc.allow_low_precision(reason="bf16/f32r attention within 2e-2 gate"):
            _body_v2(nc, tc, k_needed, mixed,
                     xqb_d, xkb_d, xvb_d, wqb_d, wkb_d, wvb_d, wob_d,
                     bq_d, bk_d, bv_d, tril_d, y_d)
    nc.compile()
    return nc


def _body_v2(nc, tc, k_needed, mixed,
             xqb_d, xkb_d, xvb_d, wqb_d, wkb_d, wvb_d, wob_d,
             bq_d, bk_d, bv_d, tril_d, y_d):
    BF = mybir.dt.bfloat16
    I16 = mybir.dt.int16
    # Schraudolph exp in bf16 bit space: i16 = round(A16*s + B16); bitcast bf16
    A16 = float(SCALE * 128.0 * LOG2E)
    B16 = float(EXPB)

    pers_cm = tc.tile_pool(name="pers", bufs=1)
    pers = pers_cm.__enter__()
    KT = pers.tile([128, 4, S], F32R)             # [eh%128, et, t]
    V65 = pers.tile([128, NKT, HPC, 65], BF)      # [t%128, kk, h, e|1]
    wqb_t = pers.tile([128, 8, EH], BF)
    wkb_t = pers.tile([128, 8, EH], BF)
    wvb_t = pers.tile([128, 8, EH], BF)
    wob_t = pers.tile([128, 4, D], BF)
    bq_t = pers.tile([128, 4], F32)
    bk_t = pers.tile([128, 4], F32)
    bv_t = pers.tile([128, HPC, DH], F32)
    tril_t = pers.tile([128, 128], BF)
    nc.sync.dma_start(bq_t[:], bq_d)
    nc.sync.dma_start(bk_t[:], bk_d)
    nc.sync.dma_start(bv_t[:], bv_d)
    nc.sync.dma_start(tril_t[:], tril_d)
    nc.sync.dma_start(wqb_t[:], wqb_d)
    nc.sync.dma_start(wkb_t[:], wkb_d)
    nc.sync.dma_start(wvb_t[:], wvb_d)
    nc.vector.memset(V65[:, :, :, 64:65], 1.0)

    px_cm = tc.tile_pool(name="px", bufs=3)
    px = px_cm.__enter__()
    pq_cm = tc.tile_pool(name="pq", bufs=2)
    pq = pq_cm.__enter__()
    pcw_cm = tc.tile_pool(name="pcw", bufs=3)
    pcw = pcw_cm.__enter__()
    ppt_cm = tc.tile_pool(name="ppt", bufs=3)
    ppt = ppt_cm.__enter__()
    pnrm_cm = tc.tile_pool(name="pnrm", bufs=2)
    pnrm = pnrm_cm.__enter__()
    py_cm = tc.tile_pool(name="py", bufs=2)
    py = py_cm.__enter__()
    pp_cm = tc.tile_pool(name="pp", bufs=2, space="PSUM")
    pp = pp_cm.__enter__()
    psc_cm = tc.tile_pool(name="psc", bufs=2, space="PSUM")
    psc = psc_cm.__enter__()
    pav_cm = tc.tile_pool(name="pav", bufs=1, space="PSUM")
    pav = pav_cm.__enter__()

    qwin = {}    # tq -> [128, 4, 512] Q^T window tile (f32r)
    ctxw = {}    # qt -> [128, 2, 2, 512] fp8 interleaved ctx tile
    state = {}
    expctr = [0]

    # ---- projection chunks ----
    def qk_chunks(tq):
        def load():
            xq_t = px.tile([128, 8, 512], BF, tag="x", name=f"xq{tq}")
            nc.sync.dma_start(xq_t[:], xqb_d[:, :, tq * 512:(tq + 1) * 512])
            xk_t = px.tile([128, 8, 512], BF, tag="x", name=f"xk{tq}")
            nc.sync.dma_start(xk_t[:], xkb_d[:, :, tq * 512:(tq + 1) * 512])
            state["xq"], state["xk"] = xq_t, xk_t
            qwin[tq] = pq.tile([128, 4, 512], F32R, tag="qw", name=f"qw{tq}")
        load.mms = 0

        def mmgroup(et, kind):
            def f():
                w_t = wqb_t if kind == "q" else wkb_t
                x_t = state["xq" if kind == "q" else "xk"]
                ps_t = pp.tile([128, 512], F32, tag="pj", name=f"ps_{kind}{tq}_{et}")
                for dk in range(8):
                    mi = nc.tensor.matmul(ps_t[:],
                                          w_t[:, dk, et * 128:(et + 1) * 128],
                                          x_t[:, dk, :],
                                          start=(dk == 0), stop=(dk == 7))
                    MM_NAMES["pj"].add(mi.ins.name)
                if kind == "q":
                    nc.vector.tensor_tensor(
                        qwin[tq][:, et, :], ps_t[:],
                        bq_t[:, et:et + 1].to_broadcast([128, 512]), AL.add)
                else:
                    nc.vector.tensor_tensor(
                        KT[:, et, tq * 512:(tq + 1) * 512], ps_t[:],
                        bk_t[:, et:et + 1].to_broadcast([128, 512]), AL.add)
            f.mms = 4
            return f

        out = [load]
        for kind in ("q", "k"):
            for et in range(4):
                out.append(mmgroup(et, kind))
        return out

    def v_chunks(tv):
        def load():
            xv_t = px.tile([128, 8, 512], BF, tag="x", name=f"xv{tv}")
            nc.sync.dma_start(xv_t[:], xvb_d[:, :, tv * 512:(tv + 1) * 512])
            state["xv"] = xv_t
        load.mms = 0

        def mmgroup(tl):
            def f():
                x_t = state["xv"]
                tt = tv * 4 + tl
                ps_t = pp.tile([128, 512], F32, tag="pj", name=f"ps_v{tt}")
                for dk in range(8):
                    mi = nc.tensor.matmul(ps_t[:],
                                          x_t[:, dk, tl * 128:(tl + 1) * 128],
                                          wvb_t[:, dk, :],
                                          start=(dk == 0), stop=(dk == 7))
                    MM_NAMES["pj"].add(mi.ins.name)
                nc.vector.tensor_tensor(
                    V65[:, tt, :, 0:64],
                    ps_t.rearrange("p (h e) -> p h e", h=HPC),
                    bv_t[:], AL.add)
            f.mms = 4
            return f

        out = [load]
        for tl in range(4):
            out.append(mmgroup(tl))
        return out

    def p3_chunks(qt):
        p3state = {}

        def half_a(tl, mc):
            def f():
                cw = ctxw[qt]
                tt = qt * 4 + tl
                ps_t = pp.tile([128, 512], F32, tag="pj", name=f"ps_o{tt}_{mc}")
                p3state[(tl, mc)] = ps_t
                for hp in range(2):
                    mi = nc.tensor.matmul(ps_t[:],
                                          cw[:, hp, tl * 128:(tl + 1) * 128],
                                          wob_t[:, hp, mc * 512:(mc + 1) * 512],
                                          start=(hp == 0), stop=False)
                    MM_NAMES["p3"].add(mi.ins.name)
            f.mms = 2
            return f

        def half_b(tl, mc):
            def f():
                cw = ctxw[qt]
                tt = qt * 4 + tl
                ps_t = p3state.pop((tl, mc))
                for hp in range(2, 4):
                    mi = nc.tensor.matmul(ps_t[:],
                                          cw[:, hp, tl * 128:(tl + 1) * 128],
                                          wob_t[:, hp, mc * 512:(mc + 1) * 512],
                                          start=False, stop=(hp == 3))
                    MM_NAMES["p3"].add(mi.ins.name)
                y_t = py.tile([128, 512], BF, tag="y", name=f"y{tt}_{mc}")
                if Y_COPY:
                    nc.vector.tensor_copy(y_t[:], ps_t[:])
                else:
                    nc.scalar.copy(y_t[:], ps_t[:])
                nc.sync.dma_start(
                    y_d[tt * 128:(tt + 1) * 128, mc * 512:(mc + 1) * 512],
                    y_t[:])
            f.mms = 2
            return f

        out = []
        for tl in range(4):
            for mc in range(2):
                out.append(half_a(tl, mc))
                out.append(half_b(tl, mc))
        return out

    # ---- attention window ----
    def window(qt, work):
        if not INTERLEAVE:
            for f in work:
                f()
            work = []
        klim = k_needed[qt]          # in k128 units
        q0 = qt * 512
        n_units = 4 * klim
        cw = pcw.tile([128, 4, 512], BF, tag="cw", name=f"cw{qt}")
        ctxw[qt] = cw
        done = 0
        wi = 0
        total_mms = sum(getattr(f, "mms", 1) for f in work) or 1
        emitted = 0
        qw = qwin[qt]
        for hp in range(4):
            av = pav.tile([65, 2, 512], F32, tag="av")
            for kk in range(klim):
                straddle = (qt, kk) in mixed
                qoff = max(0, kk * 128 - q0) if straddle else 0
                moff = 128 if qoff >= 128 else 0
                s_t = psc.tile([128, 2, 512], F32, tag="sc")
                for j in range(2):
                    mi = nc.tensor.matmul(
                        s_t[:, j, moff:512],
                        KT[j * 64:(j + 1) * 64, hp, kk * 128:(kk + 1) * 128],
                        qw[j * 64:(j + 1) * 64, hp, moff:512],
                        start=True, stop=True, tile_position=(j * 64, 0))
                    MM_NAMES[f"sc{j}"].add(mi.ins.name)
                p_t = ppt.tile([128, 2, 512], BF, tag="pt")
                use_dve = (expctr[0] % 8) < EXP_DVE8
                expctr[0] += 1
                if use_dve:
                    p_i = p_t[:].bitcast(I16)
                    nc.vector.tensor_scalar(
                        p_i[:, :, qoff:512], s_t[:, :, qoff:512],
                        A16, B16, op0=AL.mult, op1=AL.add)
                else:
                    nc.scalar.activation(p_t[:, :, qoff:512],
                                         s_t[:, :, qoff:512],
                                         AF.Exp, scale=float(SCALE))
                if straddle:
                    # zero q < qoff+p: fully-masked prefix plus diagonal band,
                    # restricted to [0:qoff+128) (beyond is all-keep)
                    w = qoff + 128
                    nc.gpsimd.affine_select(
                        p_t[:, :, 0:w], p_t[:, :, 0:w],
                        pattern=[[0, 2], [1, w]],
                        compare_op=AL.is_ge, fill=0.0,
                        base=q0 - kk * 128, channel_multiplier=-1)
                for j in range(2):
                    mi = nc.tensor.matmul(av[:, j, :],
                                          V65[:, kk, 2 * hp + j, :],
                                          p_t[:, j, :],
                                          start=(kk == 0), stop=(kk == klim - 1))
                    MM_NAMES["av"].add(mi.ins.name)
                done += 1
                if INTERLEAVE:
                    target = done * total_mms / n_units
                    while wi < len(work) and emitted < target:
                        emitted += getattr(work[wi], "mms", 1)
                        work[wi]()
                        wi += 1
            # ---- normalize: drain av psum fast, then ctx*(1/l) with slack ----
            avc = pnrm.tile([128, 512], F32, tag="avc")
            nc.vector.tensor_copy(avc[0:64, :], av[0:64, 0, :])
            nc.vector.tensor_copy(avc[64:128, :], av[0:64, 1, :])
            L2 = pnrm.tile([1, 2, 512], F32, tag="lin")
            nc.scalar.copy(L2[:], av[64:65, :, :])
            lrec = pnrm.tile([1, 2, 512], F32, tag="lrec")
            nc.vector.reciprocal_approx_fast(lrec[:], L2[:])
            rec_bc = pnrm.tile([128, 2, 512], F32, tag="rbc")
            nc.gpsimd.partition_broadcast(rec_bc[:], lrec[0:1, :, :])
            for j in range(2):
                nc.vector.tensor_tensor(
                    cw[j * 64:(j + 1) * 64, hp, :],
                    avc[j * 64:(j + 1) * 64, :],
                    rec_bc[j * 64:(j + 1) * 64, j, :], AL.mult)
        while wi < len(work):
            work[wi]()
            wi += 1

    # ---- prologue ----
    nc.sync.dma_start(wob_t[:], wob_d)
    for f in qk_chunks(0):
        f()
    for f in v_chunks(0):
        f()

    for qt in range(NQT):
        work = []
        if qt + 1 < NQT:
            work += qk_chunks(qt + 1)
            work += v_chunks(qt + 1)
        if qt == 2:
            work += p3_chunks(0)
        elif qt == 3:
            work += p3_chunks(1)
            work += p3_chunks(2)
        window(qt, work)
    for f in p3_chunks(NQT - 1):
        f()

    for cm in (pav_cm, psc_cm, pp_cm, py_cm, pnrm_cm, ppt_cm, pcw_cm, pq_cm,
               px_cm, pers_cm):
        cm.__exit__(None, None, None)


def _prep_inputs_v2(x_q, x_k, x_v, wq, wk, wv, bq, bk, bv, wo):
    f32 = np.float32
    bf16 = ml_dtypes.bfloat16
    trilb = np.triu(np.ones((128, 128), f32)).astype(bf16)
    in_maps = []

    def xb_of(x):
        # [128, 8, S]: xb[p, dk, t] = x[t, dk*128 + p]
        xr = np.asarray(x, f32).T.reshape(8, 128, S)
        return np.ascontiguousarray(xr.transpose(1, 0, 2)).astype(bf16)

    def wb_of(w, hs):
        wt = np.asarray(w[hs], f32).transpose(2, 0, 1).reshape(D, EH)
        return np.ascontiguousarray(
            wt.reshape(8, 128, EH).transpose(1, 0, 2)).astype(bf16)

    xq_c = {}
    for core in range(NC):
        b, g = divmod(core, G)
        hs = slice(g * HPC, (g + 1) * HPC)
        if b not in xq_c:
            xq_c[b] = (xb_of(x_q[b]), xb_of(x_k[b]), xb_of(x_v[b]))
        xqb, xkb, xvb = xq_c[b]
        im = {
            "xqb": xqb, "xkb": xkb, "xvb": xvb,
            "wqb": wb_of(wq, hs),
            "wkb": wb_of(wk, hs),
            "wvb": wb_of(wv, hs),
            "trilb": trilb,
        }
        woT = np.asarray(wo[:, g * EH:(g + 1) * EH], f32).T   # [EH, D]
        im["wob"] = np.ascontiguousarray(
            woT.reshape(4, 128, D).transpose(1, 0, 2)).astype(bf16)
        for name, bb in (("bq", bq), ("bk", bk)):
            flat = np.asarray(bb[hs], f32).reshape(EH)
            im[name] = np.ascontiguousarray(flat.reshape(4, 128).T)
        im["bv"] = np.broadcast_to(
            np.asarray(bv[hs], f32).reshape(1, HPC, DH), (128, HPC, DH)).copy()
        in_maps.append(im)
    return in_maps


# ======================================================================
# dense-mask fallback: original fp32r kernel
# ======================================================================

def _build_dense(k_needed, k_full, mixed):
    nc = bacc.Bacc("TRN2", target_bir_lowering=False, debug=False, num_devices=NC)

    xqT_d = nc.dram_tensor("xqT", [D, S], F32R, kind="ExternalInput").ap()
    xkT_d = nc.dram_tensor("xkT", [D, S], F32R, kind="ExternalInput").ap()
    xvT_d = nc.dram_tensor("xvT", [D, S], F32R, kind="ExternalInput").ap()
    wqT_d = nc.dram_tensor("wqT", [128, 8, EH], F32R, kind="ExternalInput").ap()
    wkT_d = nc.dram_tensor("wkT", [128, 8, EH], F32R, kind="ExternalInput").ap()
    wvT_d = nc.dram_tensor("wvT", [128, 8, EH], F32R, kind="ExternalInput").ap()
    bq_d = nc.dram_tensor("bq", [128, 4], F32, kind="ExternalInput").ap()
    bk_d = nc.dram_tensor("bk", [128, 4], F32, kind="ExternalInput").ap()
    bv_d = nc.dram_tensor("bv", [128, EH], F32, kind="ExternalInput").ap()
    woT_d = nc.dram_tensor("woT", [128, 4, D], F32R, kind="ExternalInput").ap()
    ones_d = nc.dram_tensor("ones1", [128, 1], F32R, kind="ExternalInput").ap()
    mT_d = nc.dram_tensor("maskT", [S, S], F32R, kind="ExternalInput").ap()
    mT_v = mT_d.rearrange("(kt p) q -> p kt q", p=128)
    y_d = nc.dram_tensor("y", [S, D], F32, kind="ExternalOutput").ap()

    xq_v = xqT_d.rearrange("(dk p) t -> p dk t", p=128)
    xk_v = xkT_d.rearrange("(dk p) t -> p dk t", p=128)
    xv_v = xvT_d.rearrange("(dk p) t -> p dk t", p=128)

    with tile.TileContext(nc) as tc:
        with nc.allow_low_precision(reason="fp32r storage has fp32 width"):
            _body_dense(nc, tc, k_needed, mixed,
                        xq_v, xk_v, xv_v, wqT_d, wkT_d, wvT_d,
                        bq_d, bk_d, bv_d, woT_d, ones_d, mT_v, y_d)
    nc.compile()
    return nc


def _body_dense(nc, tc, k_needed, mixed,
                xq_v, xk_v, xv_v, wqT_d, wkT_d, wvT_d,
                bq_d, bk_d, bv_d, woT_d, ones_d, mT_v, y_d):
    pers_cm = tc.tile_pool(name="pers", bufs=1)
    pers = pers_cm.__enter__()
    KT = pers.tile([128, 4, S], F32R)
    V65 = pers.tile([128, NKT, HPC, 65], F32R)
    wo_t = pers.tile([128, 4, D], F32R)
    wv_t = pers.tile([128, 8, EH], F32R)
    bq_t = pers.tile([128, 4], F32)
    bk_t = pers.tile([128, 4], F32)
    bv_t = pers.tile([128, EH], F32)
    ones_t = pers.tile([128, 1], F32R)
    nc.sync.dma_start(bq_t[:], bq_d)
    nc.sync.dma_start(bk_t[:], bk_d)
    nc.sync.dma_start(bv_t[:], bv_d)
    nc.sync.dma_start(ones_t[:], ones_d)
    nc.sync.dma_start(wv_t[:], wvT_d)
    nc.vector.tensor_copy(V65[:, :, :, 64:65],
                          ones_t[:, 0:1].to_broadcast([128, NKT, HPC, 1]))

    pw_cm = tc.tile_pool(name="pw", bufs=2)
    pw = pw_cm.__enter__()
    px_cm = tc.tile_pool(name="px", bufs=3)
    px = px_cm.__enter__()
    pq_cm = tc.tile_pool(name="pq", bufs=2)
    pq = pq_cm.__enter__()
    pcw_cm = tc.tile_pool(name="pcw", bufs=3)
    pcw = pcw_cm.__enter__()
    ppt_cm = tc.tile_pool(name="ppt", bufs=2)
    ppt = ppt_cm.__enter__()
    pnrm_cm = tc.tile_pool(name="pnrm", bufs=1)
    pnrm = pnrm_cm.__enter__()
    py_cm = tc.tile_pool(name="py", bufs=2)
    py = py_cm.__enter__()
    pp_cm = tc.tile_pool(name="pp", bufs=2, space="PSUM")
    pp = pp_cm.__enter__()
    psc_cm = tc.tile_pool(name="psc", bufs=2, space="PSUM")
    psc = psc_cm.__enter__()
    pav_cm = tc.tile_pool(name="pav", bufs=1, space="PSUM")
    pav = pav_cm.__enter__()

    qwin = {}
    ctxw = {}
    state = {}

    def qk_chunks(tq):
        def load(w_d, x_v, kind, half):
            def f():
                hs = slice(half * 4, half * 4 + 4)
                w_t = pw.tile([128, 4, EH], F32R, tag="w", name=f"w_{kind}{tq}{half}")
                nc.sync.dma_start(w_t[:], w_d[:, hs, :])
                x_t = px.tile([128, 4, 512], F32R, tag="x", name=f"x_{kind}{tq}{half}")
                nc.sync.dma_start(x_t[:], x_v[:, hs, tq * 512:(tq + 1) * 512])
                state[f"w{half}"], state[f"x{half}"] = w_t, x_t
                if kind == "q" and half == 0:
                    qwin[tq] = pq.tile([128, 4, 512], F32R, tag="qw", name=f"qw{tq}")
            return f

        def mmgroup(et, kind):
            def f():
                ps_t = pp.tile([128, 512], F32, tag="pj", name=f"ps_{kind}{tq}_{et}")
                for dk in range(8):
                    w_t = state[f"w{dk // 4}"]
                    x_t = state[f"x{dk // 4}"]
                    nc.tensor.matmul(ps_t[:],
                                     w_t[:, dk % 4, et * 128:(et + 1) * 128],
                                     x_t[:, dk % 4, :],
                                     start=(dk == 0), stop=(dk == 7))
                if kind == "q":
                    nc.vector.tensor_tensor(
                        qwin[tq][:, et, :], ps_t[:],
                        bq_t[:, et:et + 1].to_broadcast([128, 512]), AL.add)
                else:
                    nc.vector.tensor_tensor(
                        KT[:, et, tq * 512:(tq + 1) * 512], ps_t[:],
                        bk_t[:, et:et + 1].to_broadcast([128, 512]), AL.add)
            return f

        out = []
        for kind, w_d, x_v in (("q", wqT_d, xq_v), ("k", wkT_d, xk_v)):
            for half in (0, 1):
                g = load(w_d, x_v, kind, half)
                g.mms = 0
                out.append(g)
            for et in range(4):
                g = mmgroup(et, kind)
                g.mms = 8
                out.append(g)
        return out

    def v_chunks(tv):
        def load(half):
            def f():
                hs = slice(half * 4, half * 4 + 4)
                x_t = px.tile([128, 4, 512], F32R, tag="x", name=f"x_v{tv}{half}")
                nc.sync.dma_start(x_t[:], xv_v[:, hs, tv * 512:(tv + 1) * 512])
                state[f"x{half}"] = x_t
            return f

        def mmgroup(tl):
            def f():
                tt = tv * 4 + tl
                ps_t = pp.tile([128, 512], F32, tag="pj", name=f"ps_v{tt}")
                for dk in range(8):
                    x_t = state[f"x{dk // 4}"]
                    nc.tensor.matmul(ps_t[:],
                                     x_t[:, dk % 4, tl * 128:(tl + 1) * 128],
                                     wv_t[:, dk, :],
                                     start=(dk == 0), stop=(dk == 7))
                nc.vector.tensor_tensor(
                    V65[:, tt, :, 0:64],
                    ps_t.rearrange("p (h e) -> p h e", h=HPC),
                    bv_t.rearrange("p (h e) -> p h e", h=HPC), AL.add)
            return f

        out = []
        for half in (0, 1):
            g = load(half)
            g.mms = 0
            out.append(g)
        for tl in range(4):
            g = mmgroup(tl)
            g.mms = 8
            out.append(g)
        return out

    def p3_chunks(qt):
        p3state = {}

        def half_a(tl, mc):
            def f():
                cw = ctxw[qt]
                tt = qt * 4 + tl
                ps_t = pp.tile([128, 512], F32, tag="pj", name=f"ps_o{tt}_{mc}")
                p3state[(tl, mc)] = ps_t
                for hp in range(2):
                    nc.tensor.matmul(ps_t[:],
                                     cw[:, hp, tl * 128:(tl + 1) * 128],
                                     wo_t[:, hp, mc * 512:(mc + 1) * 512],
                                     start=(hp == 0), stop=False)
            f.mms = 2
            return f

        def half_b(tl, mc):
            def f():
                cw = ctxw[qt]
                tt = qt * 4 + tl
                ps_t = p3state.pop((tl, mc))
                for hp in range(2, 4):
                    nc.tensor.matmul(ps_t[:],
                                     cw[:, hp, tl * 128:(tl + 1) * 128],
                                     wo_t[:, hp, mc * 512:(mc + 1) * 512],
                                     start=False, stop=(hp == 3))
                y_t = py.tile([128, 512], F32, tag="y", name=f"y{tt}_{mc}")
                nc.vector.tensor_copy(y_t[:], ps_t[:])
                nc.sync.dma_start(
                    y_d[tt * 128:(tt + 1) * 128, mc * 512:(mc + 1) * 512],
                    y_t[:])
            f.mms = 2
            return f

        out = []
        for tl in range(4):
            for mc in range(2):
                out.append(half_a(tl, mc))
                out.append(half_b(tl, mc))
        return out

    def window(qt, work):
        if not INTERLEAVE:
            for f in work:
                f()
            work = []
        klim = k_needed[qt]
        q0 = qt * 512
        n_tiles = 4 * klim
        cw = pcw.tile([128, 4, 512], F32R, tag="cw", name=f"cw{qt}")
        ctxw[qt] = cw
        done = 0
        wi = 0
        total_mms = sum(getattr(f, "mms", 4) for f in work) or 1
        emitted = 0
        qw = qwin[qt]
        for hp in range(4):
            av0 = pav.tile([65, 512], F32, tag="av0")
            av1 = pav.tile([65, 512], F32, tag="av1")
            first = True
            for kk in range(klim):
                straddle = (qt, kk) in mixed
                s_t = psc.tile([128, 2, 512], F32, tag="sc")
                for j in range(2):
                    nc.tensor.matmul(
                        s_t[:, j, :],
                        KT[j * 64:(j + 1) * 64, hp, kk * 128:(kk + 1) * 128],
                        qw[j * 64:(j + 1) * 64, hp, :],
                        start=True, stop=True, tile_position=(j * 64, 0))
                p_t = ppt.tile([128, 2, 512], F32R, tag="pt")
                nc.scalar.activation(p_t[:], s_t[:], AF.Exp, scale=float(SCALE))
                if straddle:
                    sel_t = ppt.tile([128, 512], F32R, tag="sel")
                    nc.sync.dma_start(sel_t[:], mT_v[:, kk, q0:q0 + 512])
                    nc.vector.tensor_tensor(
                        p_t[:], p_t[:],
                        sel_t[:, None, :].to_broadcast([128, 2, 512]),
                        AL.mult)
                for j, av in ((0, av0), (1, av1)):
                    nc.tensor.matmul(av[:], V65[:, kk, 2 * hp + j, :],
                                     p_t[:, j, :],
                                     start=first, stop=(kk == klim - 1))
                first = False
                done += 1
                if INTERLEAVE:
                    target = done * total_mms / n_tiles
                    while wi < len(work) and emitted < target:
                        emitted += getattr(work[wi], "mms", 4)
                        work[wi]()
                        wi += 1
            avc = pnrm.tile([128, 512], F32, tag="avc", bufs=1)
            lin = pnrm.tile([1, 2, 512], F32, tag="lin")
            nc.vector.tensor_copy(avc[0:64, :], av0[0:64, :])
            nc.vector.tensor_copy(avc[64:128, :], av1[0:64, :])
            nc.vector.tensor_copy(lin[:, 0, :], av0[64:65, :])
            nc.vector.tensor_copy(lin[:, 1, :], av1[64:65, :])
            lrec = pnrm.tile([1, 2, 512], F32, tag="lrec")
            scr = pnrm.tile([1, 512], F32, tag="scr")
            nc.vector.reciprocal_approx_accurate(lrec[:, 0, :], lin[:, 0, :], scr[:])
            nc.vector.reciprocal_approx_accurate(lrec[:, 1, :], lin[:, 1, :], scr[:])
            rec_bc = pnrm.tile([128, 2, 512], F32, tag="rbc")
            nc.gpsimd.partition_broadcast(rec_bc[:], lrec[0:1, :, :])
            nc.vector.tensor_tensor(cw[0:64, hp, :],
                                    avc[0:64, :], rec_bc[0:64, 0, :], AL.mult)
            nc.vector.tensor_tensor(cw[64:128, hp, :],
                                    avc[64:128, :], rec_bc[64:128, 1, :], AL.mult)
        while wi < len(work):
            work[wi]()
            wi += 1

    for f in qk_chunks(0):
        f()
    for f in v_chunks(0):
        f()

    def wo_load():
        nc.sync.dma_start(wo_t[:], woT_d)
    wo_load.mms = 0

    for qt in range(NQT):
        work = []
        if qt == 0:
            work.append(wo_load)
        if qt + 1 < NQT:
            work += qk_chunks(qt + 1)
            work += v_chunks(qt + 1)
        if qt == 2:
            work += p3_chunks(0)
        elif qt == 3:
            work += p3_chunks(1)
            work += p3_chunks(2)
        window(qt, work)
    for f in p3_chunks(NQT - 1):
        f()

    for cm in (pav_cm, psc_cm, pp_cm, py_cm, pnrm_cm, ppt_cm, pcw_cm, pq_cm,
               px_cm, pw_cm, pers_cm):
        cm.__exit__(None, None, None)


def _prep_inputs_dense(x_q, x_k, x_v, mask, wq, wk, wv, bq, bk, bv, wo):
    f32 = np.float32
    in_maps = []
    ones1 = np.ones((128, 1), f32)
    for core in range(NC):
        b, g = divmod(core, G)
        hs = slice(g * HPC, (g + 1) * HPC)
        im = {
            "xqT": np.ascontiguousarray(np.asarray(x_q[b], f32).T),
            "xkT": np.ascontiguousarray(np.asarray(x_k[b], f32).T),
            "xvT": np.ascontiguousarray(np.asarray(x_v[b], f32).T),
            "ones1": ones1,
        }
        for name, w in (("wqT", wq), ("wkT", wk), ("wvT", wv)):
            wt = np.asarray(w[hs], f32).transpose(2, 0, 1).reshape(D, EH)
            im[name] = np.ascontiguousarray(wt.reshape(8, 128, EH))\
                .transpose(1, 0, 2).copy()
        for name, bb in (("bq", bq), ("bk", bk)):
            flat = np.asarray(bb[hs], f32).reshape(EH)
            im[name] = np.ascontiguousarray(flat.reshape(4, 128).T)
        im["bv"] = np.broadcast_to(np.asarray(bv[hs], f32).reshape(1, EH),
                                   (128, EH)).copy()
        woT = np.asarray(wo[:, g * EH:(g + 1) * EH], f32).T
        im["woT"] = np.ascontiguousarray(woT.reshape(4, 128, D))\
            .transpose(1, 0, 2).copy()
        im["maskT"] = np.ascontiguousarray(np.asarray(mask[b], f32).T)
        in_maps.append(im)
    return in_maps


# ======================================================================
# mask analysis + dispatch
# ======================================================================

def _analyze_mask(mask):
    m = np.asarray(mask)
    iota = np.arange(S)
    n = m.sum(axis=2)
    causal = bool((n == iota[None, :] + 1).all()) and \
        bool((m == (iota[None, None, :] < n[..., None])).all())
    allones = bool((m == 1).all())

    k_needed, k_full, mixed = [], [], set()
    if allones:
        mode = "affine"
        k_needed = [NKT] * NQT
        k_full = [NKT] * NQT
    elif causal:
        mode = "affine"
        for qt in range(NQT):
            k_needed.append(4 * qt + 4)
            k_full.append(4 * qt)
            for kk in range(4 * qt, 4 * qt + 4):
                mixed.add((qt, kk))
    else:
        mode = "dense"
        for qt in range(NQT):
            sl = m[:, qt * 512:(qt + 1) * 512, :]
            need = 0
            full = NKT
            for kk in range(NKT):
                blk = sl[:, :, kk * 128:(kk + 1) * 128]
                if blk.any():
                    need = kk + 1
                if not blk.all():
                    full = min(full, kk)
            need = max(need, 1)
            if mode == "dense":
                # keep k-tile count even for pairing safety (harmless extra)
                pass
            full = min(full, need)
            k_needed.append(need)
            k_full.append(full)
            for kk in range(full, need):
                blk = sl[:, :, kk * 128:(kk + 1) * 128]
                if not blk.all():
                    mixed.add((qt, kk))
    return mode, tuple(k_needed), tuple(k_full), frozenset(mixed)


def _run(x_q, x_k, x_v, mask, wq, wk, wv, bq, bk, bv, wo, bo,
         trace=False, trace_cores=None):
    mode, k_needed, k_full, mixed = _analyze_mask(mask)
    key = (mode, k_needed, k_full, mixed)
    if key not in _cache:
        if mode == "affine":
            _cache[key] = _build_v2(k_needed, mixed)
        else:
            _cache[key] = _build_dense(k_needed, k_full, mixed)
    nc = _cache[key]
    if mode == "affine":
        in_maps = _prep_inputs_v2(x_q, x_k, x_v, wq, wk, wv, bq, bk, bv, wo)
    else:
        in_maps = _prep_inputs_dense(x_q, x_k, x_v, mask, wq, wk, wv,
                                     bq, bk, bv, wo)
    res = run_bass_kernel_spmd(nc, in_maps, core_ids=list(range(NC)),
                               trace=trace, trace_cores=trace_cores)
    bo = np.asarray(bo, np.float32)
    out = np.empty((B, S, D), np.float32)
    for b in range(B):
        out[b] = (np.asarray(res.results[2 * b]["y"], np.float32)
                  + np.asarray(res.results[2 * b + 1]["y"], np.float32) + bo)
    return out, res


def kernel(x_q, x_k, x_v, mask, wq, wk, wv, bq, bk, bv, wo, bo):
    out, _ = _run(x_q, x_k, x_v, mask, wq, wk, wv, bq, bk, bv, wo, bo)
    return out


# revision 21
# speedup vs baseline: 1.6042x; 1.0025x over previous
"""MultiHeadAttention (B=4, S=2048, d_model=1024, H=16, dh=64) on 8 trn2 cores.

Sharding: core (b, g) = batch b in 0..3, head-group g in 0..1 (8 heads each).

Causal path (v3): scores/Q/K stay f32r (full precision, head-pair row-packed
score matmuls); x / w / wo / V / P / ctx / y are bf16 (halves DMA 55->20 MB
per core, enables FWL fast weight loads; ~0.2% rms per tensor, fine for the
2e-2 gate). Weights are loaded once (the old kernel re-streamed wq/wk per
q-window). Softmax exp alternates 4:4 between ACT (true exp, bf16 out) and
DVE (Schraudolph: tensor_scalar A*s+B -> int16 -> bitcast bf16, ~1.7% rms on
that half of P), which breaks the old ACT-only ~1.15us/tile softmax cadence.
Causal straddle tiles use one gpsimd affine_select restricted to the
[0:qoff+128) prefix (the only region with masked columns) - keeping gpsimd in
a single ucode library; a memset/tril-TT variant thrashed MODIFY_POOL_CONFIG
and cost ~14us per window. Rowsum rides the AV matmul as a 65th V column; av
psum is drained immediately (2 DVE copies + 1 ACT copy), then
reciprocal_approx_fast + partition_broadcast + the normalize TTs run with
scheduling slack (ctx is consumed a whole window later). y staged via ACT
copies (bf16) and summed across the two head-group cores on host.

Dense-mask path: original f32r kernel (unchanged).
"""
import sys
sys.path.insert(0, "/opt/trn_rl_repo")

import os
import numpy as np
import ml_dtypes

import concourse.bass as bass
import concourse.mybir as mybir
import concourse.tile as tile
from concourse import bacc
from concourse.bass_utils import run_bass_kernel_spmd

F32 = mybir.dt.float32
F32R = mybir.dt.float32r
F8 = mybir.dt.float8e4
I8 = mybir.dt.int8
AF = mybir.ActivationFunctionType
AL = mybir.AluOpType
PM = mybir.MatmulPerfMode

NP_F8 = ml_dtypes.float8_e4m3

INTERLEAVE = os.environ.get("KNOILV") != "1"
EXP_DVE8 = int(os.environ.get("KEXPDVE8", "4"))   # of 8 exp ops, how many on DVE
EXPB = float(os.environ.get("KEXPB", "16248.6"))  # Schraudolph int16/bf16 bias
BAND_GP = os.environ.get("KBANDGP", "1") == "1"   # tril band mask on gpsimd
Y_COPY = os.environ.get("KYCOPY") == "1"          # stage y in SBUF vs psum DMA

B, S, D, H, DH = 4, 2048, 1024, 16, 64
NC = 8
G = 2              # head groups (cores per batch)
HPC = H // G       # 8 heads per core
EH = HPC * DH      # 512
NQT = S // 512     # 4 q-tiles
NKT = S // 128     # 16 k-tiles
NKT2 = S // 256    # 8 k256-tiles
SCALE = 1.0 / np.sqrt(DH)
LOG2E = 1.4426950408889634

_cache = {}
MM_NAMES = {"sc0": set(), "sc1": set(), "av": set(), "pj": set(), "p3": set()}


# ======================================================================
# v2 causal/affine path
# ======================================================================

def _build_v2(k_needed, mixed):
    """k_needed[qt]: leading k128-tiles per q-tile (even). mixed: straddle set."""
    nc = bacc.Bacc("TRN2", target_bir_lowering=False, debug=False, num_devices=NC)

    BF = mybir.dt.bfloat16
    xqb_d = nc.dram_tensor("xqb", [128, 8, S], BF, kind="ExternalInput").ap()
    xkb_d = nc.dram_tensor("xkb", [128, 8, S], BF, kind="ExternalInput").ap()
    xvb_d = nc.dram_tensor("xvb", [128, 8, S], BF, kind="ExternalInput").ap()
    wqb_d = nc.dram_tensor("wqb", [128, 8, EH], BF, kind="ExternalInput").ap()
    wkb_d = nc.dram_tensor("wkb", [128, 8, EH], BF, kind="ExternalInput").ap()
    wvb_d = nc.dram_tensor("wvb", [128, 8, EH], BF, kind="ExternalInput").ap()
    wob_d = nc.dram_tensor("wob", [128, 4, D], BF, kind="ExternalInput").ap()
    bq_d = nc.dram_tensor("bq", [128, 4], F32, kind="ExternalInput").ap()
    bk_d = nc.dram_tensor("bk", [128, 4], F32, kind="ExternalInput").ap()
    bv_d = nc.dram_tensor("bv", [128, HPC, DH], F32, kind="ExternalInput").ap()
    tril_d = nc.dram_tensor("trilb", [128, 128], BF, kind="ExternalInput").ap()
    y_d = nc.dram_tensor("y", [S, D], mybir.dt.bfloat16, kind="ExternalOutput").ap()

    with tile.TileContext(nc) as tc:
        with nc.allow_low_precision(reason="bf16/f32r attention within 2e-2 gate"):
            _body_v2(nc, tc, k_needed, mixed,
                     xqb_d, xkb_d, xvb_d, wqb_d, wkb_d, wvb_d, wob_d,
                     bq_d, bk_d, bv_d, tril_d, y_d)
    nc.compile()
    return nc


def _body_v2(nc, tc, k_needed, mixed,
             xqb_d, xkb_d, xvb_d, wqb_d, wkb_d, wvb_d, wob_d,
             bq_d, bk_d, bv_d, tril_d, y_d):
    BF = mybir.dt.bfloat16
    I16 = mybir.dt.int16
    # Schraudolph exp in bf16 bit space: i16 = round(A16*s + B16); bitcast bf16
    A16 = float(SCALE * 128.0 * LOG2E)
    B16 = float(EXPB)

    pers_cm = tc.tile_pool(name="pers", bufs=1)
    pers = pers_cm.__enter__()
    KT = pers.tile([128, 4, S], F32R)             # [eh%128, et, t]
    V65 = pers.tile([128, NKT, HPC, 65], BF)      # [t%128, kk, h, e|1]
    wqb_t = pers.tile([128, 8, EH], BF)
    wkb_t = pers.tile([128, 8, EH], BF)
    wvb_t = pers.tile([128, 8, EH], BF)
    wob_t = pers.tile([128, 4, D], BF)
    bq_t = pers.tile([128, 4], F32)
    bk_t = pers.tile([128, 4], F32)
    bv_t = pers.tile([128, HPC, DH], F32)
    tril_t = pers.tile([128, 128], BF)
    nc.sync.dma_start(bq_t[:], bq_d)
    nc.sync.dma_start(bk_t[:], bk_d)
    nc.sync.dma_start(bv_t[:], bv_d)
    nc.sync.dma_start(tril_t[:], tril_d)
    nc.sync.dma_start(wqb_t[:], wqb_d)
    nc.sync.dma_start(wkb_t[:], wkb_d)
    nc.sync.dma_start(wvb_t[:], wvb_d)
    nc.vector.memset(V65[:, :, :, 64:65], 1.0)

    px_cm = tc.tile_pool(name="px", bufs=3)
    px = px_cm.__enter__()
    pq_cm = tc.tile_pool(name="pq", bufs=2)
    pq = pq_cm.__enter__()
    pcw_cm = tc.tile_pool(name="pcw", bufs=3)
    pcw = pcw_cm.__enter__()
    ppt_cm = tc.tile_pool(name="ppt", bufs=3)
    ppt = ppt_cm.__enter__()
    pnrm_cm = tc.tile_pool(name="pnrm", bufs=2)
    pnrm = pnrm_cm.__enter__()
    py_cm = tc.tile_pool(name="py", bufs=2)
    py = py_cm.__enter__()
    pp_cm = tc.tile_pool(name="pp", bufs=2, space="PSUM")
    pp = pp_cm.__enter__()
    psc_cm = tc.tile_pool(name="psc", bufs=2, space="PSUM")
    psc = psc_cm.__enter__()
    pav_cm = tc.tile_pool(name="pav", bufs=1, space="PSUM")
    pav = pav_cm.__enter__()

    qwin = {}    # tq -> [128, 4, 512] Q^T window tile (f32r)
    ctxw = {}    # qt -> [128, 2, 2, 512] fp8 interleaved ctx tile
    state = {}
    expctr = [0]

    # ---- projection chunks ----
    def qk_chunks(tq):
        def load():
            xq_t = px.tile([128, 8, 512], BF, tag="x", name=f"xq{tq}")
            nc.sync.dma_start(xq_t[:], xqb_d[:, :, tq * 512:(tq + 1) * 512])
            xk_t = px.tile([128, 8, 512], BF, tag="x", name=f"xk{tq}")
            nc.sync.dma_start(xk_t[:], xkb_d[:, :, tq * 512:(tq + 1) * 512])
            state["xq"], state["xk"] = xq_t, xk_t
            qwin[tq] = pq.tile([128, 4, 512], F32R, tag="qw", name=f"qw{tq}")
        load.mms = 0

        def mmgroup(et, kind):
            def f():
                w_t = wqb_t if kind == "q" else wkb_t
                x_t = state["xq" if kind == "q" else "xk"]
                ps_t = pp.tile([128, 512], F32, tag="pj", name=f"ps_{kind}{tq}_{et}")
                for dk in range(8):
                    mi = nc.tensor.matmul(ps_t[:],
                                          w_t[:, dk, et * 128:(et + 1) * 128],
                                          x_t[:, dk, :],
                                          start=(dk == 0), stop=(dk == 7))
                    MM_NAMES["pj"].add(mi.ins.name)
                if kind == "q":
                    nc.scalar.activation(
                        qwin[tq][:, et, :], ps_t[:], AF.Identity,
                        bias=bq_t[:, et:et + 1], scale=1.0)
                else:
                    nc.scalar.activation(
                        KT[:, et, tq * 512:(tq + 1) * 512], ps_t[:], AF.Identity,
                        bias=bk_t[:, et:et + 1], scale=1.0)
            f.mms = 4
            return f

        out = [load]
        for kind in ("q", "k"):
            for et in range(4):
                out.append(mmgroup(et, kind))
        return out

    def v_chunks(tv):
        def load():
            xv_t = px.tile([128, 8, 512], BF, tag="x", name=f"xv{tv}")
            nc.sync.dma_start(xv_t[:], xvb_d[:, :, tv * 512:(tv + 1) * 512])
            state["xv"] = xv_t
        load.mms = 0

        def mmgroup(tl):
            def f():
                x_t = state["xv"]
                tt = tv * 4 + tl
                ps_t = pp.tile([128, 512], F32, tag="pj", name=f"ps_v{tt}")
                for dk in range(8):
                    mi = nc.tensor.matmul(ps_t[:],
                                          x_t[:, dk, tl * 128:(tl + 1) * 128],
                                          wvb_t[:, dk, :],
                                          start=(dk == 0), stop=(dk == 7))
                    MM_NAMES["pj"].add(mi.ins.name)
                nc.vector.tensor_tensor(
                    V65[:, tt, :, 0:64],
                    ps_t.rearrange("p (h e) -> p h e", h=HPC),
                    bv_t[:], AL.add)
            f.mms = 4
            return f

        out = [load]
        for tl in range(4):
            out.append(mmgroup(tl))
        return out

    def p3_chunks(qt):
        p3state = {}

        def half_a(tl, mc):
            def f():
                cw = ctxw[qt]
                tt = qt * 4 + tl
                ps_t = pp.tile([128, 512], F32, tag="pj", name=f"ps_o{tt}_{mc}")
                p3state[(tl, mc)] = ps_t
                for hp in range(2):
                    mi = nc.tensor.matmul(ps_t[:],
                                          cw[:, hp, tl * 128:(tl + 1) * 128],
                                          wob_t[:, hp, mc * 512:(mc + 1) * 512],
                                          start=(hp == 0), stop=False)
                    MM_NAMES["p3"].add(mi.ins.name)
            f.mms = 2
            return f

        def half_b(tl, mc):
            def f():
                cw = ctxw[qt]
                tt = qt * 4 + tl
                ps_t = p3state.pop((tl, mc))
                for hp in range(2, 4):
                    mi = nc.tensor.matmul(ps_t[:],
                                          cw[:, hp, tl * 128:(tl + 1) * 128],
                                          wob_t[:, hp, mc * 512:(mc + 1) * 512],
                                          start=False, stop=(hp == 3))
                    MM_NAMES["p3"].add(mi.ins.name)
                y_t = py.tile([128, 512], BF, tag="y", name=f"y{tt}_{mc}")
                if Y_COPY:
                    nc.vector.tensor_copy(y_t[:], ps_t[:])
                else:
                    nc.scalar.copy(y_t[:], ps_t[:])
                nc.sync.dma_start(
                    y_d[tt * 128:(tt + 1) * 128, mc * 512:(mc + 1) * 512],
                    y_t[:])
            f.mms = 2
            return f

        out = []
        for tl in range(4):
            for mc in range(2):
                out.append(half_a(tl, mc))
                out.append(half_b(tl, mc))
        return out

    # ---- attention window ----
    def window(qt, work):
        if not INTERLEAVE:
            for f in work:
                f()
            work = []
        klim = k_needed[qt]          # in k128 units
        q0 = qt * 512
        n_units = 4 * klim
        cw = pcw.tile([128, 4, 512], BF, tag="cw", name=f"cw{qt}")
        ctxw[qt] = cw
        done = 0
        wi = 0
        total_mms = sum(getattr(f, "mms", 1) for f in work) or 1
        emitted = 0
        qw = qwin[qt]
        for hp in range(4):
            av = pav.tile([65, 2, 512], F32, tag="av")
            for kk in range(klim):
                straddle = (qt, kk) in mixed
                qoff = max(0, kk * 128 - q0) if straddle else 0
                moff = 128 if qoff >= 128 else 0
                s_t = psc.tile([128, 2, 512], F32, tag="sc")
                for j in range(2):
                    mi = nc.tensor.matmul(
                        s_t[:, j, moff:512],
                        KT[j * 64:(j + 1) * 64, hp, kk * 128:(kk + 1) * 128],
                        qw[j * 64:(j + 1) * 64, hp, moff:512],
                        start=True, stop=True, tile_position=(j * 64, 0))
                    MM_NAMES[f"sc{j}"].add(mi.ins.name)
                p_t = ppt.tile([128, 2, 512], BF, tag="pt")
                use_dve = (expctr[0] % 8) < EXP_DVE8
                expctr[0] += 1
                if use_dve:
                    p_i = p_t[:].bitcast(I16)
                    nc.vector.tensor_scalar(
                        p_i[:, :, qoff:512], s_t[:, :, qoff:512],
                        A16, B16, op0=AL.mult, op1=AL.add)
                else:
                    nc.scalar.activation(p_t[:, :, qoff:512],
                                         s_t[:, :, qoff:512],
                                         AF.Exp, scale=float(SCALE))
                if straddle:
                    # zero q < qoff+p: fully-masked prefix plus diagonal band,
                    # restricted to [0:qoff+128) (beyond is all-keep)
                    w = qoff + 128
                    nc.gpsimd.affine_select(
                        p_t[:, :, 0:w], p_t[:, :, 0:w],
                        pattern=[[0, 2], [1, w]],
                        compare_op=AL.is_ge, fill=0.0,
                        base=q0 - kk * 128, channel_multiplier=-1)
                for j in range(2):
                    mi = nc.tensor.matmul(av[:, j, :],
                                          V65[:, kk, 2 * hp + j, :],
                                          p_t[:, j, :],
                                          start=(kk == 0), stop=(kk == klim - 1))
                    MM_NAMES["av"].add(mi.ins.name)
                done += 1
                if INTERLEAVE:
                    target = done * total_mms / n_units
                    while wi < len(work) and emitted < target:
                        emitted += getattr(work[wi], "mms", 1)
                        work[wi]()
                        wi += 1
            # ---- normalize: drain av psum fast, then ctx*(1/l) with slack ----
            avc = pnrm.tile([128, 512], F32, tag="avc")
            nc.vector.tensor_copy(avc[0:64, :], av[0:64, 0, :])
            nc.vector.tensor_copy(avc[64:128, :], av[0:64, 1, :])
            L2 = pnrm.tile([1, 2, 512], F32, tag="lin")
            nc.scalar.copy(L2[:], av[64:65, :, :])
            lrec = pnrm.tile([1, 2, 512], F32, tag="lrec")
            nc.vector.reciprocal_approx_fast(lrec[:], L2[:])
            rec_bc = pnrm.tile([128, 2, 512], F32, tag="rbc")
            nc.gpsimd.partition_broadcast(rec_bc[:], lrec[0:1, :, :])
            for j in range(2):
                nc.vector.tensor_tensor(
                    cw[j * 64:(j + 1) * 64, hp, :],
                    avc[j * 64:(j + 1) * 64, :],
                    rec_bc[j * 64:(j + 1) * 64, j, :], AL.mult)
        while wi < len(work):
            work[wi]()
            wi += 1

    # ---- prologue ----
    nc.sync.dma_start(wob_t[:], wob_d)
    for f in qk_chunks(0):
        f()
    for f in v_chunks(0):
        f()

    for qt in range(NQT):
        work = []
        if qt + 1 < NQT:
            work += qk_chunks(qt + 1)
            work += v_chunks(qt + 1)
        if qt == 2:
            work += p3_chunks(0)
        elif qt == 3:
            work += p3_chunks(1)
            work += p3_chunks(2)
        window(qt, work)
    for f in p3_chunks(NQT - 1):
        f()

    for cm in (pav_cm, psc_cm, pp_cm, py_cm, pnrm_cm, ppt_cm, pcw_cm, pq_cm,
               px_cm, pers_cm):
        cm.__exit__(None, None, None)


def _prep_inputs_v2(x_q, x_k, x_v, wq, wk, wv, bq, bk, bv, wo):
    f32 = np.float32
    bf16 = ml_dtypes.bfloat16
    trilb = np.triu(np.ones((128, 128), f32)).astype(bf16)
    in_maps = []

    def xb_of(x):
        # [128, 8, S]: xb[p, dk, t] = x[t, dk*128 + p]
        xr = np.asarray(x, f32).T.reshape(8, 128, S)
        return np.ascontiguousarray(xr.transpose(1, 0, 2)).astype(bf16)

    def wb_of(w, hs):
        wt = np.asarray(w[hs], f32).transpose(2, 0, 1).reshape(D, EH)
        return np.ascontiguousarray(
            wt.reshape(8, 128, EH).transpose(1, 0, 2)).astype(bf16)

    xq_c = {}
    for core in range(NC):
        b, g = divmod(core, G)
        hs = slice(g * HPC, (g + 1) * HPC)
        if b not in xq_c:
            xq_c[b] = (xb_of(x_q[b]), xb_of(x_k[b]), xb_of(x_v[b]))
        xqb, xkb, xvb = xq_c[b]
        im = {
            "xqb": xqb, "xkb": xkb, "xvb": xvb,
            "wqb": wb_of(wq, hs),
            "wkb": wb_of(wk, hs),
            "wvb": wb_of(wv, hs),
            "trilb": trilb,
        }
        woT = np.asarray(wo[:, g * EH:(g + 1) * EH], f32).T   # [EH, D]
        im["wob"] = np.ascontiguousarray(
            woT.reshape(4, 128, D).transpose(1, 0, 2)).astype(bf16)
        for name, bb in (("bq", bq), ("bk", bk)):
            flat = np.asarray(bb[hs], f32).reshape(EH)
            im[name] = np.ascontiguousarray(flat.reshape(4, 128).T)
        im["bv"] = np.broadcast_to(
            np.asarray(bv[hs], f32).reshape(1, HPC, DH), (128, HPC, DH)).copy()
        in_maps.append(im)
    return in_maps


# ======================================================================
# dense-mask fallback: original fp32r kernel
# ======================================================================

def _build_dense(k_needed, k_full, mixed):
    nc = bacc.Bacc("TRN2", target_bir_lowering=False, debug=False, num_devices=NC)

    xqT_d = nc.dram_tensor("xqT", [D, S], F32R, kind="ExternalInput").ap()
    xkT_d = nc.dram_tensor("xkT", [D, S], F32R, kind="ExternalInput").ap()
    xvT_d = nc.dram_tensor("xvT", [D, S], F32R, kind="ExternalInput").ap()
    wqT_d = nc.dram_tensor("wqT", [128, 8, EH], F32R, kind="ExternalInput").ap()
    wkT_d = nc.dram_tensor("wkT", [128, 8, EH], F32R, kind="ExternalInput").ap()
    wvT_d = nc.dram_tensor("wvT", [128, 8, EH], F32R, kind="ExternalInput").ap()
    bq_d = nc.dram_tensor("bq", [128, 4], F32, kind="ExternalInput").ap()
    bk_d = nc.dram_tensor("bk", [128, 4], F32, kind="ExternalInput").ap()
    bv_d = nc.dram_tensor("bv", [128, EH], F32, kind="ExternalInput").ap()
    woT_d = nc.dram_tensor("woT", [128, 4, D], F32R, kind="ExternalInput").ap()
    ones_d = nc.dram_tensor("ones1", [128, 1], F32R, kind="ExternalInput").ap()
    mT_d = nc.dram_tensor("maskT", [S, S], F32R, kind="ExternalInput").ap()
    mT_v = mT_d.rearrange("(kt p) q -> p kt q", p=128)
    y_d = nc.dram_tensor("y", [S, D], F32, kind="ExternalOutput").ap()

    xq_v = xqT_d.rearrange("(dk p) t -> p dk t", p=128)
    xk_v = xkT_d.rearrange("(dk p) t -> p dk t", p=128)
    xv_v = xvT_d.rearrange("(dk p) t -> p dk t", p=128)

    with tile.TileContext(nc) as tc:
        with nc.allow_low_precision(reason="fp32r storage has fp32 width"):
            _body_dense(nc, tc, k_needed, mixed,
                        xq_v, xk_v, xv_v, wqT_d, wkT_d, wvT_d,
                        bq_d, bk_d, bv_d, woT_d, ones_d, mT_v, y_d)
    nc.compile()
    return nc


def _body_dense(nc, tc, k_needed, mixed,
                xq_v, xk_v, xv_v, wqT_d, wkT_d, wvT_d,
                bq_d, bk_d, bv_d, woT_d, ones_d, mT_v, y_d):
    pers_cm = tc.tile_pool(name="pers", bufs=1)
    pers = pers_cm.__enter__()
    KT = pers.tile([128, 4, S], F32R)
    V65 = pers.tile([128, NKT, HPC, 65], F32R)
    wo_t = pers.tile([128, 4, D], F32R)
    wv_t = pers.tile([128, 8, EH], F32R)
    bq_t = pers.tile([128, 4], F32)
    bk_t = pers.tile([128, 4], F32)
    bv_t = pers.tile([128, EH], F32)
    ones_t = pers.tile([128, 1], F32R)
    nc.sync.dma_start(bq_t[:], bq_d)
    nc.sync.dma_start(bk_t[:], bk_d)
    nc.sync.dma_start(bv_t[:], bv_d)
    nc.sync.dma_start(ones_t[:], ones_d)
    nc.sync.dma_start(wv_t[:], wvT_d)
    nc.vector.tensor_copy(V65[:, :, :, 64:65],
                          ones_t[:, 0:1].to_broadcast([128, NKT, HPC, 1]))

    pw_cm = tc.tile_pool(name="pw", bufs=2)
    pw = pw_cm.__enter__()
    px_cm = tc.tile_pool(name="px", bufs=3)
    px = px_cm.__enter__()
    pq_cm = tc.tile_pool(name="pq", bufs=2)
    pq = pq_cm.__enter__()
    pcw_cm = tc.tile_pool(name="pcw", bufs=3)
    pcw = pcw_cm.__enter__()
    ppt_cm = tc.tile_pool(name="ppt", bufs=2)
    ppt = ppt_cm.__enter__()
    pnrm_cm = tc.tile_pool(name="pnrm", bufs=1)
    pnrm = pnrm_cm.__enter__()
    py_cm = tc.tile_pool(name="py", bufs=2)
    py = py_cm.__enter__()
    pp_cm = tc.tile_pool(name="pp", bufs=2, space="PSUM")
    pp = pp_cm.__enter__()
    psc_cm = tc.tile_pool(name="psc", bufs=2, space="PSUM")
    psc = psc_cm.__enter__()
    pav_cm = tc.tile_pool(name="pav", bufs=1, space="PSUM")
    pav = pav_cm.__enter__()

    qwin = {}
    ctxw = {}
    state = {}

    def qk_chunks(tq):
        def load(w_d, x_v, kind, half):
            def f():
                hs = slice(half * 4, half * 4 + 4)
                w_t = pw.tile([128, 4, EH], F32R, tag="w", name=f"w_{kind}{tq}{half}")
                nc.sync.dma_start(w_t[:], w_d[:, hs, :])
                x_t = px.tile([128, 4, 512], F32R, tag="x", name=f"x_{kind}{tq}{half}")
                nc.sync.dma_start(x_t[:], x_v[:, hs, tq * 512:(tq + 1) * 512])
                state[f"w{half}"], state[f"x{half}"] = w_t, x_t
                if kind == "q" and half == 0:
                    qwin[tq] = pq.tile([128, 4, 512], F32R, tag="qw", name=f"qw{tq}")
            return f

        def mmgroup(et, kind):
            def f():
                ps_t = pp.tile([128, 512], F32, tag="pj", name=f"ps_{kind}{tq}_{et}")
                for dk in range(8):
                    w_t = state[f"w{dk // 4}"]
                    x_t = state[f"x{dk // 4}"]
                    nc.tensor.matmul(ps_t[:],
                                     w_t[:, dk % 4, et * 128:(et + 1) * 128],
                                     x_t[:, dk % 4, :],
                                     start=(dk == 0), stop=(dk == 7))
                if kind == "q":
                    nc.vector.tensor_tensor(
                        qwin[tq][:, et, :], ps_t[:],
                        bq_t[:, et:et + 1].to_broadcast([128, 512]), AL.add)
                else:
                    nc.vector.tensor_tensor(
                        KT[:, et, tq * 512:(tq + 1) * 512], ps_t[:],
                        bk_t[:, et:et + 1].to_broadcast([128, 512]), AL.add)
            return f

        out = []
        for kind, w_d, x_v in (("q", wqT_d, xq_v), ("k", wkT_d, xk_v)):
            for half in (0, 1):
                g = load(w_d, x_v, kind, half)
                g.mms = 0
                out.append(g)
            for et in range(4):
                g = mmgroup(et, kind)
                g.mms = 8
                out.append(g)
        return out

    def v_chunks(tv):
        def load(half):
            def f():
                hs = slice(half * 4, half * 4 + 4)
                x_t = px.tile([128, 4, 512], F32R, tag="x", name=f"x_v{tv}{half}")
                nc.sync.dma_start(x_t[:], xv_v[:, hs, tv * 512:(tv + 1) * 512])
                state[f"x{half}"] = x_t
            return f

        def mmgroup(tl):
            def f():
                tt = tv * 4 + tl
                ps_t = pp.tile([128, 512], F32, tag="pj", name=f"ps_v{tt}")
                for dk in range(8):
                    x_t = state[f"x{dk // 4}"]
                    nc.tensor.matmul(ps_t[:],
                                     x_t[:, dk % 4, tl * 128:(tl + 1) * 128],
                                     wv_t[:, dk, :],
                                     start=(dk == 0), stop=(dk == 7))
                nc.vector.tensor_tensor(
                    V65[:, tt, :, 0:64],
                    ps_t.rearrange("p (h e) -> p h e", h=HPC),
                    bv_t.rearrange("p (h e) -> p h e", h=HPC), AL.add)
            return f

        out = []
        for half in (0, 1):
            g = load(half)
            g.mms = 0
            out.append(g)
        for tl in range(4):
            g = mmgroup(tl)
            g.mms = 8
            out.append(g)
        return out

    def p3_chunks(qt):
        p3state = {}

        def half_a(tl, mc):
            def f():
                cw = ctxw[qt]
                tt = qt * 4 + tl
                ps_t = pp.tile([128, 512], F32, tag="pj", name=f"ps_o{tt}_{mc}")
                p3state[(tl, mc)] = ps_t
                for hp in range(2):
                    nc.tensor.matmul(ps_t[:],
                                     cw[:, hp, tl * 128:(tl + 1) * 128],
                                     wo_t[:, hp, mc * 512:(mc + 1) * 512],
                                     start=(hp == 0), stop=False)
            f.mms = 2
            return f

        def half_b(tl, mc):
            def f():
                cw = ctxw[qt]
                tt = qt * 4 + tl
                ps_t = p3state.pop((tl, mc))
                for hp in range(2, 4):
                    nc.tensor.matmul(ps_t[:],
                                     cw[:, hp, tl * 128:(tl + 1) * 128],
                                     wo_t[:, hp, mc * 512:(mc + 1) * 512],
                                     start=False, stop=(hp == 3))
                y_t = py.tile([128, 512], F32, tag="y", name=f"y{tt}_{mc}")
                nc.vector.tensor_copy(y_t[:], ps_t[:])
                nc.sync.dma_start(
                    y_d[tt * 128:(tt + 1) * 128, mc * 512:(mc + 1) * 512],
                    y_t[:])
            f.mms = 2
            return f

        out = []
        for tl in range(4):
            for mc in range(2):
                out.append(half_a(tl, mc))
                out.append(half_b(tl, mc))
        return out

    def window(qt, work):
        if not INTERLEAVE:
            for f in work:
                f()
            work = []
        klim = k_needed[qt]
        q0 = qt * 512
        n_tiles = 4 * klim
        cw = pcw.tile([128, 4, 512], F32R, tag="cw", name=f"cw{qt}")
        ctxw[qt] = cw
        done = 0
        wi = 0
        total_mms = sum(getattr(f, "mms", 4) for f in work) or 1
        emitted = 0
        qw = qwin[qt]
        for hp in range(4):
            av0 = pav.tile([65, 512], F32, tag="av0")
            av1 = pav.tile([65, 512], F32, tag="av1")
            first = True
            for kk in range(klim):
                straddle = (qt, kk) in mixed
                s_t = psc.tile([128, 2, 512], F32, tag="sc")
                for j in range(2):
                    nc.tensor.matmul(
                        s_t[:, j, :],
                        KT[j * 64:(j + 1) * 64, hp, kk * 128:(kk + 1) * 128],
                        qw[j * 64:(j + 1) * 64, hp, :],
                        start=True, stop=True, tile_position=(j * 64, 0))
                p_t = ppt.tile([128, 2, 512], F32R, tag="pt")
                nc.scalar.activation(p_t[:], s_t[:], AF.Exp, scale=float(SCALE))
                if straddle:
                    sel_t = ppt.tile([128, 512], F32R, tag="sel")
                    nc.sync.dma_start(sel_t[:], mT_v[:, kk, q0:q0 + 512])
                    nc.vector.tensor_tensor(
                        p_t[:], p_t[:],
                        sel_t[:, None, :].to_broadcast([128, 2, 512]),
                        AL.mult)
                for j, av in ((0, av0), (1, av1)):
                    nc.tensor.matmul(av[:], V65[:, kk, 2 * hp + j, :],
                                     p_t[:, j, :],
                                     start=first, stop=(kk == klim - 1))
                first = False
                done += 1
                if INTERLEAVE:
                    target = done * total_mms / n_tiles
                    while wi < len(work) and emitted < target:
                        emitted += getattr(work[wi], "mms", 4)
                        work[wi]()
                        wi += 1
            avc = pnrm.tile([128, 512], F32, tag="avc", bufs=1)
            lin = pnrm.tile([1, 2, 512], F32, tag="lin")
            nc.vector.tensor_copy(avc[0:64, :], av0[0:64, :])
            nc.vector.tensor_copy(avc[64:128, :], av1[0:64, :])
            nc.vector.tensor_copy(lin[:, 0, :], av0[64:65, :])
            nc.vector.tensor_copy(lin[:, 1, :], av1[64:65, :])
            lrec = pnrm.tile([1, 2, 512], F32, tag="lrec")
            scr = pnrm.tile([1, 512], F32, tag="scr")
            nc.vector.reciprocal_approx_accurate(lrec[:, 0, :], lin[:, 0, :], scr[:])
            nc.vector.reciprocal_approx_accurate(lrec[:, 1, :], lin[:, 1, :], scr[:])
            rec_bc = pnrm.tile([128, 2, 512], F32, tag="rbc")
            nc.gpsimd.partition_broadcast(rec_bc[:], lrec[0:1, :, :])
            nc.vector.tensor_tensor(cw[0:64, hp, :],
                                    avc[0:64, :], rec_bc[0:64, 0, :], AL.mult)
            nc.vector.tensor_tensor(cw[64:128, hp, :],
                                    avc[64:128, :], rec_bc[64:128, 1, :], AL.mult)
        while wi < len(work):
            work[wi]()
            wi += 1

    for f in qk_chunks(0):
        f()
    for f in v_chunks(0):
        f()

    def wo_load():
        nc.sync.dma_start(wo_t[:], woT_d)
    wo_load.mms = 0

    for qt in range(NQT):
        work = []
        if qt == 0:
            work.append(wo_load)
        if qt + 1 < NQT:
            work += qk_chunks(qt + 1)
            work += v_chunks(qt + 1)
        if qt == 2:
            work += p3_chunks(0)
        elif qt == 3:
            work += p3_chunks(1)
            work += p3_chunks(2)
        window(qt, work)
    for f in p3_chunks(NQT - 1):
        f()

    for cm in (pav_cm, psc_cm, pp_cm, py_cm, pnrm_cm, ppt_cm, pcw_cm, pq_cm,
               px_cm, pw_cm, pers_cm):
        cm.__exit__(None, None, None)


def _prep_inputs_dense(x_q, x_k, x_v, mask, wq, wk, wv, bq, bk, bv, wo):
    f32 = np.float32
    in_maps = []
    ones1 = np.ones((128, 1), f32)
    for core in range(NC):
        b, g = divmod(core, G)
        hs = slice(g * HPC, (g + 1) * HPC)
        im = {
            "xqT": np.ascontiguousarray(np.asarray(x_q[b], f32).T),
            "xkT": np.ascontiguousarray(np.asarray(x_k[b], f32).T),
            "xvT": np.ascontiguousarray(np.asarray(x_v[b], f32).T),
            "ones1": ones1,
        }
        for name, w in (("wqT", wq), ("wkT", wk), ("wvT", wv)):
            wt = np.asarray(w[hs], f32).transpose(2, 0, 1).reshape(D, EH)
            im[name] = np.ascontiguousarray(wt.reshape(8, 128, EH))\
                .transpose(1, 0, 2).copy()
        for name, bb in (("bq", bq), ("bk", bk)):
            flat = np.asarray(bb[hs], f32).reshape(EH)
            im[name] = np.ascontiguousarray(flat.reshape(4, 128).T)
        im["bv"] = np.broadcast_to(np.asarray(bv[hs], f32).reshape(1, EH),
                                   (128, EH)).copy()
        woT = np.asarray(wo[:, g * EH:(g + 1) * EH], f32).T
        im["woT"] = np.ascontiguousarray(woT.reshape(4, 128, D))\
            .transpose(1, 0, 2).copy()
        im["maskT"] = np.ascontiguousarray(np.asarray(mask[b], f32).T)
        in_maps.append(im)
    return in_maps


# ======================================================================
# mask analysis + dispatch
# ======================================================================

def _analyze_mask(mask):
    m = np.asarray(mask)
    iota = np.arange(S)
    n = m.sum(axis=2)
    causal = bool((n == iota[None, :] + 1).all()) and \
        bool((m == (iota[None, None, :] < n[..., None])).all())
    allones = bool((m == 1).all())

    k_needed, k_full, mixed = [], [], set()
    if allones:
        mode = "affine"
        k_needed = [NKT] * NQT
        k_full = [NKT] * NQT
    elif causal:
        mode = "affine"
        for qt in range(NQT):
            k_needed.append(4 * qt + 4)
            k_full.append(4 * qt)
            for kk in range(4 * qt, 4 * qt + 4):
                mixed.add((qt, kk))
    else:
        mode = "dense"
        for qt in range(NQT):
            sl = m[:, qt * 512:(qt + 1) * 512, :]
            need = 0
            full = NKT
            for kk in range(NKT):
                blk = sl[:, :, kk * 128:(kk + 1) * 128]
                if blk.any():
                    need = kk + 1
                if not blk.all():
                    full = min(full, kk)
            need = max(need, 1)
            if mode == "dense":
                # keep k-tile count even for pairing safety (harmless extra)
                pass
            full = min(full, need)
            k_needed.append(need)
            k_full.append(full)
            for kk in range(full, need):
                blk = sl[:, :, kk * 128:(kk + 1) * 128]
                if not blk.all():
                    mixed.add((qt, kk))
    return mode, tuple(k_needed), tuple(k_full), frozenset(mixed)


def _run(x_q, x_k, x_v, mask, wq, wk, wv, bq, bk, bv, wo, bo,
         trace=False, trace_cores=None):
    mode, k_needed, k_full, mixed = _analyze_mask(mask)
    key = (mode, k_needed, k_full, mixed)
    if key not in _cache:
        if mode == "affine":
            _cache[key] = _build_v2(k_needed, mixed)
        else:
            _cache[key] = _build_dense(k_needed, k_full, mixed)
    nc = _cache[key]
    if mode == "affine":
        in_maps = _prep_inputs_v2(x_q, x_k, x_v, wq, wk, wv, bq, bk, bv, wo)
    else:
        in_maps = _prep_inputs_dense(x_q, x_k, x_v, mask, wq, wk, wv,
                                     bq, bk, bv, wo)
    res = run_bass_kernel_spmd(nc, in_maps, core_ids=list(range(NC)),
                               trace=trace, trace_cores=trace_cores)
    bo = np.asarray(bo, np.float32)
    out = np.empty((B, S, D), np.float32)
    for b in range(B):
        out[b] = (np.asarray(res.results[2 * b]["y"], np.float32)
                  + np.asarray(res.results[2 * b + 1]["y"], np.float32) + bo)
    return out, res


def kernel(x_q, x_k, x_v, mask, wq, wk, wv, bq, bk, bv, wo, bo):
    out, _ = _run(x_q, x_k, x_v, mask, wq, wk, wv, bq, bk, bv, wo, bo)
    return out


# revision 22
# speedup vs baseline: 1.6314x; 1.0170x over previous
"""MultiHeadAttention (B=4, S=2048, d_model=1024, H=16, dh=64) on 8 trn2 cores.

Sharding: core (b, g) = batch b in 0..3, head-group g in 0..1 (8 heads each).

Causal path (v3): scores/Q/K stay f32r (full precision, head-pair row-packed
score matmuls); x / w / wo / V / P / ctx / y are bf16 (halves DMA 55->20 MB
per core, enables FWL fast weight loads; ~0.2% rms per tensor, fine for the
2e-2 gate). Weights are loaded once (the old kernel re-streamed wq/wk per
q-window). Softmax exp alternates 4:4 between ACT (true exp, bf16 out) and
DVE (Schraudolph: tensor_scalar A*s+B -> int16 -> bitcast bf16, ~1.7% rms on
that half of P), which breaks the old ACT-only ~1.15us/tile softmax cadence.
Causal straddle tiles use one gpsimd affine_select restricted to the
[0:qoff+128) prefix (the only region with masked columns) - keeping gpsimd in
a single ucode library; a memset/tril-TT variant thrashed MODIFY_POOL_CONFIG
and cost ~14us per window. Rowsum rides the AV matmul as a 65th V column; av
psum is drained immediately (2 DVE copies + 1 ACT copy), then
reciprocal_approx_fast + partition_broadcast + the normalize TTs run with
scheduling slack (ctx is consumed a whole window later). y staged via ACT
copies (bf16) and summed across the two head-group cores on host.

Dense-mask path: original f32r kernel (unchanged).
"""
import sys
sys.path.insert(0, "/opt/trn_rl_repo")

import os
import numpy as np
import ml_dtypes

import concourse.bass as bass
import concourse.mybir as mybir
import concourse.tile as tile
from concourse import bacc
from concourse.bass_utils import run_bass_kernel_spmd

F32 = mybir.dt.float32
F32R = mybir.dt.float32r
F8 = mybir.dt.float8e4
I8 = mybir.dt.int8
AF = mybir.ActivationFunctionType
AL = mybir.AluOpType
PM = mybir.MatmulPerfMode

NP_F8 = ml_dtypes.float8_e4m3

INTERLEAVE = os.environ.get("KNOILV") != "1"
EXP_DVE8 = int(os.environ.get("KEXPDVE8", "4"))   # of 8 exp ops, how many on DVE
EXPB = float(os.environ.get("KEXPB", "16248.6"))  # Schraudolph int16/bf16 bias
BAND_GP = os.environ.get("KBANDGP", "1") == "1"   # tril band mask on gpsimd
Y_COPY = os.environ.get("KYCOPY") == "1"          # stage y in SBUF vs psum DMA

B, S, D, H, DH = 4, 2048, 1024, 16, 64
NC = 8
G = 2              # head groups (cores per batch)
HPC = H // G       # 8 heads per core
EH = HPC * DH      # 512
NQT = S // 512     # 4 q-tiles
NKT = S // 128     # 16 k-tiles
NKT2 = S // 256    # 8 k256-tiles
SCALE = 1.0 / np.sqrt(DH)
LOG2E = 1.4426950408889634

_cache = {}
MM_NAMES = {"sc0": set(), "sc1": set(), "av": set(), "pj": set(), "p3": set()}


# ======================================================================
# v2 causal/affine path
# ======================================================================

def _build_v2(k_needed, mixed):
    """k_needed[qt]: leading k128-tiles per q-tile (even). mixed: straddle set."""
    nc = bacc.Bacc("TRN2", target_bir_lowering=False, debug=False, num_devices=NC)

    BF = mybir.dt.bfloat16
    xqb_d = nc.dram_tensor("xqb", [128, 8, S], BF, kind="ExternalInput").ap()
    xkb_d = nc.dram_tensor("xkb", [128, 8, S], BF, kind="ExternalInput").ap()
    xvb_d = nc.dram_tensor("xvb", [128, 8, S], BF, kind="ExternalInput").ap()
    wqb_d = nc.dram_tensor("wqb", [128, 8, EH], BF, kind="ExternalInput").ap()
    wkb_d = nc.dram_tensor("wkb", [128, 8, EH], BF, kind="ExternalInput").ap()
    wvb_d = nc.dram_tensor("wvb", [128, 8, EH], BF, kind="ExternalInput").ap()
    wob_d = nc.dram_tensor("wob", [128, 4, D], BF, kind="ExternalInput").ap()
    bq_d = nc.dram_tensor("bq", [128, 4], F32, kind="ExternalInput").ap()
    bk_d = nc.dram_tensor("bk", [128, 4], F32, kind="ExternalInput").ap()
    bv_d = nc.dram_tensor("bv", [128, HPC, DH], F32, kind="ExternalInput").ap()
    tril_d = nc.dram_tensor("trilb", [128, 128], BF, kind="ExternalInput").ap()
    y_d = nc.dram_tensor("y", [S, D], mybir.dt.bfloat16, kind="ExternalOutput").ap()

    with tile.TileContext(nc) as tc:
        with nc.allow_low_precision(reason="bf16/f32r attention within 2e-2 gate"):
            _body_v2(nc, tc, k_needed, mixed,
                     xqb_d, xkb_d, xvb_d, wqb_d, wkb_d, wvb_d, wob_d,
                     bq_d, bk_d, bv_d, tril_d, y_d)
    nc.compile()
    return nc


def _body_v2(nc, tc, k_needed, mixed,
             xqb_d, xkb_d, xvb_d, wqb_d, wkb_d, wvb_d, wob_d,
             bq_d, bk_d, bv_d, tril_d, y_d):
    BF = mybir.dt.bfloat16
    I16 = mybir.dt.int16
    # Schraudolph exp in bf16 bit space: i16 = round(A16*s + B16); bitcast bf16
    A16 = float(SCALE * 128.0 * LOG2E)
    B16 = float(EXPB)

    pers_cm = tc.tile_pool(name="pers", bufs=1)
    pers = pers_cm.__enter__()
    KT = pers.tile([128, 4, S], F32R)             # [eh%128, et, t]
    V65 = pers.tile([128, NKT, HPC, 65], BF)      # [t%128, kk, h, e|1]
    wqb_t = pers.tile([128, 8, EH], BF)
    wkb_t = pers.tile([128, 8, EH], BF)
    wvb_t = pers.tile([128, 8, EH], BF)
    wob_t = pers.tile([128, 4, D], BF)
    bq_t = pers.tile([128, 4], F32)
    bk_t = pers.tile([128, 4], F32)
    bv_t = pers.tile([128, HPC, DH], F32)
    tril_t = pers.tile([128, 128], BF)
    nc.sync.dma_start(wqb_t[:], wqb_d)
    nc.sync.dma_start(wkb_t[:], wkb_d)
    nc.sync.dma_start(bq_t[:], bq_d)
    nc.sync.dma_start(bk_t[:], bk_d)
    nc.sync.dma_start(wvb_t[:], wvb_d)
    nc.sync.dma_start(bv_t[:], bv_d)
    nc.sync.dma_start(tril_t[:], tril_d)
    nc.vector.memset(V65[:, :, :, 64:65], 1.0)

    px_cm = tc.tile_pool(name="px", bufs=3)
    px = px_cm.__enter__()
    pq_cm = tc.tile_pool(name="pq", bufs=2)
    pq = pq_cm.__enter__()
    pcw_cm = tc.tile_pool(name="pcw", bufs=3)
    pcw = pcw_cm.__enter__()
    ppt_cm = tc.tile_pool(name="ppt", bufs=4)
    ppt = ppt_cm.__enter__()
    pnrm_cm = tc.tile_pool(name="pnrm", bufs=2)
    pnrm = pnrm_cm.__enter__()
    py_cm = tc.tile_pool(name="py", bufs=2)
    py = py_cm.__enter__()
    pp_cm = tc.tile_pool(name="pp", bufs=2, space="PSUM")
    pp = pp_cm.__enter__()
    psc_cm = tc.tile_pool(name="psc", bufs=2, space="PSUM")
    psc = psc_cm.__enter__()
    pav_cm = tc.tile_pool(name="pav", bufs=1, space="PSUM")
    pav = pav_cm.__enter__()

    qwin = {}    # tq -> [128, 4, 512] Q^T window tile (f32r)
    ctxw = {}    # qt -> [128, 2, 2, 512] fp8 interleaved ctx tile
    state = {}
    expctr = [0]

    # ---- projection chunks ----
    def qk_chunks(tq):
        def load():
            xq_t = px.tile([128, 8, 512], BF, tag="x", name=f"xq{tq}")
            nc.sync.dma_start(xq_t[:], xqb_d[:, :, tq * 512:(tq + 1) * 512])
            xk_t = px.tile([128, 8, 512], BF, tag="x", name=f"xk{tq}")
            nc.sync.dma_start(xk_t[:], xkb_d[:, :, tq * 512:(tq + 1) * 512])
            state["xq"], state["xk"] = xq_t, xk_t
            qwin[tq] = pq.tile([128, 4, 512], F32R, tag="qw", name=f"qw{tq}")
        load.mms = 0

        def mmgroup(et, kind):
            def f():
                w_t = wqb_t if kind == "q" else wkb_t
                x_t = state["xq" if kind == "q" else "xk"]
                ps_t = pp.tile([128, 512], F32, tag="pj", name=f"ps_{kind}{tq}_{et}")
                for dk in range(8):
                    mi = nc.tensor.matmul(ps_t[:],
                                          w_t[:, dk, et * 128:(et + 1) * 128],
                                          x_t[:, dk, :],
                                          start=(dk == 0), stop=(dk == 7))
                    MM_NAMES["pj"].add(mi.ins.name)
                if kind == "q":
                    nc.scalar.activation(
                        qwin[tq][:, et, :], ps_t[:], AF.Identity,
                        bias=bq_t[:, et:et + 1], scale=1.0)
                else:
                    nc.scalar.activation(
                        KT[:, et, tq * 512:(tq + 1) * 512], ps_t[:], AF.Identity,
                        bias=bk_t[:, et:et + 1], scale=1.0)
            f.mms = 4
            return f

        out = [load]
        for kind in ("q", "k"):
            for et in range(4):
                out.append(mmgroup(et, kind))
        return out

    def v_chunks(tv):
        def load():
            xv_t = px.tile([128, 8, 512], BF, tag="x", name=f"xv{tv}")
            nc.sync.dma_start(xv_t[:], xvb_d[:, :, tv * 512:(tv + 1) * 512])
            state["xv"] = xv_t
        load.mms = 0

        def mmgroup(tl):
            def f():
                x_t = state["xv"]
                tt = tv * 4 + tl
                ps_t = pp.tile([128, 512], F32, tag="pj", name=f"ps_v{tt}")
                for dk in range(8):
                    mi = nc.tensor.matmul(ps_t[:],
                                          x_t[:, dk, tl * 128:(tl + 1) * 128],
                                          wvb_t[:, dk, :],
                                          start=(dk == 0), stop=(dk == 7))
                    MM_NAMES["pj"].add(mi.ins.name)
                nc.vector.tensor_tensor(
                    V65[:, tt, :, 0:64],
                    ps_t.rearrange("p (h e) -> p h e", h=HPC),
                    bv_t[:], AL.add)
            f.mms = 4
            return f

        out = [load]
        for tl in range(4):
            out.append(mmgroup(tl))
        return out

    def p3_chunks(qt):
        p3state = {}

        def half_a(tl, mc):
            def f():
                cw = ctxw[qt]
                tt = qt * 4 + tl
                ps_t = pp.tile([128, 512], F32, tag="pj", name=f"ps_o{tt}_{mc}")
                p3state[(tl, mc)] = ps_t
                for hp in range(2):
                    mi = nc.tensor.matmul(ps_t[:],
                                          cw[:, hp, tl * 128:(tl + 1) * 128],
                                          wob_t[:, hp, mc * 512:(mc + 1) * 512],
                                          start=(hp == 0), stop=False)
                    MM_NAMES["p3"].add(mi.ins.name)
            f.mms = 2
            return f

        def half_b(tl, mc):
            def f():
                cw = ctxw[qt]
                tt = qt * 4 + tl
                ps_t = p3state.pop((tl, mc))
                for hp in range(2, 4):
                    mi = nc.tensor.matmul(ps_t[:],
                                          cw[:, hp, tl * 128:(tl + 1) * 128],
                                          wob_t[:, hp, mc * 512:(mc + 1) * 512],
                                          start=False, stop=(hp == 3))
                    MM_NAMES["p3"].add(mi.ins.name)
                y_t = py.tile([128, 512], BF, tag="y", name=f"y{tt}_{mc}")
                if Y_COPY:
                    nc.vector.tensor_copy(y_t[:], ps_t[:])
                else:
                    nc.scalar.copy(y_t[:], ps_t[:])
                nc.sync.dma_start(
                    y_d[tt * 128:(tt + 1) * 128, mc * 512:(mc + 1) * 512],
                    y_t[:])
            f.mms = 2
            return f

        out = []
        for tl in range(4):
            for mc in range(2):
                out.append(half_a(tl, mc))
                out.append(half_b(tl, mc))
        return out

    # ---- attention window ----
    def window(qt, work):
        if not INTERLEAVE:
            for f in work:
                f()
            work = []
        klim = k_needed[qt]          # in k128 units
        q0 = qt * 512
        n_units = 4 * klim
        cw = pcw.tile([128, 4, 512], BF, tag="cw", name=f"cw{qt}")
        ctxw[qt] = cw
        done = 0
        wi = 0
        total_mms = sum(getattr(f, "mms", 1) for f in work) or 1
        emitted = 0
        qw = qwin[qt]
        for hp in range(4):
            av = pav.tile([65, 2, 512], F32, tag="av")
            for kk in range(klim):
                straddle = (qt, kk) in mixed
                qoff = max(0, kk * 128 - q0) if straddle else 0
                moff = 128 if qoff >= 128 else 0
                s_t = psc.tile([128, 2, 512], F32, tag="sc")
                for j in range(2):
                    mi = nc.tensor.matmul(
                        s_t[:, j, moff:512],
                        KT[j * 64:(j + 1) * 64, hp, kk * 128:(kk + 1) * 128],
                        qw[j * 64:(j + 1) * 64, hp, moff:512],
                        start=True, stop=True, tile_position=(j * 64, 0))
                    MM_NAMES[f"sc{j}"].add(mi.ins.name)
                p_t = ppt.tile([128, 2, 512], BF, tag="pt")
                use_dve = (expctr[0] % 8) < EXP_DVE8
                expctr[0] += 1
                if use_dve:
                    p_i = p_t[:].bitcast(I16)
                    nc.vector.tensor_scalar(
                        p_i[:, :, qoff:512], s_t[:, :, qoff:512],
                        A16, B16, op0=AL.mult, op1=AL.add)
                else:
                    nc.scalar.activation(p_t[:, :, qoff:512],
                                         s_t[:, :, qoff:512],
                                         AF.Exp, scale=float(SCALE))
                if straddle:
                    # zero q < qoff+p: fully-masked prefix plus diagonal band,
                    # restricted to [0:qoff+128) (beyond is all-keep)
                    w = qoff + 128
                    nc.gpsimd.affine_select(
                        p_t[:, :, 0:w], p_t[:, :, 0:w],
                        pattern=[[0, 2], [1, w]],
                        compare_op=AL.is_ge, fill=0.0,
                        base=q0 - kk * 128, channel_multiplier=-1)
                for j in range(2):
                    mi = nc.tensor.matmul(av[:, j, :],
                                          V65[:, kk, 2 * hp + j, :],
                                          p_t[:, j, :],
                                          start=(kk == 0), stop=(kk == klim - 1))
                    MM_NAMES["av"].add(mi.ins.name)
                done += 1
                if INTERLEAVE:
                    target = done * total_mms / n_units
                    while wi < len(work) and emitted < target:
                        emitted += getattr(work[wi], "mms", 1)
                        work[wi]()
                        wi += 1
            # ---- normalize: drain av psum fast, then ctx*(1/l) with slack ----
            avc = pnrm.tile([128, 512], F32, tag="avc")
            nc.vector.tensor_copy(avc[0:64, :], av[0:64, 0, :])
            nc.scalar.copy(avc[64:128, :], av[0:64, 1, :])
            L2 = pnrm.tile([1, 2, 512], F32, tag="lin")
            nc.scalar.copy(L2[:], av[64:65, :, :])
            lrec = pnrm.tile([1, 2, 512], F32, tag="lrec")
            nc.vector.reciprocal_approx_fast(lrec[:], L2[:])
            rec_bc = pnrm.tile([128, 2, 512], F32, tag="rbc")
            nc.gpsimd.partition_broadcast(rec_bc[:], lrec[0:1, :, :])
            for j in range(2):
                nc.vector.tensor_tensor(
                    cw[j * 64:(j + 1) * 64, hp, :],
                    avc[j * 64:(j + 1) * 64, :],
                    rec_bc[j * 64:(j + 1) * 64, j, :], AL.mult)
        while wi < len(work):
            work[wi]()
            wi += 1

    # ---- prologue ----
    nc.sync.dma_start(wob_t[:], wob_d)
    for f in qk_chunks(0):
        f()
    for f in v_chunks(0):
        f()

    for qt in range(NQT):
        work = []
        if qt + 1 < NQT:
            work += qk_chunks(qt + 1)
            work += v_chunks(qt + 1)
        if qt == 2:
            work += p3_chunks(0)
        elif qt == 3:
            work += p3_chunks(1)
            work += p3_chunks(2)
        window(qt, work)
    for f in p3_chunks(NQT - 1):
        f()

    for cm in (pav_cm, psc_cm, pp_cm, py_cm, pnrm_cm, ppt_cm, pcw_cm, pq_cm,
               px_cm, pers_cm):
        cm.__exit__(None, None, None)


def _prep_inputs_v2(x_q, x_k, x_v, wq, wk, wv, bq, bk, bv, wo):
    f32 = np.float32
    bf16 = ml_dtypes.bfloat16
    trilb = np.triu(np.ones((128, 128), f32)).astype(bf16)
    in_maps = []

    def xb_of(x):
        # [128, 8, S]: xb[p, dk, t] = x[t, dk*128 + p]
        xr = np.asarray(x, f32).T.reshape(8, 128, S)
        return np.ascontiguousarray(xr.transpose(1, 0, 2)).astype(bf16)

    def wb_of(w, hs):
        wt = np.asarray(w[hs], f32).transpose(2, 0, 1).reshape(D, EH)
        return np.ascontiguousarray(
            wt.reshape(8, 128, EH).transpose(1, 0, 2)).astype(bf16)

    xq_c = {}
    for core in range(NC):
        b, g = divmod(core, G)
        hs = slice(g * HPC, (g + 1) * HPC)
        if b not in xq_c:
            xq_c[b] = (xb_of(x_q[b]), xb_of(x_k[b]), xb_of(x_v[b]))
        xqb, xkb, xvb = xq_c[b]
        im = {
            "xqb": xqb, "xkb": xkb, "xvb": xvb,
            "wqb": wb_of(wq, hs),
            "wkb": wb_of(wk, hs),
            "wvb": wb_of(wv, hs),
            "trilb": trilb,
        }
        woT = np.asarray(wo[:, g * EH:(g + 1) * EH], f32).T   # [EH, D]
        im["wob"] = np.ascontiguousarray(
            woT.reshape(4, 128, D).transpose(1, 0, 2)).astype(bf16)
        for name, bb in (("bq", bq), ("bk", bk)):
            flat = np.asarray(bb[hs], f32).reshape(EH)
            im[name] = np.ascontiguousarray(flat.reshape(4, 128).T)
        im["bv"] = np.broadcast_to(
            np.asarray(bv[hs], f32).reshape(1, HPC, DH), (128, HPC, DH)).copy()
        in_maps.append(im)
    return in_maps


# ======================================================================
# dense-mask fallback: original fp32r kernel
# ======================================================================

def _build_dense(k_needed, k_full, mixed):
    nc = bacc.Bacc("TRN2", target_bir_lowering=False, debug=False, num_devices=NC)

    xqT_d = nc.dram_tensor("xqT", [D, S], F32R, kind="ExternalInput").ap()
    xkT_d = nc.dram_tensor("xkT", [D, S], F32R, kind="ExternalInput").ap()
    xvT_d = nc.dram_tensor("xvT", [D, S], F32R, kind="ExternalInput").ap()
    wqT_d = nc.dram_tensor("wqT", [128, 8, EH], F32R, kind="ExternalInput").ap()
    wkT_d = nc.dram_tensor("wkT", [128, 8, EH], F32R, kind="ExternalInput").ap()
    wvT_d = nc.dram_tensor("wvT", [128, 8, EH], F32R, kind="ExternalInput").ap()
    bq_d = nc.dram_tensor("bq", [128, 4], F32, kind="ExternalInput").ap()
    bk_d = nc.dram_tensor("bk", [128, 4], F32, kind="ExternalInput").ap()
    bv_d = nc.dram_tensor("bv", [128, EH], F32, kind="ExternalInput").ap()
    woT_d = nc.dram_tensor("woT", [128, 4, D], F32R, kind="ExternalInput").ap()
    ones_d = nc.dram_tensor("ones1", [128, 1], F32R, kind="ExternalInput").ap()
    mT_d = nc.dram_tensor("maskT", [S, S], F32R, kind="ExternalInput").ap()
    mT_v = mT_d.rearrange("(kt p) q -> p kt q", p=128)
    y_d = nc.dram_tensor("y", [S, D], F32, kind="ExternalOutput").ap()

    xq_v = xqT_d.rearrange("(dk p) t -> p dk t", p=128)
    xk_v = xkT_d.rearrange("(dk p) t -> p dk t", p=128)
    xv_v = xvT_d.rearrange("(dk p) t -> p dk t", p=128)

    with tile.TileContext(nc) as tc:
        with nc.allow_low_precision(reason="fp32r storage has fp32 width"):
            _body_dense(nc, tc, k_needed, mixed,
                        xq_v, xk_v, xv_v, wqT_d, wkT_d, wvT_d,
                        bq_d, bk_d, bv_d, woT_d, ones_d, mT_v, y_d)
    nc.compile()
    return nc


def _body_dense(nc, tc, k_needed, mixed,
                xq_v, xk_v, xv_v, wqT_d, wkT_d, wvT_d,
                bq_d, bk_d, bv_d, woT_d, ones_d, mT_v, y_d):
    pers_cm = tc.tile_pool(name="pers", bufs=1)
    pers = pers_cm.__enter__()
    KT = pers.tile([128, 4, S], F32R)
    V65 = pers.tile([128, NKT, HPC, 65], F32R)
    wo_t = pers.tile([128, 4, D], F32R)
    wv_t = pers.tile([128, 8, EH], F32R)
    bq_t = pers.tile([128, 4], F32)
    bk_t = pers.tile([128, 4], F32)
    bv_t = pers.tile([128, EH], F32)
    ones_t = pers.tile([128, 1], F32R)
    nc.sync.dma_start(bq_t[:], bq_d)
    nc.sync.dma_start(bk_t[:], bk_d)
    nc.sync.dma_start(bv_t[:], bv_d)
    nc.sync.dma_start(ones_t[:], ones_d)
    nc.sync.dma_start(wv_t[:], wvT_d)
    nc.vector.tensor_copy(V65[:, :, :, 64:65],
                          ones_t[:, 0:1].to_broadcast([128, NKT, HPC, 1]))

    pw_cm = tc.tile_pool(name="pw", bufs=2)
    pw = pw_cm.__enter__()
    px_cm = tc.tile_pool(name="px", bufs=3)
    px = px_cm.__enter__()
    pq_cm = tc.tile_pool(name="pq", bufs=2)
    pq = pq_cm.__enter__()
    pcw_cm = tc.tile_pool(name="pcw", bufs=3)
    pcw = pcw_cm.__enter__()
    ppt_cm = tc.tile_pool(name="ppt", bufs=2)
    ppt = ppt_cm.__enter__()
    pnrm_cm = tc.tile_pool(name="pnrm", bufs=1)
    pnrm = pnrm_cm.__enter__()
    py_cm = tc.tile_pool(name="py", bufs=2)
    py = py_cm.__enter__()
    pp_cm = tc.tile_pool(name="pp", bufs=2, space="PSUM")
    pp = pp_cm.__enter__()
    psc_cm = tc.tile_pool(name="psc", bufs=2, space="PSUM")
    psc = psc_cm.__enter__()
    pav_cm = tc.tile_pool(name="pav", bufs=1, space="PSUM")
    pav = pav_cm.__enter__()

    qwin = {}
    ctxw = {}
    state = {}

    def qk_chunks(tq):
        def load(w_d, x_v, kind, half):
            def f():
                hs = slice(half * 4, half * 4 + 4)
                w_t = pw.tile([128, 4, EH], F32R, tag="w", name=f"w_{kind}{tq}{half}")
                nc.sync.dma_start(w_t[:], w_d[:, hs, :])
                x_t = px.tile([128, 4, 512], F32R, tag="x", name=f"x_{kind}{tq}{half}")
                nc.sync.dma_start(x_t[:], x_v[:, hs, tq * 512:(tq + 1) * 512])
                state[f"w{half}"], state[f"x{half}"] = w_t, x_t
                if kind == "q" and half == 0:
                    qwin[tq] = pq.tile([128, 4, 512], F32R, tag="qw", name=f"qw{tq}")
            return f

        def mmgroup(et, kind):
            def f():
                ps_t = pp.tile([128, 512], F32, tag="pj", name=f"ps_{kind}{tq}_{et}")
                for dk in range(8):
                    w_t = state[f"w{dk // 4}"]
                    x_t = state[f"x{dk // 4}"]
                    nc.tensor.matmul(ps_t[:],
                                     w_t[:, dk % 4, et * 128:(et + 1) * 128],
                                     x_t[:, dk % 4, :],
                                     start=(dk == 0), stop=(dk == 7))
                if kind == "q":
                    nc.vector.tensor_tensor(
                        qwin[tq][:, et, :], ps_t[:],
                        bq_t[:, et:et + 1].to_broadcast([128, 512]), AL.add)
                else:
                    nc.vector.tensor_tensor(
                        KT[:, et, tq * 512:(tq + 1) * 512], ps_t[:],
                        bk_t[:, et:et + 1].to_broadcast([128, 512]), AL.add)
            return f

        out = []
        for kind, w_d, x_v in (("q", wqT_d, xq_v), ("k", wkT_d, xk_v)):
            for half in (0, 1):
                g = load(w_d, x_v, kind, half)
                g.mms = 0
                out.append(g)
            for et in range(4):
                g = mmgroup(et, kind)
                g.mms = 8
                out.append(g)
        return out

    def v_chunks(tv):
        def load(half):
            def f():
                hs = slice(half * 4, half * 4 + 4)
                x_t = px.tile([128, 4, 512], F32R, tag="x", name=f"x_v{tv}{half}")
                nc.sync.dma_start(x_t[:], xv_v[:, hs, tv * 512:(tv + 1) * 512])
                state[f"x{half}"] = x_t
            return f

        def mmgroup(tl):
            def f():
                tt = tv * 4 + tl
                ps_t = pp.tile([128, 512], F32, tag="pj", name=f"ps_v{tt}")
                for dk in range(8):
                    x_t = state[f"x{dk // 4}"]
                    nc.tensor.matmul(ps_t[:],
                                     x_t[:, dk % 4, tl * 128:(tl + 1) * 128],
                                     wv_t[:, dk, :],
                                     start=(dk == 0), stop=(dk == 7))
                nc.vector.tensor_tensor(
                    V65[:, tt, :, 0:64],
                    ps_t.rearrange("p (h e) -> p h e", h=HPC),
                    bv_t.rearrange("p (h e) -> p h e", h=HPC), AL.add)
            return f

        out = []
        for half in (0, 1):
            g = load(half)
            g.mms = 0
            out.append(g)
        for tl in range(4):
            g = mmgroup(tl)
            g.mms = 8
            out.append(g)
        return out

    def p3_chunks(qt):
        p3state = {}

        def half_a(tl, mc):
            def f():
                cw = ctxw[qt]
                tt = qt * 4 + tl
                ps_t = pp.tile([128, 512], F32, tag="pj", name=f"ps_o{tt}_{mc}")
                p3state[(tl, mc)] = ps_t
                for hp in range(2):
                    nc.tensor.matmul(ps_t[:],
                                     cw[:, hp, tl * 128:(tl + 1) * 128],
                                     wo_t[:, hp, mc * 512:(mc + 1) * 512],
                                     start=(hp == 0), stop=False)
            f.mms = 2
            return f

        def half_b(tl, mc):
            def f():
                cw = ctxw[qt]
                tt = qt * 4 + tl
                ps_t = p3state.pop((tl, mc))
                for hp in range(2, 4):
                    nc.tensor.matmul(ps_t[:],
                                     cw[:, hp, tl * 128:(tl + 1) * 128],
                                     wo_t[:, hp, mc * 512:(mc + 1) * 512],
                                     start=False, stop=(hp == 3))
                y_t = py.tile([128, 512], F32, tag="y", name=f"y{tt}_{mc}")
                nc.vector.tensor_copy(y_t[:], ps_t[:])
                nc.sync.dma_start(
                    y_d[tt * 128:(tt + 1) * 128, mc * 512:(mc + 1) * 512],
                    y_t[:])
            f.mms = 2
            return f

        out = []
        for tl in range(4):
            for mc in range(2):
                out.append(half_a(tl, mc))
                out.append(half_b(tl, mc))
        return out

    def window(qt, work):
        if not INTERLEAVE:
            for f in work:
                f()
            work = []
        klim = k_needed[qt]
        q0 = qt * 512
        n_tiles = 4 * klim
        cw = pcw.tile([128, 4, 512], F32R, tag="cw", name=f"cw{qt}")
        ctxw[qt] = cw
        done = 0
        wi = 0
        total_mms = sum(getattr(f, "mms", 4) for f in work) or 1
        emitted = 0
        qw = qwin[qt]
        for hp in range(4):
            av0 = pav.tile([65, 512], F32, tag="av0")
            av1 = pav.tile([65, 512], F32, tag="av1")
            first = True
            for kk in range(klim):
                straddle = (qt, kk) in mixed
                s_t = psc.tile([128, 2, 512], F32, tag="sc")
                for j in range(2):
                    nc.tensor.matmul(
                        s_t[:, j, :],
                        KT[j * 64:(j + 1) * 64, hp, kk * 128:(kk + 1) * 128],
                        qw[j * 64:(j + 1) * 64, hp, :],
                        start=True, stop=True, tile_position=(j * 64, 0))
                p_t = ppt.tile([128, 2, 512], F32R, tag="pt")
                nc.scalar.activation(p_t[:], s_t[:], AF.Exp, scale=float(SCALE))
                if straddle:
                    sel_t = ppt.tile([128, 512], F32R, tag="sel")
                    nc.sync.dma_start(sel_t[:], mT_v[:, kk, q0:q0 + 512])
                    nc.vector.tensor_tensor(
                        p_t[:], p_t[:],
                        sel_t[:, None, :].to_broadcast([128, 2, 512]),
                        AL.mult)
                for j, av in ((0, av0), (1, av1)):
                    nc.tensor.matmul(av[:], V65[:, kk, 2 * hp + j, :],
                                     p_t[:, j, :],
                                     start=first, stop=(kk == klim - 1))
                first = False
                done += 1
                if INTERLEAVE:
                    target = done * total_mms / n_tiles
                    while wi < len(work) and emitted < target:
                        emitted += getattr(work[wi], "mms", 4)
                        work[wi]()
                        wi += 1
            avc = pnrm.tile([128, 512], F32, tag="avc", bufs=1)
            lin = pnrm.tile([1, 2, 512], F32, tag="lin")
            nc.vector.tensor_copy(avc[0:64, :], av0[0:64, :])
            nc.vector.tensor_copy(avc[64:128, :], av1[0:64, :])
            nc.vector.tensor_copy(lin[:, 0, :], av0[64:65, :])
            nc.vector.tensor_copy(lin[:, 1, :], av1[64:65, :])
            lrec = pnrm.tile([1, 2, 512], F32, tag="lrec")
            scr = pnrm.tile([1, 512], F32, tag="scr")
            nc.vector.reciprocal_approx_accurate(lrec[:, 0, :], lin[:, 0, :], scr[:])
            nc.vector.reciprocal_approx_accurate(lrec[:, 1, :], lin[:, 1, :], scr[:])
            rec_bc = pnrm.tile([128, 2, 512], F32, tag="rbc")
            nc.gpsimd.partition_broadcast(rec_bc[:], lrec[0:1, :, :])
            nc.vector.tensor_tensor(cw[0:64, hp, :],
                                    avc[0:64, :], rec_bc[0:64, 0, :], AL.mult)
            nc.vector.tensor_tensor(cw[64:128, hp, :],
                                    avc[64:128, :], rec_bc[64:128, 1, :], AL.mult)
        while wi < len(work):
            work[wi]()
            wi += 1

    for f in qk_chunks(0):
        f()
    for f in v_chunks(0):
        f()

    def wo_load():
        nc.sync.dma_start(wo_t[:], woT_d)
    wo_load.mms = 0

    for qt in range(NQT):
        work = []
        if qt == 0:
            work.append(wo_load)
        if qt + 1 < NQT:
            work += qk_chunks(qt + 1)
            work += v_chunks(qt + 1)
        if qt == 2:
            work += p3_chunks(0)
        elif qt == 3:
            work += p3_chunks(1)
            work += p3_chunks(2)
        window(qt, work)
    for f in p3_chunks(NQT - 1):
        f()

    for cm in (pav_cm, psc_cm, pp_cm, py_cm, pnrm_cm, ppt_cm, pcw_cm, pq_cm,
               px_cm, pw_cm, pers_cm):
        cm.__exit__(None, None, None)


def _prep_inputs_dense(x_q, x_k, x_v, mask, wq, wk, wv, bq, bk, bv, wo):
    f32 = np.float32
    in_maps = []
    ones1 = np.ones((128, 1), f32)
    for core in range(NC):
        b, g = divmod(core, G)
        hs = slice(g * HPC, (g + 1) * HPC)
        im = {
            "xqT": np.ascontiguousarray(np.asarray(x_q[b], f32).T),
            "xkT": np.ascontiguousarray(np.asarray(x_k[b], f32).T),
            "xvT": np.ascontiguousarray(np.asarray(x_v[b], f32).T),
            "ones1": ones1,
        }
        for name, w in (("wqT", wq), ("wkT", wk), ("wvT", wv)):
            wt = np.asarray(w[hs], f32).transpose(2, 0, 1).reshape(D, EH)
            im[name] = np.ascontiguousarray(wt.reshape(8, 128, EH))\
                .transpose(1, 0, 2).copy()
        for name, bb in (("bq", bq), ("bk", bk)):
            flat = np.asarray(bb[hs], f32).reshape(EH)
            im[name] = np.ascontiguousarray(flat.reshape(4, 128).T)
        im["bv"] = np.broadcast_to(np.asarray(bv[hs], f32).reshape(1, EH),
                                   (128, EH)).copy()
        woT = np.asarray(wo[:, g * EH:(g + 1) * EH], f32).T
        im["woT"] = np.ascontiguousarray(woT.reshape(4, 128, D))\
            .transpose(1, 0, 2).copy()
        im["maskT"] = np.ascontiguousarray(np.asarray(mask[b], f32).T)
        in_maps.append(im)
    return in_maps


# ======================================================================
# mask analysis + dispatch
# ======================================================================

def _analyze_mask(mask):
    m = np.asarray(mask)
    iota = np.arange(S)
    n = m.sum(axis=2)
    causal = bool((n == iota[None, :] + 1).all()) and \
        bool((m == (iota[None, None, :] < n[..., None])).all())
    allones = bool((m == 1).all())

    k_needed, k_full, mixed = [], [], set()
    if allones:
        mode = "affine"
        k_needed = [NKT] * NQT
        k_full = [NKT] * NQT
    elif causal:
        mode = "affine"
        for qt in range(NQT):
            k_needed.append(4 * qt + 4)
            k_full.append(4 * qt)
            for kk in range(4 * qt, 4 * qt + 4):
                mixed.add((qt, kk))
    else:
        mode = "dense"
        for qt in range(NQT):
            sl = m[:, qt * 512:(qt + 1) * 512, :]
            need = 0
            full = NKT
            for kk in range(NKT):
                blk = sl[:, :, kk * 128:(kk + 1) * 128]
                if blk.any():
                    need = kk + 1
                if not blk.all():
                    full = min(full, kk)
            need = max(need, 1)
            if mode == "dense":
                # keep k-tile count even for pairing safety (harmless extra)
                pass
            full = min(full, need)
            k_needed.append(need)
            k_full.append(full)
            for kk in range(full, need):
                blk = sl[:, :, kk * 128:(kk + 1) * 128]
                if not blk.all():
                    mixed.add((qt, kk))
    return mode, tuple(k_needed), tuple(k_full), frozenset(mixed)


def _run(x_q, x_k, x_v, mask, wq, wk, wv, bq, bk, bv, wo, bo,
         trace=False, trace_cores=None):
    mode, k_needed, k_full, mixed = _analyze_mask(mask)
    key = (mode, k_needed, k_full, mixed)
    if key not in _cache:
        if mode == "affine":
            _cache[key] = _build_v2(k_needed, mixed)
        else:
            _cache[key] = _build_dense(k_needed, k_full, mixed)
    nc = _cache[key]
    if mode == "affine":
        in_maps = _prep_inputs_v2(x_q, x_k, x_v, wq, wk, wv, bq, bk, bv, wo)
    else:
        in_maps = _prep_inputs_dense(x_q, x_k, x_v, mask, wq, wk, wv,
                                     bq, bk, bv, wo)
    res = run_bass_kernel_spmd(nc, in_maps, core_ids=list(range(NC)),
                               trace=trace, trace_cores=trace_cores)
    bo = np.asarray(bo, np.float32)
    out = np.empty((B, S, D), np.float32)
    for b in range(B):
        out[b] = (np.asarray(res.results[2 * b]["y"], np.float32)
                  + np.asarray(res.results[2 * b + 1]["y"], np.float32) + bo)
    return out, res


def kernel(x_q, x_k, x_v, mask, wq, wk, wv, bq, bk, bv, wo, bo):
    out, _ = _run(x_q, x_k, x_v, mask, wq, wk, wv, bq, bk, bv, wo, bo)
    return out


# revision 23
# speedup vs baseline: 1.6707x; 1.0240x over previous
"""MultiHeadAttention (B=4, S=2048, d_model=1024, H=16, dh=64) on 8 trn2 cores.

Sharding: core (b, g) = batch b in 0..3, head-group g in 0..1 (8 heads each).

Causal path (v3): scores/Q/K stay f32r (full precision, head-pair row-packed
score matmuls); x / w / wo / V / P / ctx / y are bf16 (halves DMA 55->20 MB
per core, enables FWL fast weight loads; ~0.2% rms per tensor, fine for the
2e-2 gate). Weights are loaded once (the old kernel re-streamed wq/wk per
q-window). Softmax exp alternates 4:4 between ACT (true exp, bf16 out) and
DVE (Schraudolph: tensor_scalar A*s+B -> int16 -> bitcast bf16, ~1.7% rms on
that half of P), which breaks the old ACT-only ~1.15us/tile softmax cadence.
Causal straddle tiles use one gpsimd affine_select restricted to the
[0:qoff+128) prefix (the only region with masked columns) - keeping gpsimd in
a single ucode library; a memset/tril-TT variant thrashed MODIFY_POOL_CONFIG
and cost ~14us per window. Rowsum rides the AV matmul as a 65th V column; av
psum is drained immediately (2 DVE copies + 1 ACT copy), then
reciprocal_approx_fast + partition_broadcast + the normalize TTs run with
scheduling slack (ctx is consumed a whole window later). y staged via ACT
copies (bf16) and summed across the two head-group cores on host.

Dense-mask path: original f32r kernel (unchanged).
"""
import sys
sys.path.insert(0, "/opt/trn_rl_repo")

import os
import numpy as np
import ml_dtypes

import concourse.bass as bass
import concourse.mybir as mybir
import concourse.tile as tile
from concourse import bacc
from concourse.bass_utils import run_bass_kernel_spmd

F32 = mybir.dt.float32
F32R = mybir.dt.float32r
F8 = mybir.dt.float8e4
I8 = mybir.dt.int8
AF = mybir.ActivationFunctionType
AL = mybir.AluOpType
PM = mybir.MatmulPerfMode

NP_F8 = ml_dtypes.float8_e4m3

INTERLEAVE = os.environ.get("KNOILV") != "1"
EXP_DVE8 = int(os.environ.get("KEXPDVE8", "4"))   # of 8 exp ops, how many on DVE
EXPB = float(os.environ.get("KEXPB", "16248.6"))  # Schraudolph int16/bf16 bias
BAND_GP = os.environ.get("KBANDGP", "1") == "1"   # tril band mask on gpsimd
Y_COPY = os.environ.get("KYCOPY") == "1"          # stage y in SBUF vs psum DMA

B, S, D, H, DH = 4, 2048, 1024, 16, 64
NC = 8
G = 2              # head groups (cores per batch)
HPC = H // G       # 8 heads per core
EH = HPC * DH      # 512
NQT = S // 512     # 4 q-tiles
NKT = S // 128     # 16 k-tiles
NKT2 = S // 256    # 8 k256-tiles
SCALE = 1.0 / np.sqrt(DH)
LOG2E = 1.4426950408889634

_cache = {}
MM_NAMES = {"sc0": set(), "sc1": set(), "av": set(), "pj": set(), "p3": set()}


# ======================================================================
# v2 causal/affine path
# ======================================================================

def _build_v2(k_needed, mixed):
    """k_needed[qt]: leading k128-tiles per q-tile (even). mixed: straddle set."""
    nc = bacc.Bacc("TRN2", target_bir_lowering=False, debug=False, num_devices=NC)

    BF = mybir.dt.bfloat16
    xqb_d = nc.dram_tensor("xqb", [128, 8, S], BF, kind="ExternalInput").ap()
    xkb_d = nc.dram_tensor("xkb", [128, 8, S], BF, kind="ExternalInput").ap()
    xvb_d = nc.dram_tensor("xvb", [128, 8, S], BF, kind="ExternalInput").ap()
    wqb_d = nc.dram_tensor("wqb", [128, 8, EH], BF, kind="ExternalInput").ap()
    wkb_d = nc.dram_tensor("wkb", [128, 8, EH], BF, kind="ExternalInput").ap()
    wvb_d = nc.dram_tensor("wvb", [128, 8, EH], BF, kind="ExternalInput").ap()
    wob_d = nc.dram_tensor("wob", [128, 4, D], BF, kind="ExternalInput").ap()
    bq_d = nc.dram_tensor("bq", [128, 4], F32, kind="ExternalInput").ap()
    bk_d = nc.dram_tensor("bk", [128, 4], F32, kind="ExternalInput").ap()
    bv_d = nc.dram_tensor("bv", [128, HPC, DH], F32, kind="ExternalInput").ap()
    tril_d = nc.dram_tensor("trilb", [128, 128], BF, kind="ExternalInput").ap()
    y_d = nc.dram_tensor("y", [S, D], mybir.dt.bfloat16, kind="ExternalOutput").ap()

    with tile.TileContext(nc) as tc:
        with nc.allow_low_precision(reason="bf16/f32r attention within 2e-2 gate"):
            _body_v2(nc, tc, k_needed, mixed,
                     xqb_d, xkb_d, xvb_d, wqb_d, wkb_d, wvb_d, wob_d,
                     bq_d, bk_d, bv_d, tril_d, y_d)
    nc.compile()
    return nc


def _body_v2(nc, tc, k_needed, mixed,
             xqb_d, xkb_d, xvb_d, wqb_d, wkb_d, wvb_d, wob_d,
             bq_d, bk_d, bv_d, tril_d, y_d):
    BF = mybir.dt.bfloat16
    I16 = mybir.dt.int16
    # Schraudolph exp in bf16 bit space: i16 = round(A16*s + B16); bitcast bf16
    A16 = float(SCALE * 128.0 * LOG2E)
    B16 = float(EXPB)

    pers_cm = tc.tile_pool(name="pers", bufs=1)
    pers = pers_cm.__enter__()
    KT = pers.tile([128, 4, S], F32R)             # [eh%128, et, t]
    V65 = pers.tile([128, NKT, HPC, 65], BF)      # [t%128, kk, h, e|1]
    wqb_t = pers.tile([128, 8, EH], BF)
    wkb_t = pers.tile([128, 8, EH], BF)
    wvb_t = pers.tile([128, 8, EH], BF)
    wob_t = pers.tile([128, 4, D], BF)
    bq_t = pers.tile([128, 4], F32)
    bk_t = pers.tile([128, 4], F32)
    bv_t = pers.tile([128, HPC, DH], F32)
    tril_t = pers.tile([128, 128], BF)
    nc.sync.dma_start(wqb_t[:], wqb_d)
    nc.sync.dma_start(wkb_t[:], wkb_d)
    nc.sync.dma_start(bq_t[:], bq_d)
    nc.sync.dma_start(bk_t[:], bk_d)
    nc.vector.memset(V65[:, :, :, 64:65], 1.0)

    px_cm = tc.tile_pool(name="px", bufs=3)
    px = px_cm.__enter__()
    pq_cm = tc.tile_pool(name="pq", bufs=2)
    pq = pq_cm.__enter__()
    pcw_cm = tc.tile_pool(name="pcw", bufs=3)
    pcw = pcw_cm.__enter__()
    ppt_cm = tc.tile_pool(name="ppt", bufs=4)
    ppt = ppt_cm.__enter__()
    pnrm_cm = tc.tile_pool(name="pnrm", bufs=2)
    pnrm = pnrm_cm.__enter__()
    py_cm = tc.tile_pool(name="py", bufs=2)
    py = py_cm.__enter__()
    pp_cm = tc.tile_pool(name="pp", bufs=2, space="PSUM")
    pp = pp_cm.__enter__()
    psc_cm = tc.tile_pool(name="psc", bufs=2, space="PSUM")
    psc = psc_cm.__enter__()
    pav_cm = tc.tile_pool(name="pav", bufs=1, space="PSUM")
    pav = pav_cm.__enter__()

    qwin = {}    # tq -> [128, 4, 512] Q^T window tile (f32r)
    ctxw = {}    # qt -> [128, 2, 2, 512] fp8 interleaved ctx tile
    state = {}
    expctr = [0]

    # ---- projection chunks ----
    def qk_chunks(tq):
        def load():
            xq_t = px.tile([128, 8, 512], BF, tag="x", name=f"xq{tq}")
            nc.sync.dma_start(xq_t[:], xqb_d[:, :, tq * 512:(tq + 1) * 512])
            xk_t = px.tile([128, 8, 512], BF, tag="x", name=f"xk{tq}")
            nc.sync.dma_start(xk_t[:], xkb_d[:, :, tq * 512:(tq + 1) * 512])
            state["xq"], state["xk"] = xq_t, xk_t
            qwin[tq] = pq.tile([128, 4, 512], F32R, tag="qw", name=f"qw{tq}")
        load.mms = 0

        def mmgroup(et, kind):
            def f():
                w_t = wqb_t if kind == "q" else wkb_t
                x_t = state["xq" if kind == "q" else "xk"]
                ps_t = pp.tile([128, 512], F32, tag="pj", name=f"ps_{kind}{tq}_{et}")
                for dk in range(8):
                    mi = nc.tensor.matmul(ps_t[:],
                                          w_t[:, dk, et * 128:(et + 1) * 128],
                                          x_t[:, dk, :],
                                          start=(dk == 0), stop=(dk == 7))
                    MM_NAMES["pj"].add(mi.ins.name)
                if kind == "q":
                    nc.scalar.activation(
                        qwin[tq][:, et, :], ps_t[:], AF.Identity,
                        bias=bq_t[:, et:et + 1], scale=1.0)
                else:
                    nc.scalar.activation(
                        KT[:, et, tq * 512:(tq + 1) * 512], ps_t[:], AF.Identity,
                        bias=bk_t[:, et:et + 1], scale=1.0)
            f.mms = 4
            return f

        out = [load]
        for kind in ("q", "k"):
            for et in range(4):
                out.append(mmgroup(et, kind))
        return out

    def v_chunks(tv):
        def load():
            xv_t = px.tile([128, 8, 512], BF, tag="x", name=f"xv{tv}")
            nc.sync.dma_start(xv_t[:], xvb_d[:, :, tv * 512:(tv + 1) * 512])
            state["xv"] = xv_t
        load.mms = 0

        def mmgroup(tl):
            def f():
                x_t = state["xv"]
                tt = tv * 4 + tl
                ps_t = pp.tile([128, 512], F32, tag="pj", name=f"ps_v{tt}")
                for dk in range(8):
                    mi = nc.tensor.matmul(ps_t[:],
                                          x_t[:, dk, tl * 128:(tl + 1) * 128],
                                          wvb_t[:, dk, :],
                                          start=(dk == 0), stop=(dk == 7))
                    MM_NAMES["pj"].add(mi.ins.name)
                nc.vector.tensor_tensor(
                    V65[:, tt, :, 0:64],
                    ps_t.rearrange("p (h e) -> p h e", h=HPC),
                    bv_t[:], AL.add)
            f.mms = 4
            return f

        out = [load]
        for tl in range(4):
            out.append(mmgroup(tl))
        return out

    def p3_chunks(qt):
        p3state = {}

        def half_a(tl, mc):
            def f():
                cw = ctxw[qt]
                tt = qt * 4 + tl
                ps_t = pp.tile([128, 512], F32, tag="pj", name=f"ps_o{tt}_{mc}")
                p3state[(tl, mc)] = ps_t
                for hp in range(2):
                    mi = nc.tensor.matmul(ps_t[:],
                                          cw[:, hp, tl * 128:(tl + 1) * 128],
                                          wob_t[:, hp, mc * 512:(mc + 1) * 512],
                                          start=(hp == 0), stop=False)
                    MM_NAMES["p3"].add(mi.ins.name)
            f.mms = 2
            return f

        def half_b(tl, mc):
            def f():
                cw = ctxw[qt]
                tt = qt * 4 + tl
                ps_t = p3state.pop((tl, mc))
                for hp in range(2, 4):
                    mi = nc.tensor.matmul(ps_t[:],
                                          cw[:, hp, tl * 128:(tl + 1) * 128],
                                          wob_t[:, hp, mc * 512:(mc + 1) * 512],
                                          start=False, stop=(hp == 3))
                    MM_NAMES["p3"].add(mi.ins.name)
                y_t = py.tile([128, 512], BF, tag="y", name=f"y{tt}_{mc}")
                if Y_COPY:
                    nc.vector.tensor_copy(y_t[:], ps_t[:])
                else:
                    nc.scalar.copy(y_t[:], ps_t[:])
                nc.sync.dma_start(
                    y_d[tt * 128:(tt + 1) * 128, mc * 512:(mc + 1) * 512],
                    y_t[:])
            f.mms = 2
            return f

        out = []
        for tl in range(4):
            for mc in range(2):
                out.append(half_a(tl, mc))
                out.append(half_b(tl, mc))
        return out

    # ---- attention window ----
    def window(qt, work):
        if not INTERLEAVE:
            for f in work:
                f()
            work = []
        klim = k_needed[qt]          # in k128 units
        q0 = qt * 512
        n_units = 4 * klim
        cw = pcw.tile([128, 4, 512], BF, tag="cw", name=f"cw{qt}")
        ctxw[qt] = cw
        done = 0
        wi = 0
        total_mms = sum(getattr(f, "mms", 1) for f in work) or 1
        emitted = 0
        qw = qwin[qt]
        for hp in range(4):
            av = pav.tile([65, 2, 512], F32, tag="av")
            for kk in range(klim):
                straddle = (qt, kk) in mixed
                qoff = max(0, kk * 128 - q0) if straddle else 0
                moff = 128 if qoff >= 128 else 0
                s_t = psc.tile([128, 2, 512], F32, tag="sc")
                for j in range(2):
                    mi = nc.tensor.matmul(
                        s_t[:, j, moff:512],
                        KT[j * 64:(j + 1) * 64, hp, kk * 128:(kk + 1) * 128],
                        qw[j * 64:(j + 1) * 64, hp, moff:512],
                        start=True, stop=True, tile_position=(j * 64, 0))
                    MM_NAMES[f"sc{j}"].add(mi.ins.name)
                p_t = ppt.tile([128, 2, 512], BF, tag="pt")
                use_dve = (expctr[0] % 8) < EXP_DVE8
                expctr[0] += 1
                if use_dve:
                    p_i = p_t[:].bitcast(I16)
                    nc.vector.tensor_scalar(
                        p_i[:, :, qoff:512], s_t[:, :, qoff:512],
                        A16, B16, op0=AL.mult, op1=AL.add)
                else:
                    nc.scalar.activation(p_t[:, :, qoff:512],
                                         s_t[:, :, qoff:512],
                                         AF.Exp, scale=float(SCALE))
                if straddle:
                    # zero q < qoff+p: fully-masked prefix plus diagonal band,
                    # restricted to [0:qoff+128) (beyond is all-keep)
                    w = qoff + 128
                    nc.gpsimd.affine_select(
                        p_t[:, :, 0:w], p_t[:, :, 0:w],
                        pattern=[[0, 2], [1, w]],
                        compare_op=AL.is_ge, fill=0.0,
                        base=q0 - kk * 128, channel_multiplier=-1)
                for j in range(2):
                    mi = nc.tensor.matmul(av[:, j, :],
                                          V65[:, kk, 2 * hp + j, :],
                                          p_t[:, j, :],
                                          start=(kk == 0), stop=(kk == klim - 1))
                    MM_NAMES["av"].add(mi.ins.name)
                done += 1
                if INTERLEAVE:
                    target = done * total_mms / n_units
                    while wi < len(work) and emitted < target:
                        emitted += getattr(work[wi], "mms", 1)
                        work[wi]()
                        wi += 1
            # ---- normalize: drain av psum fast, then ctx*(1/l) with slack ----
            avc = pnrm.tile([128, 512], F32, tag="avc")
            nc.vector.tensor_copy(avc[0:64, :], av[0:64, 0, :])
            nc.scalar.copy(avc[64:128, :], av[0:64, 1, :])
            L2 = pnrm.tile([1, 2, 512], F32, tag="lin")
            nc.scalar.copy(L2[:], av[64:65, :, :])
            lrec = pnrm.tile([1, 2, 512], F32, tag="lrec")
            nc.vector.reciprocal_approx_fast(lrec[:], L2[:])
            rec_bc = pnrm.tile([128, 2, 512], F32, tag="rbc")
            nc.gpsimd.partition_broadcast(rec_bc[:], lrec[0:1, :, :])
            for j in range(2):
                nc.vector.tensor_tensor(
                    cw[j * 64:(j + 1) * 64, hp, :],
                    avc[j * 64:(j + 1) * 64, :],
                    rec_bc[j * 64:(j + 1) * 64, j, :], AL.mult)
        while wi < len(work):
            work[wi]()
            wi += 1

    # ---- prologue ----
    for f in qk_chunks(0):
        f()
    nc.sync.dma_start(wvb_t[:], wvb_d)
    nc.sync.dma_start(bv_t[:], bv_d)
    nc.sync.dma_start(tril_t[:], tril_d)
    nc.sync.dma_start(wob_t[:], wob_d)
    for f in v_chunks(0):
        f()

    for qt in range(NQT):
        work = []
        if qt + 1 < NQT:
            work += qk_chunks(qt + 1)
            work += v_chunks(qt + 1)
        if qt == 2:
            work += p3_chunks(0)
        elif qt == 3:
            work += p3_chunks(1)
            work += p3_chunks(2)
        window(qt, work)
    for f in p3_chunks(NQT - 1):
        f()

    for cm in (pav_cm, psc_cm, pp_cm, py_cm, pnrm_cm, ppt_cm, pcw_cm, pq_cm,
               px_cm, pers_cm):
        cm.__exit__(None, None, None)


def _prep_inputs_v2(x_q, x_k, x_v, wq, wk, wv, bq, bk, bv, wo):
    f32 = np.float32
    bf16 = ml_dtypes.bfloat16
    trilb = np.triu(np.ones((128, 128), f32)).astype(bf16)
    in_maps = []

    def xb_of(x):
        # [128, 8, S]: xb[p, dk, t] = x[t, dk*128 + p]
        xr = np.asarray(x, f32).T.reshape(8, 128, S)
        return np.ascontiguousarray(xr.transpose(1, 0, 2)).astype(bf16)

    def wb_of(w, hs):
        wt = np.asarray(w[hs], f32).transpose(2, 0, 1).reshape(D, EH)
        return np.ascontiguousarray(
            wt.reshape(8, 128, EH).transpose(1, 0, 2)).astype(bf16)

    xq_c = {}
    for core in range(NC):
        b, g = divmod(core, G)
        hs = slice(g * HPC, (g + 1) * HPC)
        if b not in xq_c:
            xq_c[b] = (xb_of(x_q[b]), xb_of(x_k[b]), xb_of(x_v[b]))
        xqb, xkb, xvb = xq_c[b]
        im = {
            "xqb": xqb, "xkb": xkb, "xvb": xvb,
            "wqb": wb_of(wq, hs),
            "wkb": wb_of(wk, hs),
            "wvb": wb_of(wv, hs),
            "trilb": trilb,
        }
        woT = np.asarray(wo[:, g * EH:(g + 1) * EH], f32).T   # [EH, D]
        im["wob"] = np.ascontiguousarray(
            woT.reshape(4, 128, D).transpose(1, 0, 2)).astype(bf16)
        for name, bb in (("bq", bq), ("bk", bk)):
            flat = np.asarray(bb[hs], f32).reshape(EH)
            im[name] = np.ascontiguousarray(flat.reshape(4, 128).T)
        im["bv"] = np.broadcast_to(
            np.asarray(bv[hs], f32).reshape(1, HPC, DH), (128, HPC, DH)).copy()
        in_maps.append(im)
    return in_maps


# ======================================================================
# dense-mask fallback: original fp32r kernel
# ======================================================================

def _build_dense(k_needed, k_full, mixed):
    nc = bacc.Bacc("TRN2", target_bir_lowering=False, debug=False, num_devices=NC)

    xqT_d = nc.dram_tensor("xqT", [D, S], F32R, kind="ExternalInput").ap()
    xkT_d = nc.dram_tensor("xkT", [D, S], F32R, kind="ExternalInput").ap()
    xvT_d = nc.dram_tensor("xvT", [D, S], F32R, kind="ExternalInput").ap()
    wqT_d = nc.dram_tensor("wqT", [128, 8, EH], F32R, kind="ExternalInput").ap()
    wkT_d = nc.dram_tensor("wkT", [128, 8, EH], F32R, kind="ExternalInput").ap()
    wvT_d = nc.dram_tensor("wvT", [128, 8, EH], F32R, kind="ExternalInput").ap()
    bq_d = nc.dram_tensor("bq", [128, 4], F32, kind="ExternalInput").ap()
    bk_d = nc.dram_tensor("bk", [128, 4], F32, kind="ExternalInput").ap()
    bv_d = nc.dram_tensor("bv", [128, EH], F32, kind="ExternalInput").ap()
    woT_d = nc.dram_tensor("woT", [128, 4, D], F32R, kind="ExternalInput").ap()
    ones_d = nc.dram_tensor("ones1", [128, 1], F32R, kind="ExternalInput").ap()
    mT_d = nc.dram_tensor("maskT", [S, S], F32R, kind="ExternalInput").ap()
    mT_v = mT_d.rearrange("(kt p) q -> p kt q", p=128)
    y_d = nc.dram_tensor("y", [S, D], F32, kind="ExternalOutput").ap()

    xq_v = xqT_d.rearrange("(dk p) t -> p dk t", p=128)
    xk_v = xkT_d.rearrange("(dk p) t -> p dk t", p=128)
    xv_v = xvT_d.rearrange("(dk p) t -> p dk t", p=128)

    with tile.TileContext(nc) as tc:
        with nc.allow_low_precision(reason="fp32r storage has fp32 width"):
            _body_dense(nc, tc, k_needed, mixed,
                        xq_v, xk_v, xv_v, wqT_d, wkT_d, wvT_d,
                        bq_d, bk_d, bv_d, woT_d, ones_d, mT_v, y_d)
    nc.compile()
    return nc


def _body_dense(nc, tc, k_needed, mixed,
                xq_v, xk_v, xv_v, wqT_d, wkT_d, wvT_d,
                bq_d, bk_d, bv_d, woT_d, ones_d, mT_v, y_d):
    pers_cm = tc.tile_pool(name="pers", bufs=1)
    pers = pers_cm.__enter__()
    KT = pers.tile([128, 4, S], F32R)
    V65 = pers.tile([128, NKT, HPC, 65], F32R)
    wo_t = pers.tile([128, 4, D], F32R)
    wv_t = pers.tile([128, 8, EH], F32R)
    bq_t = pers.tile([128, 4], F32)
    bk_t = pers.tile([128, 4], F32)
    bv_t = pers.tile([128, EH], F32)
    ones_t = pers.tile([128, 1], F32R)
    nc.sync.dma_start(bq_t[:], bq_d)
    nc.sync.dma_start(bk_t[:], bk_d)
    nc.sync.dma_start(bv_t[:], bv_d)
    nc.sync.dma_start(ones_t[:], ones_d)
    nc.sync.dma_start(wv_t[:], wvT_d)
    nc.vector.tensor_copy(V65[:, :, :, 64:65],
                          ones_t[:, 0:1].to_broadcast([128, NKT, HPC, 1]))

    pw_cm = tc.tile_pool(name="pw", bufs=2)
    pw = pw_cm.__enter__()
    px_cm = tc.tile_pool(name="px", bufs=3)
    px = px_cm.__enter__()
    pq_cm = tc.tile_pool(name="pq", bufs=2)
    pq = pq_cm.__enter__()
    pcw_cm = tc.tile_pool(name="pcw", bufs=3)
    pcw = pcw_cm.__enter__()
    ppt_cm = tc.tile_pool(name="ppt", bufs=2)
    ppt = ppt_cm.__enter__()
    pnrm_cm = tc.tile_pool(name="pnrm", bufs=1)
    pnrm = pnrm_cm.__enter__()
    py_cm = tc.tile_pool(name="py", bufs=2)
    py = py_cm.__enter__()
    pp_cm = tc.tile_pool(name="pp", bufs=2, space="PSUM")
    pp = pp_cm.__enter__()
    psc_cm = tc.tile_pool(name="psc", bufs=2, space="PSUM")
    psc = psc_cm.__enter__()
    pav_cm = tc.tile_pool(name="pav", bufs=1, space="PSUM")
    pav = pav_cm.__enter__()

    qwin = {}
    ctxw = {}
    state = {}

    def qk_chunks(tq):
        def load(w_d, x_v, kind, half):
            def f():
                hs = slice(half * 4, half * 4 + 4)
                w_t = pw.tile([128, 4, EH], F32R, tag="w", name=f"w_{kind}{tq}{half}")
                nc.sync.dma_start(w_t[:], w_d[:, hs, :])
                x_t = px.tile([128, 4, 512], F32R, tag="x", name=f"x_{kind}{tq}{half}")
                nc.sync.dma_start(x_t[:], x_v[:, hs, tq * 512:(tq + 1) * 512])
                state[f"w{half}"], state[f"x{half}"] = w_t, x_t
                if kind == "q" and half == 0:
                    qwin[tq] = pq.tile([128, 4, 512], F32R, tag="qw", name=f"qw{tq}")
            return f

        def mmgroup(et, kind):
            def f():
                ps_t = pp.tile([128, 512], F32, tag="pj", name=f"ps_{kind}{tq}_{et}")
                for dk in range(8):
                    w_t = state[f"w{dk // 4}"]
                    x_t = state[f"x{dk // 4}"]
                    nc.tensor.matmul(ps_t[:],
                                     w_t[:, dk % 4, et * 128:(et + 1) * 128],
                                     x_t[:, dk % 4, :],
                                     start=(dk == 0), stop=(dk == 7))
                if kind == "q":
                    nc.vector.tensor_tensor(
                        qwin[tq][:, et, :], ps_t[:],
                        bq_t[:, et:et + 1].to_broadcast([128, 512]), AL.add)
                else:
                    nc.vector.tensor_tensor(
                        KT[:, et, tq * 512:(tq + 1) * 512], ps_t[:],
                        bk_t[:, et:et + 1].to_broadcast([128, 512]), AL.add)
            return f

        out = []
        for kind, w_d, x_v in (("q", wqT_d, xq_v), ("k", wkT_d, xk_v)):
            for half in (0, 1):
                g = load(w_d, x_v, kind, half)
                g.mms = 0
                out.append(g)
            for et in range(4):
                g = mmgroup(et, kind)
                g.mms = 8
                out.append(g)
        return out

    def v_chunks(tv):
        def load(half):
            def f():
                hs = slice(half * 4, half * 4 + 4)
                x_t = px.tile([128, 4, 512], F32R, tag="x", name=f"x_v{tv}{half}")
                nc.sync.dma_start(x_t[:], xv_v[:, hs, tv * 512:(tv + 1) * 512])
                state[f"x{half}"] = x_t
            return f

        def mmgroup(tl):
            def f():
                tt = tv * 4 + tl
                ps_t = pp.tile([128, 512], F32, tag="pj", name=f"ps_v{tt}")
                for dk in range(8):
                    x_t = state[f"x{dk // 4}"]
                    nc.tensor.matmul(ps_t[:],
                                     x_t[:, dk % 4, tl * 128:(tl + 1) * 128],
                                     wv_t[:, dk, :],
                                     start=(dk == 0), stop=(dk == 7))
                nc.vector.tensor_tensor(
                    V65[:, tt, :, 0:64],
                    ps_t.rearrange("p (h e) -> p h e", h=HPC),
                    bv_t.rearrange("p (h e) -> p h e", h=HPC), AL.add)
            return f

        out = []
        for half in (0, 1):
            g = load(half)
            g.mms = 0
            out.append(g)
        for tl in range(4):
            g = mmgroup(tl)
            g.mms = 8
            out.append(g)
        return out

    def p3_chunks(qt):
        p3state = {}

        def half_a(tl, mc):
            def f():
                cw = ctxw[qt]
                tt = qt * 4 + tl
                ps_t = pp.tile([128, 512], F32, tag="pj", name=f"ps_o{tt}_{mc}")
                p3state[(tl, mc)] = ps_t
                for hp in range(2):
                    nc.tensor.matmul(ps_t[:],
                                     cw[:, hp, tl * 128:(tl + 1) * 128],
                                     wo_t[:, hp, mc * 512:(mc + 1) * 512],
                                     start=(hp == 0), stop=False)
            f.mms = 2
            return f

        def half_b(tl, mc):
            def f():
                cw = ctxw[qt]
                tt = qt * 4 + tl
                ps_t = p3state.pop((tl, mc))
                for hp in range(2, 4):
                    nc.tensor.matmul(ps_t[:],
                                     cw[:, hp, tl * 128:(tl + 1) * 128],
                                     wo_t[:, hp, mc * 512:(mc + 1) * 512],
                                     start=False, stop=(hp == 3))
                y_t = py.tile([128, 512], F32, tag="y", name=f"y{tt}_{mc}")
                nc.vector.tensor_copy(y_t[:], ps_t[:])
                nc.sync.dma_start(
                    y_d[tt * 128:(tt + 1) * 128, mc * 512:(mc + 1) * 512],
                    y_t[:])
            f.mms = 2
            return f

        out = []
        for tl in range(4):
            for mc in range(2):
                out.append(half_a(tl, mc))
                out.append(half_b(tl, mc))
        return out

    def window(qt, work):
        if not INTERLEAVE:
            for f in work:
                f()
            work = []
        klim = k_needed[qt]
        q0 = qt * 512
        n_tiles = 4 * klim
        cw = pcw.tile([128, 4, 512], F32R, tag="cw", name=f"cw{qt}")
        ctxw[qt] = cw
        done = 0
        wi = 0
        total_mms = sum(getattr(f, "mms", 4) for f in work) or 1
        emitted = 0
        qw = qwin[qt]
        for hp in range(4):
            av0 = pav.tile([65, 512], F32, tag="av0")
            av1 = pav.tile([65, 512], F32, tag="av1")
            first = True
            for kk in range(klim):
                straddle = (qt, kk) in mixed
                s_t = psc.tile([128, 2, 512], F32, tag="sc")
                for j in range(2):
                    nc.tensor.matmul(
                        s_t[:, j, :],
                        KT[j * 64:(j + 1) * 64, hp, kk * 128:(kk + 1) * 128],
                        qw[j * 64:(j + 1) * 64, hp, :],
                        start=True, stop=True, tile_position=(j * 64, 0))
                p_t = ppt.tile([128, 2, 512], F32R, tag="pt")
                nc.scalar.activation(p_t[:], s_t[:], AF.Exp, scale=float(SCALE))
                if straddle:
                    sel_t = ppt.tile([128, 512], F32R, tag="sel")
                    nc.sync.dma_start(sel_t[:], mT_v[:, kk, q0:q0 + 512])
                    nc.vector.tensor_tensor(
                        p_t[:], p_t[:],
                        sel_t[:, None, :].to_broadcast([128, 2, 512]),
                        AL.mult)
                for j, av in ((0, av0), (1, av1)):
                    nc.tensor.matmul(av[:], V65[:, kk, 2 * hp + j, :],
                                     p_t[:, j, :],
                                     start=first, stop=(kk == klim - 1))
                first = False
                done += 1
                if INTERLEAVE:
                    target = done * total_mms / n_tiles
                    while wi < len(work) and emitted < target:
                        emitted += getattr(work[wi], "mms", 4)
                        work[wi]()
                        wi += 1
            avc = pnrm.tile([128, 512], F32, tag="avc", bufs=1)
            lin = pnrm.tile([1, 2, 512], F32, tag="lin")
            nc.vector.tensor_copy(avc[0:64, :], av0[0:64, :])
            nc.vector.tensor_copy(avc[64:128, :], av1[0:64, :])
            nc.vector.tensor_copy(lin[:, 0, :], av0[64:65, :])
            nc.vector.tensor_copy(lin[:, 1, :], av1[64:65, :])
            lrec = pnrm.tile([1, 2, 512], F32, tag="lrec")
            scr = pnrm.tile([1, 512], F32, tag="scr")
            nc.vector.reciprocal_approx_accurate(lrec[:, 0, :], lin[:, 0, :], scr[:])
            nc.vector.reciprocal_approx_accurate(lrec[:, 1, :], lin[:, 1, :], scr[:])
            rec_bc = pnrm.tile([128, 2, 512], F32, tag="rbc")
            nc.gpsimd.partition_broadcast(rec_bc[:], lrec[0:1, :, :])
            nc.vector.tensor_tensor(cw[0:64, hp, :],
                                    avc[0:64, :], rec_bc[0:64, 0, :], AL.mult)
            nc.vector.tensor_tensor(cw[64:128, hp, :],
                                    avc[64:128, :], rec_bc[64:128, 1, :], AL.mult)
        while wi < len(work):
            work[wi]()
            wi += 1

    for f in qk_chunks(0):
        f()
    for f in v_chunks(0):
        f()

    def wo_load():
        nc.sync.dma_start(wo_t[:], woT_d)
    wo_load.mms = 0

    for qt in range(NQT):
        work = []
        if qt == 0:
            work.append(wo_load)
        if qt + 1 < NQT:
            work += qk_chunks(qt + 1)
            work += v_chunks(qt + 1)
        if qt == 2:
            work += p3_chunks(0)
        elif qt == 3:
            work += p3_chunks(1)
            work += p3_chunks(2)
        window(qt, work)
    for f in p3_chunks(NQT - 1):
        f()

    for cm in (pav_cm, psc_cm, pp_cm, py_cm, pnrm_cm, ppt_cm, pcw_cm, pq_cm,
               px_cm, pw_cm, pers_cm):
        cm.__exit__(None, None, None)


def _prep_inputs_dense(x_q, x_k, x_v, mask, wq, wk, wv, bq, bk, bv, wo):
    f32 = np.float32
    in_maps = []
    ones1 = np.ones((128, 1), f32)
    for core in range(NC):
        b, g = divmod(core, G)
        hs = slice(g * HPC, (g + 1) * HPC)
        im = {
            "xqT": np.ascontiguousarray(np.asarray(x_q[b], f32).T),
            "xkT": np.ascontiguousarray(np.asarray(x_k[b], f32).T),
            "xvT": np.ascontiguousarray(np.asarray(x_v[b], f32).T),
            "ones1": ones1,
        }
        for name, w in (("wqT", wq), ("wkT", wk), ("wvT", wv)):
            wt = np.asarray(w[hs], f32).transpose(2, 0, 1).reshape(D, EH)
            im[name] = np.ascontiguousarray(wt.reshape(8, 128, EH))\
                .transpose(1, 0, 2).copy()
        for name, bb in (("bq", bq), ("bk", bk)):
            flat = np.asarray(bb[hs], f32).reshape(EH)
            im[name] = np.ascontiguousarray(flat.reshape(4, 128).T)
        im["bv"] = np.broadcast_to(np.asarray(bv[hs], f32).reshape(1, EH),
                                   (128, EH)).copy()
        woT = np.asarray(wo[:, g * EH:(g + 1) * EH], f32).T
        im["woT"] = np.ascontiguousarray(woT.reshape(4, 128, D))\
            .transpose(1, 0, 2).copy()
        im["maskT"] = np.ascontiguousarray(np.asarray(mask[b], f32).T)
        in_maps.append(im)
    return in_maps


# ======================================================================
# mask analysis + dispatch
# ======================================================================

def _analyze_mask(mask):
    m = np.asarray(mask)
    iota = np.arange(S)
    n = m.sum(axis=2)
    causal = bool((n == iota[None, :] + 1).all()) and \
        bool((m == (iota[None, None, :] < n[..., None])).all())
    allones = bool((m == 1).all())

    k_needed, k_full, mixed = [], [], set()
    if allones:
        mode = "affine"
        k_needed = [NKT] * NQT
        k_full = [NKT] * NQT
    elif causal:
        mode = "affine"
        for qt in range(NQT):
            k_needed.append(4 * qt + 4)
            k_full.append(4 * qt)
            for kk in range(4 * qt, 4 * qt + 4):
                mixed.add((qt, kk))
    else:
        mode = "dense"
        for qt in range(NQT):
            sl = m[:, qt * 512:(qt + 1) * 512, :]
            need = 0
            full = NKT
            for kk in range(NKT):
                blk = sl[:, :, kk * 128:(kk + 1) * 128]
                if blk.any():
                    need = kk + 1
                if not blk.all():
                    full = min(full, kk)
            need = max(need, 1)
            if mode == "dense":
                # keep k-tile count even for pairing safety (harmless extra)
                pass
            full = min(full, need)
            k_needed.append(need)
            k_full.append(full)
            for kk in range(full, need):
                blk = sl[:, :, kk * 128:(kk + 1) * 128]
                if not blk.all():
                    mixed.add((qt, kk))
    return mode, tuple(k_needed), tuple(k_full), frozenset(mixed)


def _run(x_q, x_k, x_v, mask, wq, wk, wv, bq, bk, bv, wo, bo,
         trace=False, trace_cores=None):
    mode, k_needed, k_full, mixed = _analyze_mask(mask)
    key = (mode, k_needed, k_full, mixed)
    if key not in _cache:
        if mode == "affine":
            _cache[key] = _build_v2(k_needed, mixed)
        else:
            _cache[key] = _build_dense(k_needed, k_full, mixed)
    nc = _cache[key]
    if mode == "affine":
        in_maps = _prep_inputs_v2(x_q, x_k, x_v, wq, wk, wv, bq, bk, bv, wo)
    else:
        in_maps = _prep_inputs_dense(x_q, x_k, x_v, mask, wq, wk, wv,
                                     bq, bk, bv, wo)
    res = run_bass_kernel_spmd(nc, in_maps, core_ids=list(range(NC)),
                               trace=trace, trace_cores=trace_cores)
    bo = np.asarray(bo, np.float32)
    out = np.empty((B, S, D), np.float32)
    for b in range(B):
        out[b] = (np.asarray(res.results[2 * b]["y"], np.float32)
                  + np.asarray(res.results[2 * b + 1]["y"], np.float32) + bo)
    return out, res


def kernel(x_q, x_k, x_v, mask, wq, wk, wv, bq, bk, bv, wo, bo):
    out, _ = _run(x_q, x_k, x_v, mask, wq, wk, wv, bq, bk, bv, wo, bo)
    return out
